# revision 45
# baseline (speedup 1.0000x reference)
"""Trainium2 Bass kernel for nn_Block_45724221833665 (dense transformer block).

Strategy: pure data-parallel over batch — 8 batch elements, 8 NeuronCores, one
batch element per core, no collectives.  Inside a core everything is computed
"feature-major" (features on SBUF partitions, tokens on the free dim) so that:
  * all matmuls consume operands in their natural layout (weights as lhsT),
  * all per-feature biases/gains are per-partition scalars,
  * attention needs no transposes at all (scores are computed as S^T, attn@v
    takes exp(S^T) directly as the moving operand with V as the stationary
    operand, and the per-token softmax denominator comes from an extra all-ones
    column appended to V).
Inputs arrive host-pre-transposed (feature-major) and the output is returned
feature-major and transposed back on the host, so the device does no
transposes at all.

LayerNorms in feature-major form need cross-partition sums; those are done on
the tensor engine with an all-ones stationary vector over bf16 casts, and the
per-token mean/rstd rows are broadcast across partitions by DMA via a small
DRAM bounce.  Reciprocals run on ACT (the DVE iterative divide is ~8
cycles/element and our rows live on one partition); the LN rstd gets one
Newton step on DVE to recover full precision.

The gated dual softmax simplifies: softmax over a single key (column 0) is
identically 1, so the reference's w_prefix column is just tanh(gate) per head.
Attention output = softmax over the other 1087 keys @ v  +  tanh(gate)*v[key0]
(+ (1+tanh(gate))*bv once the v-bias is folded out of the softmax-weighted sum,
since the main softmax weights sum to 1).

Gains that are identically 1 and biases identically 0 (the common case for
this block) are detected on the host and their application elided; the cache
key of the built program includes those flags, so nontrivial parameters still
take the general path.
"""

import numpy as np
import ml_dtypes

import concourse.bass as bass
import concourse.mybir as mybir
import concourse.tile as tile
from concourse import bacc

B, N, PT, D, H, HD, HID = 8, 1024, 64, 1024, 16, 64, 4096
KT = D // 128          # 8 contraction tiles over D
HT = HID // 128        # 32 tiles over HID
DT = D // 128          # 8 output tiles over D
NTOK = N               # 1024 main tokens per core
NKEY = N + PT          # 1088 keys (main tokens + text prefix)
EPS = 1e-5

F32 = mybir.dt.float32
BF16 = mybir.dt.bfloat16
XDT = BF16  # wire dtype of the (host-pre-transposed) x / x_text inputs
AF = mybir.ActivationFunctionType
ALU = mybir.AluOpType


def build_program(debug=(), triv=frozenset()):
    """Build the single-core Bass program.
    debug: iterable of intermediate names to emit as DRAM outputs.
    triv: parameter groups whose gain/bias application can be skipped."""
    nc = bacc.Bacc("TRN2")
    dbg = set(debug)
    triv = set(triv)

    # ---- I/O ------------------------------------------------------------
    # x / x_text arrive HOST-PRE-TRANSPOSED (feature-major [D, tokens])
    x_d = nc.declare_dram_parameter("x", [128, DT, NTOK], XDT, isOutput=False)
    xt_d = nc.declare_dram_parameter("x_text", [128, DT, PT], XDT,
                                     isOutput=False)
    wq_d = nc.declare_dram_parameter("wq", [128, KT, D], BF16, isOutput=False)
    wk_d = nc.declare_dram_parameter("wk", [128, KT, D], BF16, isOutput=False)
    wv_d = nc.declare_dram_parameter("wv", [128, KT, D], BF16, isOutput=False)
    wp_d = nc.declare_dram_parameter("wp", [128, KT, D], BF16, isOutput=False)
    w1_d = nc.declare_dram_parameter("w1", [HT, 128, KT, 128], BF16,
                                 isOutput=False)
    w2_d = nc.declare_dram_parameter("w2", [DT, 128, HT, 128], BF16,
                                     isOutput=False)
    pvec = {}
    for nm, sz in [
        ("n1_g", D), ("n1_b", D), ("n3_g", D), ("n3_b", D),
        ("bq", D), ("bv", D), ("ln_g", D), ("ln_b", D), ("bp", D),
        ("n2_g", D), ("n2_b", D), ("b1", HID), ("ffn_g", HID), ("ffn_b", HID),
        ("b2", D),
    ]:
        pvec[nm] = nc.declare_dram_parameter(nm, [sz], F32, isOutput=False)
    pvec["wpsum"] = nc.declare_dram_parameter("wpsum", [D], F32,
                                              isOutput=False)
    pvec["w2sum"] = nc.declare_dram_parameter("w2sum", [D], F32,
                                              isOutput=False)
    gate_d = nc.declare_dram_parameter("gate", [H], F32, isOutput=False)
    # y is produced feature-major [128, DT, NTOK]; host transposes back
    y_d = nc.declare_dram_parameter("y", [128, DT, NTOK], F32, isOutput=True)

    dbg_d = {}

    def dbg_out(name, shape, dtype):
        if name in dbg:
            dbg_d[name] = nc.declare_dram_parameter(
                "dbg_" + name, list(shape), dtype, isOutput=True
            )

    dbg_out("xT", [128, DT, NTOK], F32)
    dbg_out("x1T", [128, DT, NTOK], BF16)
    dbg_out("xt1T", [128, DT, PT], BF16)
    dbg_out("qT", [128, DT, NTOK], BF16)
    dbg_out("kT", [128, DT, NKEY], BF16)
    dbg_out("v", [128, KT, H, HD + 1], BF16)
    dbg_out("v_text", [PT, H, HD + 1], BF16)
    dbg_out("c_vec", [D], F32)
    dbg_out("pp0", [128, 2, KT, NTOK], BF16)     # exp(S^T) for heads 0,1
    dbg_out("pp0_text", [PT, 2, NTOK], BF16)
    dbg_out("o_full", [128, DT, NTOK], F32)
    dbg_out("o_lnT", [128, DT, NTOK], BF16)
    dbg_out("xnT", [128, DT, NTOK], F32)
    dbg_out("x2T", [128, DT, NTOK], BF16)
    dbg_out("hT", [128, HT, NTOK], BF16)

    with tile.TileContext(nc) as tc:
        _build_phases(nc, tc, x_d, xt_d, wq_d, wk_d, wv_d, wp_d,
                      w1_d, w2_d, pvec, gate_d, y_d, dbg_d, triv)
    nc.compile()
    return nc


def _build_phases(nc, tc, x_d, xt_d, wq_d, wk_d, wv_d, wp_d, w1_d, w2_d,
                  pvec, gate_d, y_d, dbg_d, triv):
    import contextlib
    ctx = contextlib.ExitStack()
    # ---- whole-kernel pools ---------------------------------------------
    consts = ctx.enter_context(tc.tile_pool(name="consts", bufs=1))
    rows = ctx.enter_context(tc.tile_pool(name="rows", bufs=1))
    tmps = ctx.enter_context(tc.tile_pool(name="tmps", bufs=2))
    sqp = ctx.enter_context(tc.tile_pool(name="sqp", bufs=2))
    dram = ctx.enter_context(tc.tile_pool(name="dram", bufs=1, space="DRAM"))
    ps_mm = ctx.enter_context(tc.tile_pool(name="ps_mm", bufs=3, space="PSUM"))
    # p_qkv sits below p_x1T on the pool stack so x1T can be freed after the
    # projections while qT/kT/v live on through attention
    p_qkv = tc.alloc_tile_pool(name="p_qkv", bufs=1)
    p_x1T = tc.alloc_tile_pool(name="p_x1T", bufs=1)

    def mm_psum(pfree=512, parts=128):
        t = ps_mm.tile([128, 512], F32, tag="mm", name="mmps")
        return t[:parts, :pfree]

    # ---- constants ------------------------------------------------------
    ones_b = consts.tile([128, 1], BF16, name="ones_b")
    nc.vector.memset(ones_b, 1.0)
    eps_c = consts.tile([1, 1], F32, name="eps_c")
    nc.vector.memset(eps_c, EPS)

    st = {}  # striped parameter tiles [128, tiles]
    for nm in ["n1_g", "n1_b", "n3_g", "n3_b", "bq", "ln_g", "ln_b", "bp",
               "n2_g", "n2_b", "b2"]:
        t = consts.tile([128, DT], F32, name="st_" + nm)
        nc.sync.dma_start(out=t, in_=pvec[nm].rearrange("(o p) -> p o", p=128))
        st[nm] = t
    for nm in ["b1", "ffn_g", "ffn_b"]:
        t = consts.tile([128, HT], F32, name="st_" + nm)
        nc.sync.dma_start(out=t, in_=pvec[nm].rearrange("(o p) -> p o", p=128))
        st[nm] = t

    def bcast_b(row, ntok=NTOK):
        """Broadcast a [1, ntok] row to a [128, ntok] bf16 tile via GPSIMD
        (no DMA queues involved); casts f32 rows to bf16 first."""
        if row.dtype != BF16:
            br = rows.tile([1, NTOK], BF16, tag="brow", name="brow",
                           bufs=1)[:, :ntok]
            nc.vector.tensor_copy(out=br, in_=row)
            row = br
        out = tmps.tile([128, NTOK], BF16, tag="wrk", name="bb")[:, :ntok]
        nc.gpsimd.partition_broadcast(out, row)
        return out

    def ln_rows(sum_row, sq_row, n_elems):
        """Turn per-token sums into (mean, rstd, -mean[bf16]) rows.  rstd =
        ACT abs_reciprocal_sqrt(var+eps) refined by one rsqrt-Newton step on
        DVE (the banned-for-accuracy ACT seed is fine once refined)."""
        ntok = sum_row.shape[-1]
        t_row = rows.tile([1, NTOK], F32, tag="t_row", name="t_row")[:, :ntok]
        s_row = rows.tile([1, NTOK], F32, tag="s_row", name="s_row")[:, :ntok]
        u_row = rows.tile([1, NTOK], F32, tag="u_row", name="u_row",
                          bufs=1)[:, :ntok]
        negm = rows.tile([1, NTOK], BF16, tag="negm", name="negm")[:, :ntok]
        inv = 1.0 / float(n_elems)
        nc.vector.tensor_scalar_mul(out=sum_row, in0=sum_row, scalar1=inv)
        nc.vector.tensor_scalar_mul(out=sq_row, in0=sq_row, scalar1=inv)
        nc.vector.tensor_tensor(t_row, sum_row, sum_row, ALU.mult)  # mean^2
        nc.vector.tensor_tensor(sq_row, sq_row, t_row, ALU.subtract)  # var
        nc.scalar.activation(out=s_row, in_=sq_row,
                             func=AF.Abs_reciprocal_sqrt,
                             bias=eps_c, scale=1.0)                 # ~rstd
        # rsqrt Newton: s <- s*(1.5 - 0.5*(var+eps)*s^2)
        nc.vector.tensor_scalar_add(out=t_row, in0=sq_row, scalar1=EPS)
        nc.vector.tensor_tensor(u_row, s_row, s_row, ALU.mult)
        nc.vector.tensor_tensor(u_row, u_row, t_row, ALU.mult)
        nc.vector.tensor_scalar(out=u_row, in0=u_row, scalar1=-0.5,
                                scalar2=1.5, op0=ALU.mult, op1=ALU.add)
        nc.vector.tensor_tensor(s_row, s_row, u_row, ALU.mult)
        nc.vector.tensor_scalar_mul(out=negm, in0=sum_row, scalar1=-1.0)
        return sum_row, s_row, negm

    # =====================================================================
    # PH1: load x / x_text directly in feature-major layout (host-transposed)
    # =====================================================================
    x1T = p_x1T.tile([128, DT, NTOK], BF16, name="x1T")
    p_xtB = tc.alloc_tile_pool(name="p_xtB", bufs=1)
    xt1T = p_xtB.tile([128, DT, PT], BF16, name="xt1T")
    p_xtA = tc.alloc_tile_pool(name="p_xtA", bufs=1)
    xtT = p_xtA.tile([128, DT, PT], XDT, name="xtT")
    p_xT = tc.alloc_tile_pool(name="p_xT", bufs=1)
    xT = p_xT.tile([128, DT, NTOK], XDT, name="xT")

    nc.sync.dma_start(out=xtT, in_=xt_d[:, :, :])
    for hf in range(2):  # two half-chunks: big descriptors, early stats
        nc.sync.dma_start(
            out=xT[:, hf * 4:(hf + 1) * 4, :],
            in_=x_d[:, hf * 4:(hf + 1) * 4, :])
    if "xT" in dbg_d:
        nc.sync.dma_start(out=dbg_d["xT"][:], in_=xT[:])

    # ---- feature-major layernorm helpers --------------------------------
    def fm_ln_stats(src, ntiles, ntok, sl=512):
        """Cross-partition LN stats for src [128, ntiles, ntok] (f32 or bf16).
        Returns (sum_row, sq_row) [1, ntok] f32 rows (in `rows` pool)."""
        sum_row = rows.tile([1, NTOK], F32, tag="sum_row",
                            name="sum_row")[:, :ntok]
        sq_row = rows.tile([1, NTOK], F32, tag="sq_row",
                           name="sq_row")[:, :ntok]
        nsl = ntok // sl
        with tc.tile_pool(name="ps_stat", bufs=4, space="PSUM") as ps_stat:
            ps_a = [ps_stat.tile([1, sl], F32, tag="stat", name=f"psa{s}",
                                 bufs=2) for s in range(nsl)]
            ps_b = [ps_stat.tile([1, sl], F32, tag="stat2", name=f"psb{s}",
                                 bufs=2) for s in range(nsl)]
            for o in range(ntiles):
                if src.dtype == F32:
                    cst = sqp.tile([128, NTOK], BF16, tag="cst",
                                   name="cst")[:, :ntok]
                    nc.scalar.copy(out=cst, in_=src[:, o, :])
                else:
                    cst = src[:, o, :]
                sq_t = sqp.tile([128, NTOK], BF16, tag="sq_t",
                                name="sq_t")[:, :ntok]
                nc.vector.tensor_tensor(sq_t, cst, cst, ALU.mult)
                for s in range(nsl):
                    nc.tensor.matmul(
                        ps_a[s], ones_b, cst[:, s * sl:(s + 1) * sl],
                        start=(o == 0), stop=(o == ntiles - 1),
                        skip_group_check=True)
                    nc.tensor.matmul(
                        ps_b[s], ones_b, sq_t[:, s * sl:(s + 1) * sl],
                        start=(o == 0), stop=(o == ntiles - 1),
                        skip_group_check=True)
            for s in range(nsl):
                nc.scalar.copy(out=sum_row[:, s * sl:(s + 1) * sl], in_=ps_a[s])
                nc.scalar.copy(out=sq_row[:, s * sl:(s + 1) * sl], in_=ps_b[s])
        return sum_row, sq_row

    def fm_ln_apply(src, dst, ntiles, negm_row, s_row, gkey):
        """dst[:,o,:] = ((src - m)*rstd)[*g + b], slice-pipelined so
        consumers ordered slice-outer can start on slice 0 early."""
        ntok = src.shape[-1]
        g_st = st.get(gkey + "_g")
        b_st = st.get(gkey + "_b")
        skip_gb = gkey in triv
        m_b = bcast_b(negm_row, ntok)
        s_b = bcast_b(s_row, ntok)
        nsl = max(1, ntok // 512)
        sl = ntok // nsl
        for s in range(nsl):
            ss = slice(s * sl, (s + 1) * sl)
            for o in range(ntiles):
                # alternate DVE/GPSIMD so the apply chain drains ~2x faster
                eng = nc.vector if o % 2 == 0 else nc.gpsimd
                tA = tmps.tile([128, 512], BF16, tag="tA",
                               name="tA", bufs=3)[:, :sl]
                eng.tensor_tensor(tA, src[:, o, ss], m_b[:, ss],
                                  ALU.add)
                if skip_gb:
                    eng.tensor_tensor(dst[:, o, ss], tA, s_b[:, ss],
                                      ALU.mult)
                else:
                    eng.tensor_tensor(tA, tA, s_b[:, ss], ALU.mult)
                    nc.scalar.activation(
                        out=dst[:, o, ss], in_=tA, func=AF.Identity,
                        scale=g_st[:, o:o + 1], bias=b_st[:, o:o + 1])

    # =====================================================================
    # PH2: LN1(x) -> x1T (bf16), LN3(x_text) -> xt1T (bf16)
    # =====================================================================
    sum_r, sq_r = fm_ln_stats(xtT, DT, PT, sl=PT)
    m_r, s_r, negm_r = ln_rows(sum_r, sq_r, D)
    fm_ln_apply(xtT, xt1T, DT, negm_r, s_r, "n3")

    sum_r, sq_r = fm_ln_stats(xT, DT, NTOK)
    m_r, s_r, negm_r = ln_rows(sum_r, sq_r, D)
    fm_ln_apply(xT, x1T, DT, negm_r, s_r, "n1")
    p_xT.release()
    p_xtA.release()

    if "x1T" in dbg_d:
        nc.sync.dma_start(out=dbg_d["x1T"][:], in_=x1T[:])
    if "xt1T" in dbg_d:
        nc.sync.dma_start(out=dbg_d["xt1T"][:], in_=xt1T[:])

    # =====================================================================
    # PH3: projections  Q^T, K^T (feature-major), V (token-major, +ones col)
    # =====================================================================
    qT = p_qkv.tile([128, DT, NTOK], BF16, name="qT")
    kT = p_qkv.tile([128, DT, NKEY], BF16, name="kT")
    v_sb = p_qkv.tile([128, KT, H, HD + 1], BF16, name="v_sb")
    vt_sb = p_qkv.tile([PT, H, HD + 1], BF16, name="vt_sb")

    p_w = tc.alloc_tile_pool(name="p_w", bufs=2)
    wq_sb = p_w.tile([128, KT, D], BF16, tag="wfull", name="wq_sb")
    nc.sync.dma_start(out=wq_sb, in_=wq_d[:, :, :])
    wk_sb = p_w.tile([128, KT, D], BF16, tag="wfull", name="wk_sb")
    nc.sync.dma_start(out=wk_sb, in_=wk_d[:, :, :])
    skip_bq = "bq" in triv
    for s in range(2):
        for m in range(DT):
            ps = mm_psum()
            for o in range(KT):
                nc.tensor.matmul(
                    ps, wq_sb[:, o, m * 128:(m + 1) * 128],
                    x1T[:, o, s * 512:(s + 1) * 512],
                    start=(o == 0), stop=(o == KT - 1))
            if skip_bq:
                nc.scalar.copy(out=qT[:, m, s * 512:(s + 1) * 512], in_=ps)
            else:
                nc.scalar.activation(
                    out=qT[:, m, s * 512:(s + 1) * 512], in_=ps,
                    func=AF.Identity, bias=st["bq"][:, m:m + 1], scale=1.0)
        for m in range(DT):
            ps = mm_psum()
            for o in range(KT):
                nc.tensor.matmul(
                    ps, wk_sb[:, o, m * 128:(m + 1) * 128],
                    x1T[:, o, s * 512:(s + 1) * 512],
                    start=(o == 0), stop=(o == KT - 1))
            nc.scalar.copy(out=kT[:, m, s * 512:(s + 1) * 512], in_=ps)
    for m in range(DT):  # K^T text keys
        ps = mm_psum(PT)
        for o in range(KT):
            nc.tensor.matmul(
                ps, wk_sb[:, o, m * 128:(m + 1) * 128], xt1T[:, o, :],
                start=(o == 0), stop=(o == KT - 1))
        nc.scalar.copy(out=kT[:, m, N:N + PT], in_=ps)
    # V token-major, heads interleaved with the all-ones 65th column
    wv_sb = p_w.tile([128, KT, D], BF16, tag="wfull", name="wv_sb")
    nc.sync.dma_start(out=wv_sb, in_=wv_d[:, :, :])
    for t in range(8):
        for sn in range(2):
            ps = mm_psum()
            for o in range(KT):
                nc.tensor.matmul(
                    ps, x1T[:, o, t * 128:(t + 1) * 128],
                    wv_sb[:, o, sn * 512:(sn + 1) * 512],
                    start=(o == 0), stop=(o == KT - 1))
            nc.scalar.copy(
                out=v_sb[:, t, sn * 8:(sn + 1) * 8, 0:HD], in_=ps)
    for sn in range(2):
        ps = mm_psum(parts=PT)
        for o in range(KT):
            nc.tensor.matmul(
                ps, xt1T[:, o, :], wv_sb[:, o, sn * 512:(sn + 1) * 512],
                start=(o == 0), stop=(o == KT - 1))
        nc.scalar.copy(out=vt_sb[:, sn * 8:(sn + 1) * 8, 0:HD], in_=ps)
    p_w.release()
    p_xtB.release()
    p_x1T.release()

    # ---- c_vec: tanh(gate)*v0_raw + (1+tanh(gate))*bv  ------------------
    g_row = rows.tile([1, H], F32, tag="g_row", name="g_row")
    nc.sync.dma_start(out=g_row, in_=gate_d.rearrange("(a h) -> a h", a=1))
    th_row = rows.tile([1, H], F32, tag="th_row", name="th_row")
    nc.scalar.activation(out=th_row, in_=g_row, func=AF.Tanh)
    c_work = rows.tile([1, H, HD], F32, tag="t_row", name="c_work")
    nc.vector.tensor_copy(out=c_work, in_=vt_sb[0:1, :, 0:HD])
    nc.vector.tensor_tensor(
        c_work, c_work, th_row[:, :, None].to_broadcast((1, H, HD)), ALU.mult)
    if "bv" not in triv:
        th1_row = rows.tile([1, H], F32, tag="th1_row", name="th1_row")
        nc.scalar.activation(out=th1_row, in_=th_row, func=AF.Identity,
                             bias=1.0)
        bv_row = rows.tile([1, H, HD], F32, tag="s_row", name="bv_row")
        nc.sync.dma_start(
            out=bv_row, in_=pvec["bv"].rearrange("(a h d) -> a h d", a=1, h=H))
        nc.vector.tensor_tensor(
            bv_row, bv_row, th1_row[:, :, None].to_broadcast((1, H, HD)),
            ALU.mult)
        nc.vector.tensor_tensor(c_work, c_work, bv_row, ALU.add)
    c_dram = dram.tile([D], F32, name="c_dram")
    nc.sync.dma_start(
        out=c_dram.rearrange("(a h d) -> a h d", a=1, h=H), in_=c_work)
    c_st = consts.tile([128, DT], F32, name="c_st")
    nc.sync.dma_start(out=c_st, in_=c_dram.rearrange("(o p) -> p o", p=128))
    if "c_vec" in dbg_d:
        nc.sync.dma_start(out=dbg_d["c_vec"][:], in_=c_dram[:])

    # ones column + zero out reference-key-0 (first text token)
    nc.vector.memset(v_sb[:, :, :, HD:HD + 1], 1.0)
    nc.vector.memset(vt_sb[:, :, HD:HD + 1], 1.0)
    nc.vector.memset(vt_sb[0:1, :, :], 0.0)

    # prefetch the first half of the proj weight while attention runs
    p_wA = tc.alloc_tile_pool(name="p_wA", bufs=1, side="right")
    wp_a = p_wA.tile([128, KT // 2, D], BF16, name="wp_a")
    nc.sync.dma_start(out=wp_a, in_=wp_d[:, 0:KT // 2, :])

    # =====================================================================
    # PH4: attention — per (pair, kt): 4 score matmuls into one 4-bank
    # [128,2048] psum, ONE exp over all 2048 cols (amortizes the ~350-cycle
    # ACT ramp), attn@v of the previous pair interleaved into emission so the
    # PE fills the exp-wait bubbles; softmax reciprocal on DVE (no ACT table
    # switches); subln stats of the last pair deferred into the proj phase.
    # =====================================================================
    p_OlnT = tc.alloc_tile_pool(name="p_OlnT", bufs=1, side="right")
    o_lnT = p_OlnT.tile([128, DT, NTOK], BF16, name="o_lnT")
    sumO_row = rows.tile([1, NTOK], F32, tag="sum_row", name="sumO_row")
    sqO_row = rows.tile([1, NTOK], F32, tag="sq_row", name="sqO_row")
    nc.vector.memset(sumO_row, 0.0)
    nc.vector.memset(sqO_row, 0.0)

    def emit_subln_stats(oj, sq_t4):
        for s in range(2):
            ps_a = mm_psum()[:1, :]
            nc.tensor.matmul(
                ps_a, ones_b, oj[:, s * 512:(s + 1) * 512],
                start=True, stop=True, skip_group_check=True)
            nc.vector.tensor_tensor(
                sumO_row[:, s * 512:(s + 1) * 512],
                sumO_row[:, s * 512:(s + 1) * 512], ps_a, ALU.add)
            ps_b = mm_psum()[:1, :]
            nc.tensor.matmul(
                ps_b, ones_b, sq_t4[:, s * 512:(s + 1) * 512],
                start=True, stop=True, skip_group_check=True)
            nc.vector.tensor_tensor(
                sqO_row[:, s * 512:(s + 1) * 512],
                sqO_row[:, s * 512:(s + 1) * 512], ps_b, ALU.add)

    attn_ctx = contextlib.ExitStack()
    p_attn = attn_ctx.enter_context(tc.tile_pool(name="p_attn", bufs=2))
    ps_big = attn_ctx.enter_context(
        tc.tile_pool(name="ps_big", bufs=2, space="PSUM"))

    def emit_normalize(j, se_pr, with_stats=True):
        nc.vector.reciprocal_approx_fast(out=se_pr, in_=se_pr)
        se_b = rows.tile([1, 2, NTOK], BF16, tag="se_b", name="se_b",
                         bufs=1)
        nc.vector.tensor_copy(out=se_b, in_=se_pr)
        rb = tmps.tile([128, 2, NTOK], BF16, tag="rb", name="rb", bufs=1)
        nc.gpsimd.partition_broadcast(rb, se_b)
        oj = o_lnT[:, j, :]
        nc.vector.tensor_tensor(oj[0:64, :], oj[0:64, :], rb[0:64, 0, :],
                                ALU.mult)
        nc.vector.tensor_tensor(oj[64:128, :], oj[64:128, :],
                                rb[64:128, 1, :], ALU.mult)
        nc.vector.tensor_scalar_add(out=oj, in0=oj,
                                    scalar1=c_st[:, j:j + 1])
        sq_t4 = sqp.tile([128, NTOK], BF16, tag="sq_t", name="sq_t4")
        nc.vector.tensor_tensor(sq_t4, oj, oj, ALU.mult)
        if with_stats:
            emit_subln_stats(oj, sq_t4)
        return oj, sq_t4

    def attnv_chunks(jj, pp, ppt, se_pr):
        chunks = []
        for hh in range(2):
            for s in range(2):
                def ch(hh=hh, s=s):
                    h = 2 * jj + hh
                    base = hh * 64
                    ps = mm_psum()[:HD + 1, :]
                    for kt in range(KT):
                        nc.tensor.matmul(
                            ps, v_sb[:, kt, h, :],
                            pp[:, kt,
                               hh * 1024 + s * 512:hh * 1024 + (s + 1) * 512],
                            start=(kt == 0), stop=False,
                            skip_group_check=True)
                    nc.tensor.matmul(
                        ps, vt_sb[:, h, :],
                        ppt[:, hh, s * 512:(s + 1) * 512],
                        start=False, stop=True, skip_group_check=True)
                    nc.vector.tensor_copy(
                        out=o_lnT[base:base + 64, jj, s * 512:(s + 1) * 512],
                        in_=ps[0:HD, :])
                    nc.vector.tensor_copy(
                        out=se_pr[:, hh, s * 512:(s + 1) * 512],
                        in_=ps[HD:HD + 1, :])
                chunks.append(ch)
        return chunks

    prev = None
    for j in range(8):      # head pairs
        pp = p_attn.tile([128, KT, 2048], BF16, tag="pp", name="pp")
        ppt = p_attn.tile([PT, 2, NTOK], BF16, tag="ppt", name="ppt")
        pend = attnv_chunks(*prev) if prev is not None else []
        for kt in range(KT):
            for hh in range(2):
                base = hh * 64
                psb = ps_big.tile([128, 1024], F32, tag="sc", name="psb")
                for s in range(2):
                    nc.tensor.matmul(
                        psb[:, s * 512:(s + 1) * 512],
                        kT[base:base + 64, j, kt * 128:(kt + 1) * 128],
                        qT[base:base + 64, j, s * 512:(s + 1) * 512],
                        start=True, stop=True, tile_position=(base, 0),
                        skip_group_check=True)
                nc.scalar.activation(
                    out=pp[:, kt, hh * 1024:(hh + 1) * 1024], in_=psb,
                    func=AF.Exp, scale=0.125)
            if kt % 2 == 0 and pend:
                pend.pop(0)()
        for hh in range(2):
            base = hh * 64
            psb = ps_big.tile([128, 1024], F32, tag="sc", name="psb")
            for s in range(2):
                nc.tensor.matmul(
                    psb[:PT, s * 512:(s + 1) * 512],
                    kT[base:base + 64, j, N:N + PT],
                    qT[base:base + 64, j, s * 512:(s + 1) * 512],
                    start=True, stop=True, tile_position=(base, 0),
                    skip_group_check=True)
            nc.scalar.activation(
                out=ppt[:, hh, :], in_=psb[:PT, :], func=AF.Exp,
                scale=0.125)
        for ch in pend:
            ch()
        if prev is not None:
            emit_normalize(prev[0], prev[3])
        se_pr = p_attn.tile([1, 2, NTOK], F32, tag="se_pr",
                            name="se_pr", bufs=1)
        prev = (j, pp, ppt, se_pr)
    # tail: attn@v + normalize of pair 7; its subln stats ride after the
    # first proj chain so they do not block the proj matmuls in the PE FIFO
    for ch in attnv_chunks(*prev):
        ch()
    oj7, sq7 = emit_normalize(7, prev[3], with_stats=False)
    attn_ctx.close()

    p_qkv.release()
    if "o_lnT" in dbg_d:
        nc.sync.dma_start(out=dbg_d["o_lnT"][:], in_=o_lnT[:])

    # =====================================================================
    # PH5+6: proj (deferred subln) + residual -> xnT, software-pipelined:
    # chain(ms) ... epilogue(ms-1); subln rows computed after chain 0;
    # LN2 stats interleaved per output tile.
    # =====================================================================
    p_xnT = tc.alloc_tile_pool(name="p_xnT", bufs=1)
    xnT = p_xnT.tile([128, DT, NTOK], F32, name="xnT")
    p_wB = tc.alloc_tile_pool(name="p_wB", bufs=1, side="right")
    wp_b = p_wB.tile([128, KT - KT // 2, D], BF16, name="wp_b")
    nc.sync.dma_start(out=wp_b, in_=wp_d[:, KT // 2:, :])
    wpsum_st = consts.tile([128, DT], F32, name="wpsum_st")
    nc.sync.dma_start(out=wpsum_st,
                      in_=pvec["wpsum"].rearrange("(o p) -> p o", p=128))
    bc = {}

    # reload x^T for the residual (straight from the pre-transposed input)
    p_xTr = tc.alloc_tile_pool(name="p_xTr", bufs=1)
    xTr = p_xTr.tile([128, DT, NTOK], XDT, name="xTr")
    nc.sync.dma_start(out=xTr, in_=x_d[:, :, :])

    skip_bp = "bp" in triv
    sum2_row = rows.tile([1, NTOK], F32, tag="sum_row", name="sum2_row")
    sq2_row = rows.tile([1, NTOK], F32, tag="sq_row", name="sq2_row")

    def emit_subln_rows():
        _, s_sub, negm_sub = ln_rows(sumO_row, sqO_row, D)
        bc["ssub"] = bcast_b(s_sub)
        ns_sub = rows.tile([1, NTOK], BF16, tag="nsrow", name="ns_sub",
                           bufs=1)
        nc.vector.tensor_tensor(ns_sub, negm_sub, s_sub, ALU.mult)
        bc["nsub"] = bcast_b(ns_sub)

    def proj_epilogue(m, s, ps):
        corr_m = tmps.tile([128, 512], BF16, tag="corr", name="corr_m",
                           bufs=2)
        nc.vector.tensor_scalar_mul(
            out=corr_m, in0=bc["nsub"][:, s * 512:(s + 1) * 512],
            scalar1=wpsum_st[:, m:m + 1])
        nc.vector.tensor_tensor(
            ps, ps, bc["ssub"][:, s * 512:(s + 1) * 512], ALU.mult)
        nc.vector.tensor_tensor(ps, ps, corr_m, ALU.add)
        if not skip_bp:
            nc.vector.tensor_scalar(
                out=ps, in0=ps, scalar1=st["bp"][:, m:m + 1],
                scalar2=None, op0=ALU.add)
        nc.vector.tensor_tensor(
            xnT[:, m, s * 512:(s + 1) * 512], ps,
            xTr[:, m, s * 512:(s + 1) * 512], ALU.add)

    def emit_ln2_tile_stats(m, hold):
        cst = sqp.tile([128, NTOK], BF16, tag="cst", name="cst2")
        nc.scalar.copy(out=cst, in_=xnT[:, m, :])
        sq_t = sqp.tile([128, NTOK], BF16, tag="sq_t", name="sq_t2")
        nc.vector.tensor_tensor(sq_t, cst, cst, ALU.mult)
        for s in range(2):
            nc.tensor.matmul(
                hold[0][s], ones_b, cst[:, s * 512:(s + 1) * 512],
                start=(m == 0), stop=(m == DT - 1), skip_group_check=True)
            nc.tensor.matmul(
                hold[1][s], ones_b, sq_t[:, s * 512:(s + 1) * 512],
                start=(m == 0), stop=(m == DT - 1), skip_group_check=True)

    with tc.tile_pool(name="ps_ln2", bufs=1, space="PSUM") as ps_ln2:
        hold = [[ps_ln2.tile([1, 512], F32, tag=f"l2{a}{s}",
                             name=f"l2{a}{s}") for s in range(2)]
                for a in range(2)]
        pending_ep = None
        for ms in range(16):
            m, s = divmod(ms, 2)
            ps = mm_psum()
            for o in range(KT):
                wsrc = (wp_a[:, o, :] if o < KT // 2
                        else wp_b[:, o - KT // 2, :])
                nc.tensor.matmul(
                    ps, wsrc[:, m * 128:(m + 1) * 128],
                    o_lnT[:, o, s * 512:(s + 1) * 512],
                    start=(o == 0), stop=(o == KT - 1))
            if ms == 0:
                emit_subln_stats(oj7, sq7)
                emit_subln_rows()
            if pending_ep is not None:
                proj_epilogue(*pending_ep)
                if pending_ep[1] == 1:
                    emit_ln2_tile_stats(pending_ep[0], hold)
            pending_ep = (m, s, ps)
        proj_epilogue(*pending_ep)
        emit_ln2_tile_stats(DT - 1, hold)
        for s in range(2):
            nc.scalar.copy(out=sum2_row[:, s * 512:(s + 1) * 512],
                           in_=hold[0][s])
            nc.scalar.copy(out=sq2_row[:, s * 512:(s + 1) * 512],
                           in_=hold[1][s])
    p_xTr.release()
    p_wB.release()
    p_OlnT.release()
    p_wA.release()
    if "xnT" in dbg_d:
        nc.sync.dma_start(out=dbg_d["xnT"][:], in_=xnT[:])

    # =====================================================================
    # PH7: LN2 -> x2T (bf16)
    # =====================================================================
    p_x2T = tc.alloc_tile_pool(name="p_x2T", bufs=1)
    x2T = p_x2T.tile([128, DT, NTOK], BF16, name="x2T")
    m_r, s_r, negm_r = ln_rows(sum2_row, sq2_row, D)
    fm_ln_apply(xnT, x2T, DT, negm_r, s_r, "n2")
    if "x2T" in dbg_d:
        nc.sync.dma_start(out=dbg_d["x2T"][:], in_=x2T[:])

    # =====================================================================
    # PH8: fc1 + gelu -> hT (bf16), with fused ffn_ln stats
    # =====================================================================
    p_hT = tc.alloc_tile_pool(name="p_hT", bufs=1, side="right")
    hT = p_hT.tile([128, HT, NTOK], BF16, name="hT")
    hsum_row = rows.tile([1, NTOK], F32, tag="sum_row", name="hsum_row")
    hsq_row = rows.tile([1, NTOK], F32, tag="sq_row", name="hsq_row")
    skip_b1 = "b1" in triv
    # ffn_ln stats via held accumulating ones-matmuls on the PE (the old DVE
    # accumulation made Vector the fc1 bottleneck at 93% busy); squares on DVE
    # (bf16, cheap), cross-partition sums ride 4 held psum banks.
    with tc.tile_pool(name="p_wblk", bufs=3) as p_wblk, \
         tc.tile_pool(name="ps_ffn", bufs=1, space="PSUM") as ps_ffn:
        ps_sum = [ps_ffn.tile([1, 512], F32, tag=f"ffsum{s}", name=f"ffsum{s}")
                  for s in range(2)]
        ps_sq = [ps_ffn.tile([1, 512], F32, tag=f"ffsq{s}", name=f"ffsq{s}")
                 for s in range(2)]
        for hm in range(HT):
            w1blk = p_wblk.tile([128, KT, 128], BF16, tag="w1blk",
                                name="w1blk")
            nc.sync.dma_start(out=w1blk, in_=w1_d[hm])
            for s in range(2):
                ps = mm_psum()
                for o in range(KT):
                    nc.tensor.matmul(
                        ps, w1blk[:, o, :],
                        x2T[:, o, s * 512:(s + 1) * 512],
                        start=(o == 0), stop=(o == KT - 1))
                hslice = hT[:, hm, s * 512:(s + 1) * 512]
                if skip_b1:
                    nc.scalar.activation(out=hslice, in_=ps, func=AF.Gelu)
                else:
                    nc.scalar.activation(
                        out=hslice, in_=ps, func=AF.Gelu,
                        bias=st["b1"][:, hm:hm + 1], scale=1.0)
                sq_t = sqp.tile([128, NTOK], BF16, tag="sq_t",
                                name="sq_tf")[:, :512]
                nc.vector.tensor_tensor(sq_t, hslice, hslice, ALU.mult)
                nc.tensor.matmul(
                    ps_sum[s], ones_b, hslice,
                    start=(hm == 0), stop=(hm == HT - 1),
                    skip_group_check=True)
                nc.tensor.matmul(
                    ps_sq[s], ones_b, sq_t,
                    start=(hm == 0), stop=(hm == HT - 1),
                    skip_group_check=True)
        for s in range(2):
            nc.scalar.copy(out=hsum_row[:, s * 512:(s + 1) * 512],
                           in_=ps_sum[s])
            nc.scalar.copy(out=hsq_row[:, s * 512:(s + 1) * 512],
                           in_=ps_sq[s])
    p_x2T.release()
    if "hT" in dbg_d:
        nc.sync.dma_start(out=dbg_d["hT"][:], in_=hT[:])

    # =====================================================================
    # PH9: ffn_ln rows only (normalization deferred into fc2: an extra K=1
    # matmul row adds -mean*colsum(W2); psum scaled by rstd in the epilogue)
    # =====================================================================
    _, s_ffn, negm_ffn = ln_rows(hsum_row, hsq_row, HID)
    sffn_b = bcast_b(s_ffn)
    w2sum_st = consts.tile([128, DT], F32, name="w2sum_st")
    nc.sync.dma_start(out=w2sum_st,
                      in_=pvec["w2sum"].rearrange("(o p) -> p o", p=128))
    ns_ffn = rows.tile([1, NTOK], BF16, tag="nsrow", name="ns_ffn", bufs=1)
    nc.vector.tensor_tensor(ns_ffn, negm_ffn, s_ffn, ALU.mult)
    nffn_b = bcast_b(ns_ffn)

    # =====================================================================
    # PH10: fc2 (with deferred ffn_ln) + residual + transpose + store
    # =====================================================================
    skip_b2 = "b2" in triv
    with tc.tile_pool(name="p_w2blk", bufs=3) as p_w2blk, \
         tc.tile_pool(name="p_out", bufs=4) as p_out:
        for m in range(DT):
            w2blk = p_w2blk.tile([128, HT, 128], BF16, tag="w2blk",
                                 name="w2blk")
            nc.sync.dma_start(out=w2blk, in_=w2_d[m])
            for s in range(2):
                corr2 = tmps.tile([128, 512], BF16, tag="corr",
                                  name="corr2", bufs=2)
                nc.vector.tensor_scalar_mul(
                    out=corr2, in0=nffn_b[:, s * 512:(s + 1) * 512],
                    scalar1=w2sum_st[:, m:m + 1])
                outm = p_out.tile([128, 512], F32, tag="outm", name="outm")
                ps = mm_psum()
                for o in range(HT):
                    nc.tensor.matmul(
                        ps, w2blk[:, o, :],
                        hT[:, o, s * 512:(s + 1) * 512],
                        start=(o == 0), stop=(o == HT - 1))
                nc.vector.tensor_tensor(
                    ps, ps, sffn_b[:, s * 512:(s + 1) * 512], ALU.mult)
                nc.vector.tensor_tensor(ps, ps, corr2, ALU.add)
                if not skip_b2:
                    nc.vector.tensor_scalar(
                        out=ps, in0=ps, scalar1=st["b2"][:, m:m + 1],
                        scalar2=None, op0=ALU.add)
                nc.vector.tensor_tensor(
                    outm, ps, xnT[:, m, s * 512:(s + 1) * 512], ALU.add)
                nc.sync.dma_start(
                    out=y_d[:, m, s * 512:(s + 1) * 512], in_=outm)
    p_hT.release()
    p_xnT.release()
    ctx.close()


# --------------------------------------------------------------------------
# host glue
# --------------------------------------------------------------------------

_PROGRAM_CACHE = {}


def get_program(debug=(), triv=frozenset()):
    key = (tuple(sorted(debug)), tuple(sorted(triv)))
    if key not in _PROGRAM_CACHE:
        _PROGRAM_CACHE[key] = build_program(debug=key[0], triv=key[1])
    return _PROGRAM_CACHE[key]


def compute_triv(inputs):
    f32 = np.float32
    triv = set()
    for k in ["n1", "n3", "ln", "n2", "ffn"]:
        g = np.asarray(inputs[k + "_g"], f32)
        b = np.asarray(inputs[k + "_b"], f32)
        if np.all(g == 1.0) and np.all(b == 0.0):
            triv.add(k)
    for k in ["bq", "bv", "b1"]:
        if np.all(np.asarray(inputs[k], f32) == 0.0):
            triv.add(k)
    bp_eff = (np.asarray(inputs["bp"], f32)
              + np.asarray(inputs["ln_b"], f32) @ np.asarray(inputs["Wp"], f32))
    if np.all(bp_eff == 0.0):
        triv.add("bp")
    b2_eff = (np.asarray(inputs["b2"], f32)
              + np.asarray(inputs["ffn_b"], f32) @ np.asarray(inputs["W2"], f32))
    if np.all(b2_eff == 0.0):
        triv.add("b2")
    return frozenset(triv)


def make_in_maps(inputs):
    """Build the 8 per-core input maps from the full-problem input dict."""
    bf = ml_dtypes.bfloat16
    f32 = np.float32

    def host(name):
        return np.asarray(inputs[name], dtype=f32)

    # fold the subln (ln_g/ln_b) into Wp/bp and the ffn_ln (ffn_g/ffn_b)
    # into W2/b2 — the kernel defers those norms into the matmuls and only
    # applies (x-mean)*rstd
    wp_eff = host("ln_g")[:, None] * host("Wp")
    bp_eff = host("bp") + host("ln_b") @ host("Wp")
    w2_eff = host("ffn_g")[:, None] * host("W2")
    b2_eff = host("b2") + host("ffn_b") @ host("W2")
    wp_bf = wp_eff.astype(bf)
    w2_bf = w2_eff.astype(bf)
    shared = dict(
        wq=np.ascontiguousarray(
            host("Wq").reshape(KT, 128, D).astype(bf).transpose(1, 0, 2)),
        wk=np.ascontiguousarray(
            host("Wk").reshape(KT, 128, D).astype(bf).transpose(1, 0, 2)),
        wv=np.ascontiguousarray(
            host("Wv").reshape(KT, 128, D).astype(bf).transpose(1, 0, 2)),
        wp=np.ascontiguousarray(
            wp_bf.reshape(KT, 128, D).transpose(1, 0, 2)),
        wpsum=wp_bf.astype(np.float32).sum(0).astype(f32),
        w1=np.ascontiguousarray(
            host("W1").reshape(KT, 128, HT, 128).transpose(2, 1, 0, 3)
        ).astype(bf),
        w2=np.ascontiguousarray(
            w2_bf.reshape(HT, 128, DT, 128).transpose(2, 1, 0, 3)),
        w2sum=w2_bf.astype(np.float32).sum(0).astype(f32),
        n1_g=host("n1_g"), n1_b=host("n1_b"),
        n3_g=host("n3_g"), n3_b=host("n3_b"),
        bq=host("bq"), bv=host("bv"),
        ln_g=host("ln_g"), ln_b=host("ln_b"),
        bp=bp_eff.astype(f32),
        n2_g=host("n2_g"), n2_b=host("n2_b"),
        b1=host("b1"), ffn_g=host("ffn_g"), ffn_b=host("ffn_b"),
        b2=b2_eff.astype(f32),
        gate=host("gate").reshape(H),
    )
    x = host("x")
    xt = host("x_text")
    in_maps = []
    for b in range(B):
        m = dict(shared)
        # device consumes feature-major, partition-major inputs
        m["x"] = np.ascontiguousarray(
            x[b].T.reshape(DT, 128, N).transpose(1, 0, 2)).astype(bf)
        m["x_text"] = np.ascontiguousarray(
            xt[b].T.reshape(DT, 128, PT).transpose(1, 0, 2)).astype(bf)
        in_maps.append(m)
    return in_maps


def unpack_y(y):
    """Device output is feature-major [128, DT, NTOK]; back to [NTOK, D]."""
    y = np.asarray(y)
    return np.transpose(y, (2, 1, 0)).reshape(NTOK, D)


def kernel(**inputs) -> np.ndarray:
    from concourse.bass_utils import run_bass_kernel_spmd

    nc = get_program(triv=compute_triv(inputs))
    in_maps = make_in_maps(inputs)
    res = run_bass_kernel_spmd(nc, in_maps, list(range(B)))
    out = np.stack([unpack_y(res.results[b]["y"]) for b in range(B)], axis=0)
    return out.astype(np.float32)



# revision 46
# speedup vs baseline: 1.2667x; 1.2667x over previous
"""Trainium2 Bass kernel for nn_Block_45724221833665 (dense transformer block).

Strategy: pure data-parallel over batch — 8 batch elements, 8 NeuronCores, one
batch element per core, no collectives.  Inside a core everything is computed
"feature-major" (features on SBUF partitions, tokens on the free dim) so that:
  * all matmuls consume operands in their natural layout (weights as lhsT),
  * all per-feature biases/gains are per-partition scalars,
  * attention needs no transposes at all (scores are computed as S^T, attn@v
    takes exp(S^T) directly as the moving operand with V as the stationary
    operand, and the per-token softmax denominator comes from an extra all-ones
    column appended to V).
Inputs arrive host-pre-transposed (feature-major) and the output is returned
feature-major and transposed back on the host, so the device does no
transposes at all.

LayerNorms in feature-major form need cross-partition sums; those are done on
the tensor engine with an all-ones stationary vector over bf16 casts, and the
per-token mean/rstd rows are broadcast across partitions by DMA via a small
DRAM bounce.  Reciprocals run on ACT (the DVE iterative divide is ~8
cycles/element and our rows live on one partition); the LN rstd gets one
Newton step on DVE to recover full precision.

The gated dual softmax simplifies: softmax over a single key (column 0) is
identically 1, so the reference's w_prefix column is just tanh(gate) per head.
Attention output = softmax over the other 1087 keys @ v  +  tanh(gate)*v[key0]
(+ (1+tanh(gate))*bv once the v-bias is folded out of the softmax-weighted sum,
since the main softmax weights sum to 1).

Gains that are identically 1 and biases identically 0 (the common case for
this block) are detected on the host and their application elided; the cache
key of the built program includes those flags, so nontrivial parameters still
take the general path.
"""

import numpy as np
import ml_dtypes

import concourse.bass as bass
import concourse.mybir as mybir
import concourse.tile as tile
from concourse import bacc

B, N, PT, D, H, HD, HID = 8, 1024, 64, 1024, 16, 64, 4096
KT = D // 128          # 8 contraction tiles over D
HT = HID // 128        # 32 tiles over HID
DT = D // 128          # 8 output tiles over D
NTOK = N               # 1024 main tokens per core
NKEY = N + PT          # 1088 keys (main tokens + text prefix)
EPS = 1e-5

F32 = mybir.dt.float32
BF16 = mybir.dt.bfloat16
XDT = BF16  # wire dtype of the (host-pre-transposed) x / x_text inputs
AF = mybir.ActivationFunctionType
ALU = mybir.AluOpType


def build_program(debug=(), triv=frozenset()):
    """Build the single-core Bass program.
    debug: iterable of intermediate names to emit as DRAM outputs.
    triv: parameter groups whose gain/bias application can be skipped."""
    nc = bacc.Bacc("TRN2")
    dbg = set(debug)
    triv = set(triv)

    # ---- I/O ------------------------------------------------------------
    # x / x_text arrive HOST-PRE-TRANSPOSED (feature-major [D, tokens])
    x_d = nc.declare_dram_parameter("x", [128, DT, NTOK], XDT, isOutput=False)
    xt_d = nc.declare_dram_parameter("x_text", [128, DT, PT], XDT,
                                     isOutput=False)
    wq_d = nc.declare_dram_parameter("wq", [128, KT, D], BF16, isOutput=False)
    wk_d = nc.declare_dram_parameter("wk", [128, KT, D], BF16, isOutput=False)
    wv_d = nc.declare_dram_parameter("wv", [128, KT, D], BF16, isOutput=False)
    wp_d = nc.declare_dram_parameter("wp", [128, KT, D], BF16, isOutput=False)
    w1_d = nc.declare_dram_parameter("w1", [HT, 128, KT, 128], BF16,
                                 isOutput=False)
    w2_d = nc.declare_dram_parameter("w2", [DT, 128, HT, 128], BF16,
                                     isOutput=False)
    pvec = {}
    for nm, sz in [
        ("n1_g", D), ("n1_b", D), ("n3_g", D), ("n3_b", D),
        ("bq", D), ("bv", D), ("ln_g", D), ("ln_b", D), ("bp", D),
        ("n2_g", D), ("n2_b", D), ("b1", HID), ("ffn_g", HID), ("ffn_b", HID),
        ("b2", D),
    ]:
        pvec[nm] = nc.declare_dram_parameter(nm, [sz], F32, isOutput=False)
    pvec["wpsum"] = nc.declare_dram_parameter("wpsum", [D], F32,
                                              isOutput=False)
    pvec["w2sum"] = nc.declare_dram_parameter("w2sum", [D], F32,
                                              isOutput=False)
    gate_d = nc.declare_dram_parameter("gate", [H], F32, isOutput=False)
    # y is produced feature-major [128, DT, NTOK]; host transposes back
    y_d = nc.declare_dram_parameter("y", [128, DT, NTOK], F32, isOutput=True)

    dbg_d = {}

    def dbg_out(name, shape, dtype):
        if name in dbg:
            dbg_d[name] = nc.declare_dram_parameter(
                "dbg_" + name, list(shape), dtype, isOutput=True
            )

    dbg_out("xT", [128, DT, NTOK], F32)
    dbg_out("x1T", [128, DT, NTOK], BF16)
    dbg_out("xt1T", [128, DT, PT], BF16)
    dbg_out("qT", [128, DT, NTOK], BF16)
    dbg_out("kT", [128, DT, NKEY], BF16)
    dbg_out("v", [128, KT, H, HD + 1], BF16)
    dbg_out("v_text", [PT, H, HD + 1], BF16)
    dbg_out("c_vec", [D], F32)
    dbg_out("pp0", [128, 2, KT, NTOK], BF16)     # exp(S^T) for heads 0,1
    dbg_out("pp0_text", [PT, 2, NTOK], BF16)
    dbg_out("o_full", [128, DT, NTOK], F32)
    dbg_out("o_lnT", [128, DT, NTOK], BF16)
    dbg_out("xnT", [128, DT, NTOK], F32)
    dbg_out("x2T", [128, DT, NTOK], BF16)
    dbg_out("hT", [128, HT, NTOK], BF16)

    with tile.TileContext(nc) as tc:
        _build_phases(nc, tc, x_d, xt_d, wq_d, wk_d, wv_d, wp_d,
                      w1_d, w2_d, pvec, gate_d, y_d, dbg_d, triv)
    nc.compile()
    return nc


def _build_phases(nc, tc, x_d, xt_d, wq_d, wk_d, wv_d, wp_d, w1_d, w2_d,
                  pvec, gate_d, y_d, dbg_d, triv):
    import contextlib
    ctx = contextlib.ExitStack()
    # ---- whole-kernel pools ---------------------------------------------
    consts = ctx.enter_context(tc.tile_pool(name="consts", bufs=1))
    rows = ctx.enter_context(tc.tile_pool(name="rows", bufs=1))
    tmps = ctx.enter_context(tc.tile_pool(name="tmps", bufs=2))
    sqp = ctx.enter_context(tc.tile_pool(name="sqp", bufs=2))
    dram = ctx.enter_context(tc.tile_pool(name="dram", bufs=1, space="DRAM"))
    ps_mm = ctx.enter_context(tc.tile_pool(name="ps_mm", bufs=3, space="PSUM"))
    # p_qkv sits below p_x1T on the pool stack so x1T can be freed after the
    # projections while qT/kT/v live on through attention
    p_qkv = tc.alloc_tile_pool(name="p_qkv", bufs=1)
    p_x1T = tc.alloc_tile_pool(name="p_x1T", bufs=1)

    def mm_psum(pfree=512, parts=128):
        t = ps_mm.tile([128, 512], F32, tag="mm", name="mmps")
        return t[:parts, :pfree]

    # ---- constants ------------------------------------------------------
    ones_b = consts.tile([128, 1], BF16, name="ones_b")
    nc.vector.memset(ones_b, 1.0)
    eps_c = consts.tile([1, 1], F32, name="eps_c")
    nc.vector.memset(eps_c, EPS)

    st = {}  # striped parameter tiles [128, tiles]
    for nm in ["n1_g", "n1_b", "n3_g", "n3_b", "bq", "ln_g", "ln_b", "bp",
               "n2_g", "n2_b", "b2"]:
        t = consts.tile([128, DT], F32, name="st_" + nm)
        nc.sync.dma_start(out=t, in_=pvec[nm].rearrange("(o p) -> p o", p=128))
        st[nm] = t
    for nm in ["b1", "ffn_g", "ffn_b"]:
        t = consts.tile([128, HT], F32, name="st_" + nm)
        nc.sync.dma_start(out=t, in_=pvec[nm].rearrange("(o p) -> p o", p=128))
        st[nm] = t

    def bcast_b(row, ntok=NTOK):
        """Broadcast a [1, ntok] row to a [128, ntok] bf16 tile via GPSIMD
        (no DMA queues involved); casts f32 rows to bf16 first."""
        if row.dtype != BF16:
            br = rows.tile([1, NTOK], BF16, tag="brow", name="brow",
                           bufs=1)[:, :ntok]
            nc.vector.tensor_copy(out=br, in_=row)
            row = br
        out = tmps.tile([128, NTOK], BF16, tag="wrk", name="bb")[:, :ntok]
        nc.gpsimd.partition_broadcast(out, row)
        return out

    def ln_rows(sum_row, sq_row, n_elems):
        """Turn per-token sums into (mean, rstd, -mean[bf16]) rows.  rstd =
        ACT abs_reciprocal_sqrt(var+eps) refined by one rsqrt-Newton step on
        DVE (the banned-for-accuracy ACT seed is fine once refined)."""
        ntok = sum_row.shape[-1]
        t_row = rows.tile([1, NTOK], F32, tag="t_row", name="t_row")[:, :ntok]
        s_row = rows.tile([1, NTOK], F32, tag="s_row", name="s_row")[:, :ntok]
        u_row = rows.tile([1, NTOK], F32, tag="u_row", name="u_row",
                          bufs=1)[:, :ntok]
        negm = rows.tile([1, NTOK], BF16, tag="negm", name="negm")[:, :ntok]
        inv = 1.0 / float(n_elems)
        nc.vector.tensor_scalar_mul(out=sum_row, in0=sum_row, scalar1=inv)
        nc.vector.tensor_scalar_mul(out=sq_row, in0=sq_row, scalar1=inv)
        nc.vector.tensor_tensor(t_row, sum_row, sum_row, ALU.mult)  # mean^2
        nc.vector.tensor_tensor(sq_row, sq_row, t_row, ALU.subtract)  # var
        nc.scalar.activation(out=s_row, in_=sq_row,
                             func=AF.Abs_reciprocal_sqrt,
                             bias=eps_c, scale=1.0)                 # ~rstd
        # rsqrt Newton: s <- s*(1.5 - 0.5*(var+eps)*s^2)
        nc.vector.tensor_scalar_add(out=t_row, in0=sq_row, scalar1=EPS)
        nc.vector.tensor_tensor(u_row, s_row, s_row, ALU.mult)
        nc.vector.tensor_tensor(u_row, u_row, t_row, ALU.mult)
        nc.vector.tensor_scalar(out=u_row, in0=u_row, scalar1=-0.5,
                                scalar2=1.5, op0=ALU.mult, op1=ALU.add)
        nc.vector.tensor_tensor(s_row, s_row, u_row, ALU.mult)
        nc.vector.tensor_scalar_mul(out=negm, in0=sum_row, scalar1=-1.0)
        return sum_row, s_row, negm

    # =====================================================================
    # PH1: load x / x_text directly in feature-major layout (host-transposed)
    # =====================================================================
    x1T = p_x1T.tile([128, DT, NTOK], BF16, name="x1T")
    p_xtB = tc.alloc_tile_pool(name="p_xtB", bufs=1)
    xt1T = p_xtB.tile([128, DT, PT], BF16, name="xt1T")
    p_xtA = tc.alloc_tile_pool(name="p_xtA", bufs=1)
    xtT = p_xtA.tile([128, DT, PT], XDT, name="xtT")
    p_xT = tc.alloc_tile_pool(name="p_xT", bufs=1)
    xT = p_xT.tile([128, DT, NTOK], XDT, name="xT")

    nc.sync.dma_start(out=xtT, in_=xt_d[:, :, :])
    for hf in range(2):  # two half-chunks: big descriptors, early stats
        nc.sync.dma_start(
            out=xT[:, hf * 4:(hf + 1) * 4, :],
            in_=x_d[:, hf * 4:(hf + 1) * 4, :])
    if "xT" in dbg_d:
        nc.sync.dma_start(out=dbg_d["xT"][:], in_=xT[:])

    # ---- feature-major layernorm helpers --------------------------------
    def fm_ln_stats(src, ntiles, ntok, sl=512):
        """Cross-partition LN stats for src [128, ntiles, ntok] (f32 or bf16).
        Returns (sum_row, sq_row) [1, ntok] f32 rows (in `rows` pool)."""
        sum_row = rows.tile([1, NTOK], F32, tag="sum_row",
                            name="sum_row")[:, :ntok]
        sq_row = rows.tile([1, NTOK], F32, tag="sq_row",
                           name="sq_row")[:, :ntok]
        nsl = ntok // sl
        with tc.tile_pool(name="ps_stat", bufs=4, space="PSUM") as ps_stat:
            ps_a = [ps_stat.tile([1, sl], F32, tag="stat", name=f"psa{s}",
                                 bufs=2) for s in range(nsl)]
            ps_b = [ps_stat.tile([1, sl], F32, tag="stat2", name=f"psb{s}",
                                 bufs=2) for s in range(nsl)]
            for o in range(ntiles):
                if src.dtype == F32:
                    cst = sqp.tile([128, NTOK], BF16, tag="cst",
                                   name="cst")[:, :ntok]
                    nc.scalar.copy(out=cst, in_=src[:, o, :])
                else:
                    cst = src[:, o, :]
                sq_t = sqp.tile([128, NTOK], BF16, tag="sq_t",
                                name="sq_t")[:, :ntok]
                nc.vector.tensor_tensor(sq_t, cst, cst, ALU.mult)
                for s in range(nsl):
                    nc.tensor.matmul(
                        ps_a[s], ones_b, cst[:, s * sl:(s + 1) * sl],
                        start=(o == 0), stop=(o == ntiles - 1),
                        skip_group_check=True)
                    nc.tensor.matmul(
                        ps_b[s], ones_b, sq_t[:, s * sl:(s + 1) * sl],
                        start=(o == 0), stop=(o == ntiles - 1),
                        skip_group_check=True)
            for s in range(nsl):
                nc.scalar.copy(out=sum_row[:, s * sl:(s + 1) * sl], in_=ps_a[s])
                nc.scalar.copy(out=sq_row[:, s * sl:(s + 1) * sl], in_=ps_b[s])
        return sum_row, sq_row

    def fm_ln_apply(src, dst, ntiles, negm_row, s_row, gkey):
        """dst[:,o,:] = ((src - m)*rstd)[*g + b], slice-pipelined so
        consumers ordered slice-outer can start on slice 0 early."""
        ntok = src.shape[-1]
        g_st = st.get(gkey + "_g")
        b_st = st.get(gkey + "_b")
        skip_gb = gkey in triv
        m_b = bcast_b(negm_row, ntok)
        s_b = bcast_b(s_row, ntok)
        nsl = max(1, ntok // 512)
        sl = ntok // nsl
        for s in range(nsl):
            ss = slice(s * sl, (s + 1) * sl)
            for o in range(ntiles):
                tA = tmps.tile([128, 512], BF16, tag="tA",
                               name="tA")[:, :sl]
                nc.vector.tensor_tensor(tA, src[:, o, ss], m_b[:, ss],
                                        ALU.add)
                if skip_gb:
                    nc.vector.tensor_tensor(dst[:, o, ss], tA, s_b[:, ss],
                                            ALU.mult)
                else:
                    nc.vector.tensor_tensor(tA, tA, s_b[:, ss], ALU.mult)
                    nc.scalar.activation(
                        out=dst[:, o, ss], in_=tA, func=AF.Identity,
                        scale=g_st[:, o:o + 1], bias=b_st[:, o:o + 1])

    # =====================================================================
    # PH2: LN1(x) -> x1T (bf16), LN3(x_text) -> xt1T (bf16)
    # =====================================================================
    sum_r, sq_r = fm_ln_stats(xtT, DT, PT, sl=PT)
    m_r, s_r, negm_r = ln_rows(sum_r, sq_r, D)
    fm_ln_apply(xtT, xt1T, DT, negm_r, s_r, "n3")

    sum_r, sq_r = fm_ln_stats(xT, DT, NTOK)
    m_r, s_r, negm_r = ln_rows(sum_r, sq_r, D)
    fm_ln_apply(xT, x1T, DT, negm_r, s_r, "n1")
    p_xT.release()
    p_xtA.release()

    if "x1T" in dbg_d:
        nc.sync.dma_start(out=dbg_d["x1T"][:], in_=x1T[:])
    if "xt1T" in dbg_d:
        nc.sync.dma_start(out=dbg_d["xt1T"][:], in_=xt1T[:])

    # =====================================================================
    # PH3: projections  Q^T, K^T (feature-major), V (token-major, +ones col)
    # =====================================================================
    qT = p_qkv.tile([128, DT, NTOK], BF16, name="qT")
    kT = p_qkv.tile([128, DT, NKEY], BF16, name="kT")
    v_sb = p_qkv.tile([128, KT, H, HD + 1], BF16, name="v_sb")
    vt_sb = p_qkv.tile([PT, H, HD + 1], BF16, name="vt_sb")

    p_w = tc.alloc_tile_pool(name="p_w", bufs=2)
    wq_sb = p_w.tile([128, KT, D], BF16, tag="wfull", name="wq_sb")
    nc.sync.dma_start(out=wq_sb, in_=wq_d[:, :, :])
    wk_sb = p_w.tile([128, KT, D], BF16, tag="wfull", name="wk_sb")
    nc.sync.dma_start(out=wk_sb, in_=wk_d[:, :, :])
    skip_bq = "bq" in triv
    for s in range(2):
        for m in range(DT):
            ps = mm_psum()
            for o in range(KT):
                nc.tensor.matmul(
                    ps, wq_sb[:, o, m * 128:(m + 1) * 128],
                    x1T[:, o, s * 512:(s + 1) * 512],
                    start=(o == 0), stop=(o == KT - 1))
            if skip_bq:
                nc.scalar.copy(out=qT[:, m, s * 512:(s + 1) * 512], in_=ps)
            else:
                nc.scalar.activation(
                    out=qT[:, m, s * 512:(s + 1) * 512], in_=ps,
                    func=AF.Identity, bias=st["bq"][:, m:m + 1], scale=1.0)
        for m in range(DT):
            ps = mm_psum()
            for o in range(KT):
                nc.tensor.matmul(
                    ps, wk_sb[:, o, m * 128:(m + 1) * 128],
                    x1T[:, o, s * 512:(s + 1) * 512],
                    start=(o == 0), stop=(o == KT - 1))
            nc.scalar.copy(out=kT[:, m, s * 512:(s + 1) * 512], in_=ps)
    for m in range(DT):  # K^T text keys
        ps = mm_psum(PT)
        for o in range(KT):
            nc.tensor.matmul(
                ps, wk_sb[:, o, m * 128:(m + 1) * 128], xt1T[:, o, :],
                start=(o == 0), stop=(o == KT - 1))
        nc.scalar.copy(out=kT[:, m, N:N + PT], in_=ps)
    # V token-major, heads interleaved with the all-ones 65th column
    wv_sb = p_w.tile([128, KT, D], BF16, tag="wfull", name="wv_sb")
    nc.sync.dma_start(out=wv_sb, in_=wv_d[:, :, :])
    for t in range(8):
        for sn in range(2):
            ps = mm_psum()
            for o in range(KT):
                nc.tensor.matmul(
                    ps, x1T[:, o, t * 128:(t + 1) * 128],
                    wv_sb[:, o, sn * 512:(sn + 1) * 512],
                    start=(o == 0), stop=(o == KT - 1))
            nc.scalar.copy(
                out=v_sb[:, t, sn * 8:(sn + 1) * 8, 0:HD], in_=ps)
    for sn in range(2):
        ps = mm_psum(parts=PT)
        for o in range(KT):
            nc.tensor.matmul(
                ps, xt1T[:, o, :], wv_sb[:, o, sn * 512:(sn + 1) * 512],
                start=(o == 0), stop=(o == KT - 1))
        nc.scalar.copy(out=vt_sb[:, sn * 8:(sn + 1) * 8, 0:HD], in_=ps)
    p_w.release()
    p_xtB.release()
    p_x1T.release()

    # ---- c_vec: tanh(gate)*v0_raw + (1+tanh(gate))*bv  ------------------
    g_row = rows.tile([1, H], F32, tag="g_row", name="g_row")
    nc.sync.dma_start(out=g_row, in_=gate_d.rearrange("(a h) -> a h", a=1))
    th_row = rows.tile([1, H], F32, tag="th_row", name="th_row")
    nc.scalar.activation(out=th_row, in_=g_row, func=AF.Tanh)
    c_work = rows.tile([1, H, HD], F32, tag="t_row", name="c_work")
    nc.vector.tensor_copy(out=c_work, in_=vt_sb[0:1, :, 0:HD])
    nc.vector.tensor_tensor(
        c_work, c_work, th_row[:, :, None].to_broadcast((1, H, HD)), ALU.mult)
    if "bv" not in triv:
        th1_row = rows.tile([1, H], F32, tag="th1_row", name="th1_row")
        nc.scalar.activation(out=th1_row, in_=th_row, func=AF.Identity,
                             bias=1.0)
        bv_row = rows.tile([1, H, HD], F32, tag="s_row", name="bv_row")
        nc.sync.dma_start(
            out=bv_row, in_=pvec["bv"].rearrange("(a h d) -> a h d", a=1, h=H))
        nc.vector.tensor_tensor(
            bv_row, bv_row, th1_row[:, :, None].to_broadcast((1, H, HD)),
            ALU.mult)
        nc.vector.tensor_tensor(c_work, c_work, bv_row, ALU.add)
    c_dram = dram.tile([D], F32, name="c_dram")
    nc.sync.dma_start(
        out=c_dram.rearrange("(a h d) -> a h d", a=1, h=H), in_=c_work)
    c_st = consts.tile([128, DT], F32, name="c_st")
    nc.sync.dma_start(out=c_st, in_=c_dram.rearrange("(o p) -> p o", p=128))
    if "c_vec" in dbg_d:
        nc.sync.dma_start(out=dbg_d["c_vec"][:], in_=c_dram[:])

    # ones column + zero out reference-key-0 (first text token)
    nc.vector.memset(v_sb[:, :, :, HD:HD + 1], 1.0)
    nc.vector.memset(vt_sb[:, :, HD:HD + 1], 1.0)
    nc.vector.memset(vt_sb[0:1, :, :], 0.0)

    # prefetch the first half of the proj weight while attention runs
    p_wA = tc.alloc_tile_pool(name="p_wA", bufs=1, side="right")
    wp_a = p_wA.tile([128, KT // 2, D], BF16, name="wp_a")
    nc.sync.dma_start(out=wp_a, in_=wp_d[:, 0:KT // 2, :])

    # =====================================================================
    # PH4: attention — per (pair, kt): 4 score matmuls into one 4-bank
    # [128,2048] psum, ONE exp over all 2048 cols (amortizes the ~350-cycle
    # ACT ramp), attn@v of the previous pair interleaved into emission so the
    # PE fills the exp-wait bubbles; softmax reciprocal on DVE (no ACT table
    # switches); subln stats of the last pair deferred into the proj phase.
    # =====================================================================
    p_OlnT = tc.alloc_tile_pool(name="p_OlnT", bufs=1, side="right")
    o_lnT = p_OlnT.tile([128, DT, NTOK], BF16, name="o_lnT")
    sumO_row = rows.tile([1, NTOK], F32, tag="sum_row", name="sumO_row")
    sqO_row = rows.tile([1, NTOK], F32, tag="sq_row", name="sqO_row")
    nc.vector.memset(sumO_row, 0.0)
    nc.vector.memset(sqO_row, 0.0)

    def emit_subln_stats(oj, sq_t4):
        for s in range(2):
            ps_a = mm_psum()[:1, :]
            nc.tensor.matmul(
                ps_a, ones_b, oj[:, s * 512:(s + 1) * 512],
                start=True, stop=True, skip_group_check=True)
            nc.vector.tensor_tensor(
                sumO_row[:, s * 512:(s + 1) * 512],
                sumO_row[:, s * 512:(s + 1) * 512], ps_a, ALU.add)
            ps_b = mm_psum()[:1, :]
            nc.tensor.matmul(
                ps_b, ones_b, sq_t4[:, s * 512:(s + 1) * 512],
                start=True, stop=True, skip_group_check=True)
            nc.vector.tensor_tensor(
                sqO_row[:, s * 512:(s + 1) * 512],
                sqO_row[:, s * 512:(s + 1) * 512], ps_b, ALU.add)

    attn_ctx = contextlib.ExitStack()
    p_attn = attn_ctx.enter_context(tc.tile_pool(name="p_attn", bufs=2))
    ps_big = attn_ctx.enter_context(
        tc.tile_pool(name="ps_big", bufs=2, space="PSUM"))

    def emit_normalize(j, se_pr, with_stats=True):
        nc.vector.reciprocal_approx_fast(out=se_pr, in_=se_pr)
        se_b = rows.tile([1, 2, NTOK], BF16, tag="se_b", name="se_b",
                         bufs=1)
        nc.vector.tensor_copy(out=se_b, in_=se_pr)
        rb = tmps.tile([128, 2, NTOK], BF16, tag="rb", name="rb", bufs=1)
        nc.gpsimd.partition_broadcast(rb, se_b)
        oj = o_lnT[:, j, :]
        nc.vector.tensor_tensor(oj[0:64, :], oj[0:64, :], rb[0:64, 0, :],
                                ALU.mult)
        nc.vector.tensor_tensor(oj[64:128, :], oj[64:128, :],
                                rb[64:128, 1, :], ALU.mult)
        nc.vector.tensor_scalar_add(out=oj, in0=oj,
                                    scalar1=c_st[:, j:j + 1])
        sq_t4 = sqp.tile([128, NTOK], BF16, tag="sq_t", name="sq_t4")
        nc.vector.tensor_tensor(sq_t4, oj, oj, ALU.mult)
        if with_stats:
            emit_subln_stats(oj, sq_t4)
        return oj, sq_t4

    def attnv_chunks(jj, pp, ppt, se_pr):
        chunks = []
        for hh in range(2):
            for s in range(2):
                def ch(hh=hh, s=s):
                    h = 2 * jj + hh
                    base = hh * 64
                    ps = mm_psum()[:HD + 1, :]
                    for kt in range(KT):
                        nc.tensor.matmul(
                            ps, v_sb[:, kt, h, :],
                            pp[:, kt,
                               hh * 1024 + s * 512:hh * 1024 + (s + 1) * 512],
                            start=(kt == 0), stop=False,
                            skip_group_check=True)
                    nc.tensor.matmul(
                        ps, vt_sb[:, h, :],
                        ppt[:, hh, s * 512:(s + 1) * 512],
                        start=False, stop=True, skip_group_check=True)
                    nc.vector.tensor_copy(
                        out=o_lnT[base:base + 64, jj, s * 512:(s + 1) * 512],
                        in_=ps[0:HD, :])
                    nc.vector.tensor_copy(
                        out=se_pr[:, hh, s * 512:(s + 1) * 512],
                        in_=ps[HD:HD + 1, :])
                chunks.append(ch)
        return chunks

    prev = None
    for j in range(8):      # head pairs
        pp = p_attn.tile([128, KT, 2048], BF16, tag="pp", name="pp")
        ppt = p_attn.tile([PT, 2, NTOK], BF16, tag="ppt", name="ppt")
        pend = attnv_chunks(*prev) if prev is not None else []
        for kt in range(KT):
            for hh in range(2):
                base = hh * 64
                psb = ps_big.tile([128, 1024], F32, tag="sc", name="psb")
                for s in range(2):
                    nc.tensor.matmul(
                        psb[:, s * 512:(s + 1) * 512],
                        kT[base:base + 64, j, kt * 128:(kt + 1) * 128],
                        qT[base:base + 64, j, s * 512:(s + 1) * 512],
                        start=True, stop=True, tile_position=(base, 0),
                        skip_group_check=True)
                nc.scalar.activation(
                    out=pp[:, kt, hh * 1024:(hh + 1) * 1024], in_=psb,
                    func=AF.Exp, scale=0.125)
            if kt % 2 == 0 and pend:
                pend.pop(0)()
        for hh in range(2):
            base = hh * 64
            psb = ps_big.tile([128, 1024], F32, tag="sc", name="psb")
            for s in range(2):
                nc.tensor.matmul(
                    psb[:PT, s * 512:(s + 1) * 512],
                    kT[base:base + 64, j, N:N + PT],
                    qT[base:base + 64, j, s * 512:(s + 1) * 512],
                    start=True, stop=True, tile_position=(base, 0),
                    skip_group_check=True)
            nc.scalar.activation(
                out=ppt[:, hh, :], in_=psb[:PT, :], func=AF.Exp,
                scale=0.125)
        for ch in pend:
            ch()
        if prev is not None:
            emit_normalize(prev[0], prev[3])
        se_pr = p_attn.tile([1, 2, NTOK], F32, tag="se_pr",
                            name="se_pr", bufs=1)
        prev = (j, pp, ppt, se_pr)
    # tail: attn@v + normalize of pair 7; its subln stats ride after the
    # first proj chain so they do not block the proj matmuls in the PE FIFO
    for ch in attnv_chunks(*prev):
        ch()
    oj7, sq7 = emit_normalize(7, prev[3], with_stats=False)
    attn_ctx.close()

    p_qkv.release()
    if "o_lnT" in dbg_d:
        nc.sync.dma_start(out=dbg_d["o_lnT"][:], in_=o_lnT[:])

    # =====================================================================
    # PH5+6: proj (deferred subln) + residual -> xnT, software-pipelined:
    # chain(ms) ... epilogue(ms-1); subln rows computed after chain 0;
    # LN2 stats interleaved per output tile.
    # =====================================================================
    p_xnT = tc.alloc_tile_pool(name="p_xnT", bufs=1)
    xnT = p_xnT.tile([128, DT, NTOK], F32, name="xnT")
    p_wB = tc.alloc_tile_pool(name="p_wB", bufs=1, side="right")
    wp_b = p_wB.tile([128, KT - KT // 2, D], BF16, name="wp_b")
    nc.sync.dma_start(out=wp_b, in_=wp_d[:, KT // 2:, :])
    wpsum_st = consts.tile([128, DT], F32, name="wpsum_st")
    nc.sync.dma_start(out=wpsum_st,
                      in_=pvec["wpsum"].rearrange("(o p) -> p o", p=128))
    bc = {}

    # reload x^T for the residual (straight from the pre-transposed input)
    p_xTr = tc.alloc_tile_pool(name="p_xTr", bufs=1)
    xTr = p_xTr.tile([128, DT, NTOK], XDT, name="xTr")
    nc.sync.dma_start(out=xTr, in_=x_d[:, :, :])

    skip_bp = "bp" in triv
    sum2_row = rows.tile([1, NTOK], F32, tag="sum_row", name="sum2_row")
    sq2_row = rows.tile([1, NTOK], F32, tag="sq_row", name="sq2_row")

    def emit_subln_rows():
        _, s_sub, negm_sub = ln_rows(sumO_row, sqO_row, D)
        bc["ssub"] = bcast_b(s_sub)
        ns_sub = rows.tile([1, NTOK], BF16, tag="nsrow", name="ns_sub",
                           bufs=1)
        nc.vector.tensor_tensor(ns_sub, negm_sub, s_sub, ALU.mult)
        bc["nsub"] = bcast_b(ns_sub)

    def proj_epilogue(m, s, ps):
        corr_m = tmps.tile([128, 512], BF16, tag="corr", name="corr_m",
                           bufs=2)
        nc.vector.tensor_scalar_mul(
            out=corr_m, in0=bc["nsub"][:, s * 512:(s + 1) * 512],
            scalar1=wpsum_st[:, m:m + 1])
        nc.vector.tensor_tensor(
            ps, ps, bc["ssub"][:, s * 512:(s + 1) * 512], ALU.mult)
        nc.vector.tensor_tensor(ps, ps, corr_m, ALU.add)
        if not skip_bp:
            nc.vector.tensor_scalar(
                out=ps, in0=ps, scalar1=st["bp"][:, m:m + 1],
                scalar2=None, op0=ALU.add)
        nc.vector.tensor_tensor(
            xnT[:, m, s * 512:(s + 1) * 512], ps,
            xTr[:, m, s * 512:(s + 1) * 512], ALU.add)

    def emit_ln2_tile_stats(m, hold):
        cst = sqp.tile([128, NTOK], BF16, tag="cst", name="cst2")
        nc.scalar.copy(out=cst, in_=xnT[:, m, :])
        sq_t = sqp.tile([128, NTOK], BF16, tag="sq_t", name="sq_t2")
        nc.vector.tensor_tensor(sq_t, cst, cst, ALU.mult)
        for s in range(2):
            nc.tensor.matmul(
                hold[0][s], ones_b, cst[:, s * 512:(s + 1) * 512],
                start=(m == 0), stop=(m == DT - 1), skip_group_check=True)
            nc.tensor.matmul(
                hold[1][s], ones_b, sq_t[:, s * 512:(s + 1) * 512],
                start=(m == 0), stop=(m == DT - 1), skip_group_check=True)

    with tc.tile_pool(name="ps_ln2", bufs=1, space="PSUM") as ps_ln2:
        hold = [[ps_ln2.tile([1, 512], F32, tag=f"l2{a}{s}",
                             name=f"l2{a}{s}") for s in range(2)]
                for a in range(2)]
        pending_ep = None
        for ms in range(16):
            m, s = divmod(ms, 2)
            ps = mm_psum()
            for o in range(KT):
                wsrc = (wp_a[:, o, :] if o < KT // 2
                        else wp_b[:, o - KT // 2, :])
                nc.tensor.matmul(
                    ps, wsrc[:, m * 128:(m + 1) * 128],
                    o_lnT[:, o, s * 512:(s + 1) * 512],
                    start=(o == 0), stop=(o == KT - 1))
            if ms == 0:
                emit_subln_stats(oj7, sq7)
                emit_subln_rows()
            if pending_ep is not None:
                proj_epilogue(*pending_ep)
                if pending_ep[1] == 1:
                    emit_ln2_tile_stats(pending_ep[0], hold)
            pending_ep = (m, s, ps)
        proj_epilogue(*pending_ep)
        emit_ln2_tile_stats(DT - 1, hold)
        for s in range(2):
            nc.scalar.copy(out=sum2_row[:, s * 512:(s + 1) * 512],
                           in_=hold[0][s])
            nc.scalar.copy(out=sq2_row[:, s * 512:(s + 1) * 512],
                           in_=hold[1][s])
    p_xTr.release()
    p_wB.release()
    p_OlnT.release()
    p_wA.release()
    if "xnT" in dbg_d:
        nc.sync.dma_start(out=dbg_d["xnT"][:], in_=xnT[:])

    # =====================================================================
    # PH7: LN2 -> x2T (bf16)
    # =====================================================================
    p_x2T = tc.alloc_tile_pool(name="p_x2T", bufs=1)
    x2T = p_x2T.tile([128, DT, NTOK], BF16, name="x2T")
    m_r, s_r, negm_r = ln_rows(sum2_row, sq2_row, D)
    fm_ln_apply(xnT, x2T, DT, negm_r, s_r, "n2")
    if "x2T" in dbg_d:
        nc.sync.dma_start(out=dbg_d["x2T"][:], in_=x2T[:])

    # =====================================================================
    # PH8: fc1 + gelu -> hT (bf16), with fused ffn_ln stats
    # =====================================================================
    p_hT = tc.alloc_tile_pool(name="p_hT", bufs=1, side="right")
    hT = p_hT.tile([128, HT, NTOK], BF16, name="hT")
    hsum_row = rows.tile([1, NTOK], F32, tag="sum_row", name="hsum_row")
    hsq_row = rows.tile([1, NTOK], F32, tag="sq_row", name="hsq_row")
    skip_b1 = "b1" in triv
    # ffn_ln stats via held accumulating ones-matmuls on the PE (the old DVE
    # accumulation made Vector the fc1 bottleneck at 93% busy); squares on DVE
    # (bf16, cheap), cross-partition sums ride 4 held psum banks.
    with tc.tile_pool(name="p_wblk", bufs=3) as p_wblk, \
         tc.tile_pool(name="ps_ffn", bufs=1, space="PSUM") as ps_ffn:
        ps_sum = [ps_ffn.tile([1, 512], F32, tag=f"ffsum{s}", name=f"ffsum{s}")
                  for s in range(2)]
        ps_sq = [ps_ffn.tile([1, 512], F32, tag=f"ffsq{s}", name=f"ffsq{s}")
                 for s in range(2)]
        for hm in range(HT):
            w1blk = p_wblk.tile([128, KT, 128], BF16, tag="w1blk",
                                name="w1blk")
            nc.sync.dma_start(out=w1blk, in_=w1_d[hm])
            for s in range(2):
                ps = mm_psum()
                for o in range(KT):
                    nc.tensor.matmul(
                        ps, w1blk[:, o, :],
                        x2T[:, o, s * 512:(s + 1) * 512],
                        start=(o == 0), stop=(o == KT - 1))
                hslice = hT[:, hm, s * 512:(s + 1) * 512]
                if skip_b1:
                    nc.scalar.activation(out=hslice, in_=ps, func=AF.Gelu)
                else:
                    nc.scalar.activation(
                        out=hslice, in_=ps, func=AF.Gelu,
                        bias=st["b1"][:, hm:hm + 1], scale=1.0)
                sq_t = sqp.tile([128, NTOK], BF16, tag="sq_t",
                                name="sq_tf")[:, :512]
                nc.vector.tensor_tensor(sq_t, hslice, hslice, ALU.mult)
                nc.tensor.matmul(
                    ps_sum[s], ones_b, hslice,
                    start=(hm == 0), stop=(hm == HT - 1),
                    skip_group_check=True)
                nc.tensor.matmul(
                    ps_sq[s], ones_b, sq_t,
                    start=(hm == 0), stop=(hm == HT - 1),
                    skip_group_check=True)
        for s in range(2):
            nc.scalar.copy(out=hsum_row[:, s * 512:(s + 1) * 512],
                           in_=ps_sum[s])
            nc.scalar.copy(out=hsq_row[:, s * 512:(s + 1) * 512],
                           in_=ps_sq[s])
    p_x2T.release()
    if "hT" in dbg_d:
        nc.sync.dma_start(out=dbg_d["hT"][:], in_=hT[:])

    # =====================================================================
    # PH9: ffn_ln rows only (normalization deferred into fc2: an extra K=1
    # matmul row adds -mean*colsum(W2); psum scaled by rstd in the epilogue)
    # =====================================================================
    _, s_ffn, negm_ffn = ln_rows(hsum_row, hsq_row, HID)
    sffn_b = bcast_b(s_ffn)
    w2sum_st = consts.tile([128, DT], F32, name="w2sum_st")
    nc.sync.dma_start(out=w2sum_st,
                      in_=pvec["w2sum"].rearrange("(o p) -> p o", p=128))
    ns_ffn = rows.tile([1, NTOK], BF16, tag="nsrow", name="ns_ffn", bufs=1)
    nc.vector.tensor_tensor(ns_ffn, negm_ffn, s_ffn, ALU.mult)
    nffn_b = bcast_b(ns_ffn)

    # =====================================================================
    # PH10: fc2 (with deferred ffn_ln) + residual + transpose + store
    # =====================================================================
    skip_b2 = "b2" in triv
    with tc.tile_pool(name="p_w2blk", bufs=3) as p_w2blk, \
         tc.tile_pool(name="p_out", bufs=4) as p_out:
        for m in range(DT):
            w2blk = p_w2blk.tile([128, HT, 128], BF16, tag="w2blk",
                                 name="w2blk")
            nc.sync.dma_start(out=w2blk, in_=w2_d[m])
            for s in range(2):
                corr2 = tmps.tile([128, 512], BF16, tag="corr",
                                  name="corr2", bufs=2)
                nc.vector.tensor_scalar_mul(
                    out=corr2, in0=nffn_b[:, s * 512:(s + 1) * 512],
                    scalar1=w2sum_st[:, m:m + 1])
                outm = p_out.tile([128, 512], F32, tag="outm", name="outm")
                ps = mm_psum()
                for o in range(HT):
                    nc.tensor.matmul(
                        ps, w2blk[:, o, :],
                        hT[:, o, s * 512:(s + 1) * 512],
                        start=(o == 0), stop=(o == HT - 1))
                nc.vector.tensor_tensor(
                    ps, ps, sffn_b[:, s * 512:(s + 1) * 512], ALU.mult)
                nc.vector.tensor_tensor(ps, ps, corr2, ALU.add)
                if not skip_b2:
                    nc.vector.tensor_scalar(
                        out=ps, in0=ps, scalar1=st["b2"][:, m:m + 1],
                        scalar2=None, op0=ALU.add)
                nc.vector.tensor_tensor(
                    outm, ps, xnT[:, m, s * 512:(s + 1) * 512], ALU.add)
                nc.sync.dma_start(
                    out=y_d[:, m, s * 512:(s + 1) * 512], in_=outm)
    p_hT.release()
    p_xnT.release()
    ctx.close()


# --------------------------------------------------------------------------
# host glue
# --------------------------------------------------------------------------

_PROGRAM_CACHE = {}


def get_program(debug=(), triv=frozenset()):
    key = (tuple(sorted(debug)), tuple(sorted(triv)))
    if key not in _PROGRAM_CACHE:
        _PROGRAM_CACHE[key] = build_program(debug=key[0], triv=key[1])
    return _PROGRAM_CACHE[key]


def compute_triv(inputs):
    f32 = np.float32
    triv = set()
    for k in ["n1", "n3", "ln", "n2", "ffn"]:
        g = np.asarray(inputs[k + "_g"], f32)
        b = np.asarray(inputs[k + "_b"], f32)
        if np.all(g == 1.0) and np.all(b == 0.0):
            triv.add(k)
    for k in ["bq", "bv", "b1"]:
        if np.all(np.asarray(inputs[k], f32) == 0.0):
            triv.add(k)
    bp_eff = (np.asarray(inputs["bp"], f32)
              + np.asarray(inputs["ln_b"], f32) @ np.asarray(inputs["Wp"], f32))
    if np.all(bp_eff == 0.0):
        triv.add("bp")
    b2_eff = (np.asarray(inputs["b2"], f32)
              + np.asarray(inputs["ffn_b"], f32) @ np.asarray(inputs["W2"], f32))
    if np.all(b2_eff == 0.0):
        triv.add("b2")
    return frozenset(triv)


def make_in_maps(inputs):
    """Build the 8 per-core input maps from the full-problem input dict."""
    bf = ml_dtypes.bfloat16
    f32 = np.float32

    def host(name):
        return np.asarray(inputs[name], dtype=f32)

    # fold the subln (ln_g/ln_b) into Wp/bp and the ffn_ln (ffn_g/ffn_b)
    # into W2/b2 — the kernel defers those norms into the matmuls and only
    # applies (x-mean)*rstd
    wp_eff = host("ln_g")[:, None] * host("Wp")
    bp_eff = host("bp") + host("ln_b") @ host("Wp")
    w2_eff = host("ffn_g")[:, None] * host("W2")
    b2_eff = host("b2") + host("ffn_b") @ host("W2")
    wp_bf = wp_eff.astype(bf)
    w2_bf = w2_eff.astype(bf)
    shared = dict(
        wq=np.ascontiguousarray(
            host("Wq").reshape(KT, 128, D).astype(bf).transpose(1, 0, 2)),
        wk=np.ascontiguousarray(
            host("Wk").reshape(KT, 128, D).astype(bf).transpose(1, 0, 2)),
        wv=np.ascontiguousarray(
            host("Wv").reshape(KT, 128, D).astype(bf).transpose(1, 0, 2)),
        wp=np.ascontiguousarray(
            wp_bf.reshape(KT, 128, D).transpose(1, 0, 2)),
        wpsum=wp_bf.astype(np.float32).sum(0).astype(f32),
        w1=np.ascontiguousarray(
            host("W1").reshape(KT, 128, HT, 128).transpose(2, 1, 0, 3)
        ).astype(bf),
        w2=np.ascontiguousarray(
            w2_bf.reshape(HT, 128, DT, 128).transpose(2, 1, 0, 3)),
        w2sum=w2_bf.astype(np.float32).sum(0).astype(f32),
        n1_g=host("n1_g"), n1_b=host("n1_b"),
        n3_g=host("n3_g"), n3_b=host("n3_b"),
        bq=host("bq"), bv=host("bv"),
        ln_g=host("ln_g"), ln_b=host("ln_b"),
        bp=bp_eff.astype(f32),
        n2_g=host("n2_g"), n2_b=host("n2_b"),
        b1=host("b1"), ffn_g=host("ffn_g"), ffn_b=host("ffn_b"),
        b2=b2_eff.astype(f32),
        gate=host("gate").reshape(H),
    )
    x = host("x")
    xt = host("x_text")
    in_maps = []
    for b in range(B):
        m = dict(shared)
        # device consumes feature-major, partition-major inputs
        m["x"] = np.ascontiguousarray(
            x[b].T.reshape(DT, 128, N).transpose(1, 0, 2)).astype(bf)
        m["x_text"] = np.ascontiguousarray(
            xt[b].T.reshape(DT, 128, PT).transpose(1, 0, 2)).astype(bf)
        in_maps.append(m)
    return in_maps


def unpack_y(y):
    """Device output is feature-major [128, DT, NTOK]; back to [NTOK, D]."""
    y = np.asarray(y)
    return np.transpose(y, (2, 1, 0)).reshape(NTOK, D)


def kernel(**inputs) -> np.ndarray:
    from concourse.bass_utils import run_bass_kernel_spmd

    nc = get_program(triv=compute_triv(inputs))
    in_maps = make_in_maps(inputs)
    res = run_bass_kernel_spmd(nc, in_maps, list(range(B)))
    out = np.stack([unpack_y(res.results[b]["y"]) for b in range(B)], axis=0)
    return out.astype(np.float32)



# revision 49
# speedup vs baseline: 1.2809x; 1.0112x over previous
"""Trainium2 Bass kernel for nn_Block_45724221833665 (dense transformer block).

Strategy: pure data-parallel over batch — 8 batch elements, 8 NeuronCores, one
batch element per core, no collectives.  Inside a core everything is computed
"feature-major" (features on SBUF partitions, tokens on the free dim) so that:
  * all matmuls consume operands in their natural layout (weights as lhsT),
  * all per-feature biases/gains are per-partition scalars,
  * attention needs no transposes at all (scores are computed as S^T, attn@v
    takes exp(S^T) directly as the moving operand with V as the stationary
    operand, and the per-token softmax denominator comes from an extra all-ones
    column appended to V).
Inputs arrive host-pre-transposed (feature-major) and the output is returned
feature-major and transposed back on the host, so the device does no
transposes at all.

LayerNorms in feature-major form need cross-partition sums; those are done on
the tensor engine with an all-ones stationary vector over bf16 casts, and the
per-token mean/rstd rows are broadcast across partitions by DMA via a small
DRAM bounce.  Reciprocals run on ACT (the DVE iterative divide is ~8
cycles/element and our rows live on one partition); the LN rstd gets one
Newton step on DVE to recover full precision.

The gated dual softmax simplifies: softmax over a single key (column 0) is
identically 1, so the reference's w_prefix column is just tanh(gate) per head.
Attention output = softmax over the other 1087 keys @ v  +  tanh(gate)*v[key0]
(+ (1+tanh(gate))*bv once the v-bias is folded out of the softmax-weighted sum,
since the main softmax weights sum to 1).

Gains that are identically 1 and biases identically 0 (the common case for
this block) are detected on the host and their application elided; the cache
key of the built program includes those flags, so nontrivial parameters still
take the general path.
"""

import numpy as np
import ml_dtypes

import concourse.bass as bass
import concourse.mybir as mybir
import concourse.tile as tile
from concourse import bacc

B, N, PT, D, H, HD, HID = 8, 1024, 64, 1024, 16, 64, 4096
KT = D // 128          # 8 contraction tiles over D
HT = HID // 128        # 32 tiles over HID
DT = D // 128          # 8 output tiles over D
NTOK = N               # 1024 main tokens per core
NKEY = N + PT          # 1088 keys (main tokens + text prefix)
EPS = 1e-5

F32 = mybir.dt.float32
BF16 = mybir.dt.bfloat16
XDT = BF16  # wire dtype of the (host-pre-transposed) x / x_text inputs
AF = mybir.ActivationFunctionType
ALU = mybir.AluOpType


def build_program(debug=(), triv=frozenset()):
    """Build the single-core Bass program.
    debug: iterable of intermediate names to emit as DRAM outputs.
    triv: parameter groups whose gain/bias application can be skipped."""
    nc = bacc.Bacc("TRN2")
    dbg = set(debug)
    triv = set(triv)

    # ---- I/O ------------------------------------------------------------
    # x / x_text arrive HOST-PRE-TRANSPOSED (feature-major [D, tokens])
    x_d = nc.declare_dram_parameter("x", [128, DT, NTOK], XDT, isOutput=False)
    xt_d = nc.declare_dram_parameter("x_text", [128, DT, PT], XDT,
                                     isOutput=False)
    wq_d = nc.declare_dram_parameter("wq", [128, KT, D], BF16, isOutput=False)
    wk_d = nc.declare_dram_parameter("wk", [128, KT, D], BF16, isOutput=False)
    wv_d = nc.declare_dram_parameter("wv", [128, KT, D], BF16, isOutput=False)
    wp_d = nc.declare_dram_parameter("wp", [128, KT, D], BF16, isOutput=False)
    w1_d = nc.declare_dram_parameter("w1", [HT, 128, KT, 128], BF16,
                                 isOutput=False)
    w2_d = nc.declare_dram_parameter("w2", [DT, 128, HT, 128], BF16,
                                     isOutput=False)
    pvec = {}
    for nm, sz in [
        ("n1_g", D), ("n1_b", D), ("n3_g", D), ("n3_b", D),
        ("bq", D), ("bv", D), ("ln_g", D), ("ln_b", D), ("bp", D),
        ("n2_g", D), ("n2_b", D), ("b1", HID), ("ffn_g", HID), ("ffn_b", HID),
        ("b2", D),
    ]:
        pvec[nm] = nc.declare_dram_parameter(nm, [sz], F32, isOutput=False)
    pvec["wpsum"] = nc.declare_dram_parameter("wpsum", [D], F32,
                                              isOutput=False)
    pvec["w2sum"] = nc.declare_dram_parameter("w2sum", [D], F32,
                                              isOutput=False)
    gate_d = nc.declare_dram_parameter("gate", [H], F32, isOutput=False)
    # y is produced feature-major [128, DT, NTOK]; host transposes back
    y_d = nc.declare_dram_parameter("y", [128, DT, NTOK], F32, isOutput=True)

    dbg_d = {}

    def dbg_out(name, shape, dtype):
        if name in dbg:
            dbg_d[name] = nc.declare_dram_parameter(
                "dbg_" + name, list(shape), dtype, isOutput=True
            )

    dbg_out("xT", [128, DT, NTOK], F32)
    dbg_out("x1T", [128, DT, NTOK], BF16)
    dbg_out("xt1T", [128, DT, PT], BF16)
    dbg_out("qT", [128, DT, NTOK], BF16)
    dbg_out("kT", [128, DT, NKEY], BF16)
    dbg_out("v", [128, KT, H, HD + 1], BF16)
    dbg_out("v_text", [PT, H, HD + 1], BF16)
    dbg_out("c_vec", [D], F32)
    dbg_out("pp0", [128, 2, KT, NTOK], BF16)     # exp(S^T) for heads 0,1
    dbg_out("pp0_text", [PT, 2, NTOK], BF16)
    dbg_out("o_full", [128, DT, NTOK], F32)
    dbg_out("o_lnT", [128, DT, NTOK], BF16)
    dbg_out("xnT", [128, DT, NTOK], F32)
    dbg_out("x2T", [128, DT, NTOK], BF16)
    dbg_out("hT", [128, HT, NTOK], BF16)

    with tile.TileContext(nc) as tc:
        _build_phases(nc, tc, x_d, xt_d, wq_d, wk_d, wv_d, wp_d,
                      w1_d, w2_d, pvec, gate_d, y_d, dbg_d, triv)
    nc.compile()
    return nc


def _build_phases(nc, tc, x_d, xt_d, wq_d, wk_d, wv_d, wp_d, w1_d, w2_d,
                  pvec, gate_d, y_d, dbg_d, triv):
    import contextlib
    ctx = contextlib.ExitStack()
    # ---- whole-kernel pools ---------------------------------------------
    consts = ctx.enter_context(tc.tile_pool(name="consts", bufs=1))
    rows = ctx.enter_context(tc.tile_pool(name="rows", bufs=1))
    tmps = ctx.enter_context(tc.tile_pool(name="tmps", bufs=2))
    sqp = ctx.enter_context(tc.tile_pool(name="sqp", bufs=2))
    dram = ctx.enter_context(tc.tile_pool(name="dram", bufs=1, space="DRAM"))
    ps_mm = ctx.enter_context(tc.tile_pool(name="ps_mm", bufs=3, space="PSUM"))
    # p_qkv sits below p_x1T on the pool stack so x1T can be freed after the
    # projections while qT/kT/v live on through attention
    p_qkv = tc.alloc_tile_pool(name="p_qkv", bufs=1)
    p_x1T = tc.alloc_tile_pool(name="p_x1T", bufs=1)

    def mm_psum(pfree=512, parts=128):
        t = ps_mm.tile([128, 512], F32, tag="mm", name="mmps")
        return t[:parts, :pfree]

    # ---- constants ------------------------------------------------------
    ones_b = consts.tile([128, 1], BF16, name="ones_b")
    nc.vector.memset(ones_b, 1.0)
    ones_r = consts.tile([1, 128], BF16, name="ones_r")
    nc.vector.memset(ones_r, 1.0)
    eps_c = consts.tile([1, 1], F32, name="eps_c")
    nc.vector.memset(eps_c, EPS)

    st = {}  # striped parameter tiles [128, tiles]; triv params never read,
    # so skip their DMAs (each is 128 tiny descriptors clogging the queues)
    def group_used(nm):
        g = nm.split("_")[0] if "_" in nm else nm
        return g not in triv
    for nm in ["n1_g", "n1_b", "n3_g", "n3_b", "bq", "ln_g", "ln_b", "bp",
               "n2_g", "n2_b", "b2"]:
        if not group_used(nm):
            continue
        t = consts.tile([128, DT], F32, name="st_" + nm)
        nc.sync.dma_start(out=t, in_=pvec[nm].rearrange("(o p) -> p o", p=128))
        st[nm] = t
    for nm in ["b1", "ffn_g", "ffn_b"]:
        if not group_used(nm):
            continue
        t = consts.tile([128, HT], F32, name="st_" + nm)
        nc.sync.dma_start(out=t, in_=pvec[nm].rearrange("(o p) -> p o", p=128))
        st[nm] = t

    def bcast_b(row, ntok=NTOK):
        """Broadcast a [1, ntok] row to a [128, ntok] bf16 tile via a K=1
        PE outer product with an all-ones stationary (the PE is idle at the
        LN phase boundaries where these sit, and its latency is ~1us vs the
        multi-us dispatch stalls seen on GPSIMD/DMA-bounce paths)."""
        if row.dtype != BF16:
            br = rows.tile([1, NTOK], BF16, tag="brow", name="brow",
                           bufs=1)[:, :ntok]
            nc.vector.tensor_copy(out=br, in_=row)
            row = br
        out = tmps.tile([128, NTOK], BF16, tag="wrk", name="bb")[:, :ntok]
        nsl = max(1, ntok // 512)
        sl = ntok // nsl
        for s in range(nsl):
            ps = mm_psum(sl)
            nc.tensor.matmul(ps, ones_r, row[:, s * sl:(s + 1) * sl],
                             start=True, stop=True, skip_group_check=True)
            nc.vector.tensor_copy(out=out[:, s * sl:(s + 1) * sl], in_=ps)
        return out

    def ln_rows(sum_row, sq_row, n_elems):
        """Turn per-token sums into (mean, rstd, -mean[bf16]) rows.  rstd =
        ACT abs_reciprocal_sqrt(var+eps) refined by one rsqrt-Newton step on
        DVE (the banned-for-accuracy ACT seed is fine once refined)."""
        ntok = sum_row.shape[-1]
        t_row = rows.tile([1, NTOK], F32, tag="t_row", name="t_row")[:, :ntok]
        s_row = rows.tile([1, NTOK], F32, tag="s_row", name="s_row")[:, :ntok]
        u_row = rows.tile([1, NTOK], F32, tag="u_row", name="u_row",
                          bufs=1)[:, :ntok]
        negm = rows.tile([1, NTOK], BF16, tag="negm", name="negm")[:, :ntok]
        inv = 1.0 / float(n_elems)
        nc.vector.tensor_scalar_mul(out=sum_row, in0=sum_row, scalar1=inv)
        nc.vector.tensor_scalar_mul(out=sq_row, in0=sq_row, scalar1=inv)
        nc.vector.tensor_tensor(t_row, sum_row, sum_row, ALU.mult)  # mean^2
        nc.vector.tensor_tensor(sq_row, sq_row, t_row, ALU.subtract)  # var
        nc.scalar.activation(out=s_row, in_=sq_row,
                             func=AF.Abs_reciprocal_sqrt,
                             bias=eps_c, scale=1.0)                 # ~rstd
        # rsqrt Newton: s <- s*(1.5 - 0.5*(var+eps)*s^2)
        nc.vector.tensor_scalar_add(out=t_row, in0=sq_row, scalar1=EPS)
        nc.vector.tensor_tensor(u_row, s_row, s_row, ALU.mult)
        nc.vector.tensor_tensor(u_row, u_row, t_row, ALU.mult)
        nc.vector.tensor_scalar(out=u_row, in0=u_row, scalar1=-0.5,
                                scalar2=1.5, op0=ALU.mult, op1=ALU.add)
        nc.vector.tensor_tensor(s_row, s_row, u_row, ALU.mult)
        nc.vector.tensor_scalar_mul(out=negm, in0=sum_row, scalar1=-1.0)
        return sum_row, s_row, negm

    # =====================================================================
    # PH1: load x / x_text directly in feature-major layout (host-transposed)
    # =====================================================================
    x1T = p_x1T.tile([128, DT, NTOK], BF16, name="x1T")
    p_xtB = tc.alloc_tile_pool(name="p_xtB", bufs=1)
    xt1T = p_xtB.tile([128, DT, PT], BF16, name="xt1T")
    p_xtA = tc.alloc_tile_pool(name="p_xtA", bufs=1)
    xtT = p_xtA.tile([128, DT, PT], XDT, name="xtT")
    p_xT = tc.alloc_tile_pool(name="p_xT", bufs=1)
    xT = p_xT.tile([128, DT, NTOK], XDT, name="xT")

    nc.sync.dma_start(out=xtT, in_=xt_d[:, :, :])
    for hf in range(2):  # two half-chunks: big descriptors, early stats
        nc.sync.dma_start(
            out=xT[:, hf * 4:(hf + 1) * 4, :],
            in_=x_d[:, hf * 4:(hf + 1) * 4, :])
    if "xT" in dbg_d:
        nc.sync.dma_start(out=dbg_d["xT"][:], in_=xT[:])

    # ---- feature-major layernorm helpers --------------------------------
    def fm_ln_stats(src, ntiles, ntok, sl=512):
        """Cross-partition LN stats for src [128, ntiles, ntok] (f32 or bf16).
        Returns (sum_row, sq_row) [1, ntok] f32 rows (in `rows` pool)."""
        sum_row = rows.tile([1, NTOK], F32, tag="sum_row",
                            name="sum_row")[:, :ntok]
        sq_row = rows.tile([1, NTOK], F32, tag="sq_row",
                           name="sq_row")[:, :ntok]
        nsl = ntok // sl
        with tc.tile_pool(name="ps_stat", bufs=4, space="PSUM") as ps_stat:
            ps_a = [ps_stat.tile([1, sl], F32, tag="stat", name=f"psa{s}",
                                 bufs=2) for s in range(nsl)]
            ps_b = [ps_stat.tile([1, sl], F32, tag="stat2", name=f"psb{s}",
                                 bufs=2) for s in range(nsl)]
            for o in range(ntiles):
                if src.dtype == F32:
                    cst = sqp.tile([128, NTOK], BF16, tag="cst",
                                   name="cst")[:, :ntok]
                    nc.scalar.copy(out=cst, in_=src[:, o, :])
                else:
                    cst = src[:, o, :]
                sq_t = sqp.tile([128, NTOK], BF16, tag="sq_t",
                                name="sq_t")[:, :ntok]
                nc.vector.tensor_tensor(sq_t, cst, cst, ALU.mult)
                for s in range(nsl):
                    nc.tensor.matmul(
                        ps_a[s], ones_b, cst[:, s * sl:(s + 1) * sl],
                        start=(o == 0), stop=(o == ntiles - 1),
                        skip_group_check=True)
                    nc.tensor.matmul(
                        ps_b[s], ones_b, sq_t[:, s * sl:(s + 1) * sl],
                        start=(o == 0), stop=(o == ntiles - 1),
                        skip_group_check=True)
            for s in range(nsl):
                nc.scalar.copy(out=sum_row[:, s * sl:(s + 1) * sl], in_=ps_a[s])
                nc.scalar.copy(out=sq_row[:, s * sl:(s + 1) * sl], in_=ps_b[s])
        return sum_row, sq_row

    def fm_ln_apply(src, dst, ntiles, negm_row, s_row, gkey):
        """dst[:,o,:] = ((src - m)*rstd)[*g + b], slice-pipelined so
        consumers ordered slice-outer can start on slice 0 early."""
        ntok = src.shape[-1]
        g_st = st.get(gkey + "_g")
        b_st = st.get(gkey + "_b")
        skip_gb = gkey in triv
        m_b = bcast_b(negm_row, ntok)
        s_b = bcast_b(s_row, ntok)
        nsl = max(1, ntok // 512)
        sl = ntok // nsl
        for s in range(nsl):
            ss = slice(s * sl, (s + 1) * sl)
            for o in range(ntiles):
                tA = tmps.tile([128, 512], BF16, tag="tA",
                               name="tA")[:, :sl]
                nc.vector.tensor_tensor(tA, src[:, o, ss], m_b[:, ss],
                                        ALU.add)
                if skip_gb:
                    nc.vector.tensor_tensor(dst[:, o, ss], tA, s_b[:, ss],
                                            ALU.mult)
                else:
                    nc.vector.tensor_tensor(tA, tA, s_b[:, ss], ALU.mult)
                    nc.scalar.activation(
                        out=dst[:, o, ss], in_=tA, func=AF.Identity,
                        scale=g_st[:, o:o + 1], bias=b_st[:, o:o + 1])

    # =====================================================================
    # PH2: LN1(x) -> x1T (bf16), LN3(x_text) -> xt1T (bf16)
    # =====================================================================
    sum_r, sq_r = fm_ln_stats(xtT, DT, PT, sl=PT)
    m_r, s_r, negm_r = ln_rows(sum_r, sq_r, D)
    fm_ln_apply(xtT, xt1T, DT, negm_r, s_r, "n3")

    sum_r, sq_r = fm_ln_stats(xT, DT, NTOK)
    m_r, s_r, negm_r = ln_rows(sum_r, sq_r, D)
    fm_ln_apply(xT, x1T, DT, negm_r, s_r, "n1")
    p_xT.release()
    p_xtA.release()

    if "x1T" in dbg_d:
        nc.sync.dma_start(out=dbg_d["x1T"][:], in_=x1T[:])
    if "xt1T" in dbg_d:
        nc.sync.dma_start(out=dbg_d["xt1T"][:], in_=xt1T[:])

    # =====================================================================
    # PH3: projections  Q^T, K^T (feature-major), V (token-major, +ones col)
    # =====================================================================
    qT = p_qkv.tile([128, DT, NTOK], BF16, name="qT")
    kT = p_qkv.tile([128, DT, NKEY], BF16, name="kT")
    v_sb = p_qkv.tile([128, KT, H, HD + 1], BF16, name="v_sb")
    vt_sb = p_qkv.tile([PT, H, HD + 1], BF16, name="vt_sb")

    p_w = tc.alloc_tile_pool(name="p_w", bufs=2)
    wq_sb = p_w.tile([128, KT, D], BF16, tag="wfull", name="wq_sb")
    nc.sync.dma_start(out=wq_sb, in_=wq_d[:, :, :])
    wk_sb = p_w.tile([128, KT, D], BF16, tag="wfull", name="wk_sb")
    nc.sync.dma_start(out=wk_sb, in_=wk_d[:, :, :])
    skip_bq = "bq" in triv
    for s in range(2):
        for m in range(DT):
            ps = mm_psum()
            for o in range(KT):
                nc.tensor.matmul(
                    ps, wq_sb[:, o, m * 128:(m + 1) * 128],
                    x1T[:, o, s * 512:(s + 1) * 512],
                    start=(o == 0), stop=(o == KT - 1))
            if skip_bq:
                nc.scalar.copy(out=qT[:, m, s * 512:(s + 1) * 512], in_=ps)
            else:
                nc.scalar.activation(
                    out=qT[:, m, s * 512:(s + 1) * 512], in_=ps,
                    func=AF.Identity, bias=st["bq"][:, m:m + 1], scale=1.0)
        for m in range(DT):
            ps = mm_psum()
            for o in range(KT):
                nc.tensor.matmul(
                    ps, wk_sb[:, o, m * 128:(m + 1) * 128],
                    x1T[:, o, s * 512:(s + 1) * 512],
                    start=(o == 0), stop=(o == KT - 1))
            nc.scalar.copy(out=kT[:, m, s * 512:(s + 1) * 512], in_=ps)
    for m in range(DT):  # K^T text keys
        ps = mm_psum(PT)
        for o in range(KT):
            nc.tensor.matmul(
                ps, wk_sb[:, o, m * 128:(m + 1) * 128], xt1T[:, o, :],
                start=(o == 0), stop=(o == KT - 1))
        nc.scalar.copy(out=kT[:, m, N:N + PT], in_=ps)
    # V token-major, heads interleaved with the all-ones 65th column
    wv_sb = p_w.tile([128, KT, D], BF16, tag="wfull", name="wv_sb")
    nc.sync.dma_start(out=wv_sb, in_=wv_d[:, :, :])
    for t in range(8):
        for sn in range(2):
            ps = mm_psum()
            for o in range(KT):
                nc.tensor.matmul(
                    ps, x1T[:, o, t * 128:(t + 1) * 128],
                    wv_sb[:, o, sn * 512:(sn + 1) * 512],
                    start=(o == 0), stop=(o == KT - 1))
            nc.scalar.copy(
                out=v_sb[:, t, sn * 8:(sn + 1) * 8, 0:HD], in_=ps)
    for sn in range(2):
        ps = mm_psum(parts=PT)
        for o in range(KT):
            nc.tensor.matmul(
                ps, xt1T[:, o, :], wv_sb[:, o, sn * 512:(sn + 1) * 512],
                start=(o == 0), stop=(o == KT - 1))
        nc.scalar.copy(out=vt_sb[:, sn * 8:(sn + 1) * 8, 0:HD], in_=ps)
    p_w.release()
    p_xtB.release()
    p_x1T.release()

    # ---- c_vec: tanh(gate)*v0_raw + (1+tanh(gate))*bv  ------------------
    g_row = rows.tile([1, H], F32, tag="g_row", name="g_row")
    nc.sync.dma_start(out=g_row, in_=gate_d.rearrange("(a h) -> a h", a=1))
    th_row = rows.tile([1, H], F32, tag="th_row", name="th_row")
    nc.scalar.activation(out=th_row, in_=g_row, func=AF.Tanh)
    c_work = rows.tile([1, H, HD], F32, tag="t_row", name="c_work")
    nc.vector.tensor_copy(out=c_work, in_=vt_sb[0:1, :, 0:HD])
    nc.vector.tensor_tensor(
        c_work, c_work, th_row[:, :, None].to_broadcast((1, H, HD)), ALU.mult)
    if "bv" not in triv:
        th1_row = rows.tile([1, H], F32, tag="th1_row", name="th1_row")
        nc.scalar.activation(out=th1_row, in_=th_row, func=AF.Identity,
                             bias=1.0)
        bv_row = rows.tile([1, H, HD], F32, tag="s_row", name="bv_row")
        nc.sync.dma_start(
            out=bv_row, in_=pvec["bv"].rearrange("(a h d) -> a h d", a=1, h=H))
        nc.vector.tensor_tensor(
            bv_row, bv_row, th1_row[:, :, None].to_broadcast((1, H, HD)),
            ALU.mult)
        nc.vector.tensor_tensor(c_work, c_work, bv_row, ALU.add)
    c_dram = dram.tile([D], F32, name="c_dram")
    nc.sync.dma_start(
        out=c_dram.rearrange("(a h d) -> a h d", a=1, h=H), in_=c_work)
    c_st = consts.tile([128, DT], F32, name="c_st")
    nc.sync.dma_start(out=c_st, in_=c_dram.rearrange("(o p) -> p o", p=128))
    if "c_vec" in dbg_d:
        nc.sync.dma_start(out=dbg_d["c_vec"][:], in_=c_dram[:])

    # ones column + zero out reference-key-0 (first text token)
    nc.vector.memset(v_sb[:, :, :, HD:HD + 1], 1.0)
    nc.vector.memset(vt_sb[:, :, HD:HD + 1], 1.0)
    nc.vector.memset(vt_sb[0:1, :, :], 0.0)

    # prefetch the first half of the proj weight while attention runs
    p_wA = tc.alloc_tile_pool(name="p_wA", bufs=1, side="right")
    wp_a = p_wA.tile([128, KT // 2, D], BF16, name="wp_a")
    nc.sync.dma_start(out=wp_a, in_=wp_d[:, 0:KT // 2, :])

    # =====================================================================
    # PH4: attention — per (pair, kt): 4 score matmuls into one 4-bank
    # [128,2048] psum, ONE exp over all 2048 cols (amortizes the ~350-cycle
    # ACT ramp), attn@v of the previous pair interleaved into emission so the
    # PE fills the exp-wait bubbles; softmax reciprocal on DVE (no ACT table
    # switches); subln stats of the last pair deferred into the proj phase.
    # =====================================================================
    p_OlnT = tc.alloc_tile_pool(name="p_OlnT", bufs=1, side="right")
    o_lnT = p_OlnT.tile([128, DT, NTOK], BF16, name="o_lnT")
    sumO_row = rows.tile([1, NTOK], F32, tag="sum_row", name="sumO_row")
    sqO_row = rows.tile([1, NTOK], F32, tag="sq_row", name="sqO_row")
    nc.vector.memset(sumO_row, 0.0)
    nc.vector.memset(sqO_row, 0.0)

    def emit_subln_stats(oj, sq_t4):
        for s in range(2):
            ps_a = mm_psum()[:1, :]
            nc.tensor.matmul(
                ps_a, ones_b, oj[:, s * 512:(s + 1) * 512],
                start=True, stop=True, skip_group_check=True)
            nc.vector.tensor_tensor(
                sumO_row[:, s * 512:(s + 1) * 512],
                sumO_row[:, s * 512:(s + 1) * 512], ps_a, ALU.add)
            ps_b = mm_psum()[:1, :]
            nc.tensor.matmul(
                ps_b, ones_b, sq_t4[:, s * 512:(s + 1) * 512],
                start=True, stop=True, skip_group_check=True)
            nc.vector.tensor_tensor(
                sqO_row[:, s * 512:(s + 1) * 512],
                sqO_row[:, s * 512:(s + 1) * 512], ps_b, ALU.add)

    attn_ctx = contextlib.ExitStack()
    p_attn = attn_ctx.enter_context(tc.tile_pool(name="p_attn", bufs=2))
    ps_big = attn_ctx.enter_context(
        tc.tile_pool(name="ps_big", bufs=2, space="PSUM"))

    def emit_normalize(j, se_pr, with_stats=True):
        nc.vector.reciprocal_approx_fast(out=se_pr, in_=se_pr)
        se_b = rows.tile([1, 2, NTOK], BF16, tag="se_b", name="se_b",
                         bufs=1)
        nc.vector.tensor_copy(out=se_b, in_=se_pr)
        rb = tmps.tile([128, 2, NTOK], BF16, tag="rb", name="rb", bufs=1)
        nc.gpsimd.partition_broadcast(rb, se_b)
        oj = o_lnT[:, j, :]
        nc.vector.tensor_tensor(oj[0:64, :], oj[0:64, :], rb[0:64, 0, :],
                                ALU.mult)
        nc.vector.tensor_tensor(oj[64:128, :], oj[64:128, :],
                                rb[64:128, 1, :], ALU.mult)
        nc.vector.tensor_scalar_add(out=oj, in0=oj,
                                    scalar1=c_st[:, j:j + 1])
        sq_t4 = sqp.tile([128, NTOK], BF16, tag="sq_t", name="sq_t4")
        nc.vector.tensor_tensor(sq_t4, oj, oj, ALU.mult)
        if with_stats:
            emit_subln_stats(oj, sq_t4)
        return oj, sq_t4

    def attnv_chunks(jj, pp, ppt, se_pr):
        chunks = []
        for hh in range(2):
            for s in range(2):
                def ch(hh=hh, s=s):
                    h = 2 * jj + hh
                    base = hh * 64
                    ps = mm_psum()[:HD + 1, :]
                    for kt in range(KT):
                        nc.tensor.matmul(
                            ps, v_sb[:, kt, h, :],
                            pp[:, kt,
                               hh * 1024 + s * 512:hh * 1024 + (s + 1) * 512],
                            start=(kt == 0), stop=False,
                            skip_group_check=True)
                    nc.tensor.matmul(
                        ps, vt_sb[:, h, :],
                        ppt[:, hh, s * 512:(s + 1) * 512],
                        start=False, stop=True, skip_group_check=True)
                    nc.vector.tensor_copy(
                        out=o_lnT[base:base + 64, jj, s * 512:(s + 1) * 512],
                        in_=ps[0:HD, :])
                    nc.vector.tensor_copy(
                        out=se_pr[:, hh, s * 512:(s + 1) * 512],
                        in_=ps[HD:HD + 1, :])
                chunks.append(ch)
        return chunks

    prev = None
    for j in range(8):      # head pairs
        pp = p_attn.tile([128, KT, 2048], BF16, tag="pp", name="pp")
        ppt = p_attn.tile([PT, 2, NTOK], BF16, tag="ppt", name="ppt")
        pend = attnv_chunks(*prev) if prev is not None else []
        for kt in range(KT):
            for hh in range(2):
                base = hh * 64
                psb = ps_big.tile([128, 1024], F32, tag="sc", name="psb")
                for s in range(2):
                    nc.tensor.matmul(
                        psb[:, s * 512:(s + 1) * 512],
                        kT[base:base + 64, j, kt * 128:(kt + 1) * 128],
                        qT[base:base + 64, j, s * 512:(s + 1) * 512],
                        start=True, stop=True, tile_position=(base, 0),
                        skip_group_check=True)
                nc.scalar.activation(
                    out=pp[:, kt, hh * 1024:(hh + 1) * 1024], in_=psb,
                    func=AF.Exp, scale=0.125)
            if kt % 2 == 0 and pend:
                pend.pop(0)()
        for hh in range(2):
            base = hh * 64
            psb = ps_big.tile([128, 1024], F32, tag="sc", name="psb")
            for s in range(2):
                nc.tensor.matmul(
                    psb[:PT, s * 512:(s + 1) * 512],
                    kT[base:base + 64, j, N:N + PT],
                    qT[base:base + 64, j, s * 512:(s + 1) * 512],
                    start=True, stop=True, tile_position=(base, 0),
                    skip_group_check=True)
            nc.scalar.activation(
                out=ppt[:, hh, :], in_=psb[:PT, :], func=AF.Exp,
                scale=0.125)
        for ch in pend:
            ch()
        if prev is not None:
            emit_normalize(prev[0], prev[3])
        se_pr = p_attn.tile([1, 2, NTOK], F32, tag="se_pr",
                            name="se_pr", bufs=1)
        prev = (j, pp, ppt, se_pr)
    # tail: attn@v + normalize of pair 7; its subln stats ride after the
    # first proj chain so they do not block the proj matmuls in the PE FIFO
    for ch in attnv_chunks(*prev):
        ch()
    oj7, sq7 = emit_normalize(7, prev[3], with_stats=False)
    attn_ctx.close()

    p_qkv.release()
    if "o_lnT" in dbg_d:
        nc.sync.dma_start(out=dbg_d["o_lnT"][:], in_=o_lnT[:])

    # =====================================================================
    # PH5+6: proj (deferred subln) + residual -> xnT, software-pipelined:
    # chain(ms) ... epilogue(ms-1); subln rows computed after chain 0;
    # LN2 stats interleaved per output tile.
    # =====================================================================
    p_xnT = tc.alloc_tile_pool(name="p_xnT", bufs=1)
    xnT = p_xnT.tile([128, DT, NTOK], F32, name="xnT")
    p_wB = tc.alloc_tile_pool(name="p_wB", bufs=1, side="right")
    wp_b = p_wB.tile([128, KT - KT // 2, D], BF16, name="wp_b")
    nc.sync.dma_start(out=wp_b, in_=wp_d[:, KT // 2:, :])
    wpsum_st = consts.tile([128, DT], F32, name="wpsum_st")
    nc.sync.dma_start(out=wpsum_st,
                      in_=pvec["wpsum"].rearrange("(o p) -> p o", p=128))
    bc = {}

    # reload x^T for the residual (straight from the pre-transposed input)
    p_xTr = tc.alloc_tile_pool(name="p_xTr", bufs=1)
    xTr = p_xTr.tile([128, DT, NTOK], XDT, name="xTr")
    nc.sync.dma_start(out=xTr, in_=x_d[:, :, :])

    skip_bp = "bp" in triv
    sum2_row = rows.tile([1, NTOK], F32, tag="sum_row", name="sum2_row")
    sq2_row = rows.tile([1, NTOK], F32, tag="sq_row", name="sq2_row")

    def emit_subln_rows():
        _, s_sub, negm_sub = ln_rows(sumO_row, sqO_row, D)
        bc["ssub"] = bcast_b(s_sub)
        ns_sub = rows.tile([1, NTOK], BF16, tag="nsrow", name="ns_sub",
                           bufs=1)
        nc.vector.tensor_tensor(ns_sub, negm_sub, s_sub, ALU.mult)
        bc["nsub"] = bcast_b(ns_sub)

    def proj_epilogue(m, s, ps):
        corr_m = tmps.tile([128, 512], BF16, tag="corr", name="corr_m",
                           bufs=2)
        nc.vector.tensor_scalar_mul(
            out=corr_m, in0=bc["nsub"][:, s * 512:(s + 1) * 512],
            scalar1=wpsum_st[:, m:m + 1])
        nc.vector.tensor_tensor(
            ps, ps, bc["ssub"][:, s * 512:(s + 1) * 512], ALU.mult)
        nc.vector.tensor_tensor(ps, ps, corr_m, ALU.add)
        if not skip_bp:
            nc.vector.tensor_scalar(
                out=ps, in0=ps, scalar1=st["bp"][:, m:m + 1],
                scalar2=None, op0=ALU.add)
        nc.vector.tensor_tensor(
            xnT[:, m, s * 512:(s + 1) * 512], ps,
            xTr[:, m, s * 512:(s + 1) * 512], ALU.add)

    def emit_ln2_tile_stats(m, hold):
        cst = sqp.tile([128, NTOK], BF16, tag="cst", name="cst2")
        nc.scalar.copy(out=cst, in_=xnT[:, m, :])
        sq_t = sqp.tile([128, NTOK], BF16, tag="sq_t", name="sq_t2")
        nc.vector.tensor_tensor(sq_t, cst, cst, ALU.mult)
        for s in range(2):
            nc.tensor.matmul(
                hold[0][s], ones_b, cst[:, s * 512:(s + 1) * 512],
                start=(m == 0), stop=(m == DT - 1), skip_group_check=True)
            nc.tensor.matmul(
                hold[1][s], ones_b, sq_t[:, s * 512:(s + 1) * 512],
                start=(m == 0), stop=(m == DT - 1), skip_group_check=True)

    with tc.tile_pool(name="ps_ln2", bufs=1, space="PSUM") as ps_ln2:
        hold = [[ps_ln2.tile([1, 512], F32, tag=f"l2{a}{s}",
                             name=f"l2{a}{s}") for s in range(2)]
                for a in range(2)]
        pending_ep = None
        for ms in range(16):
            m, s = divmod(ms, 2)
            ps = mm_psum()
            for o in range(KT):
                wsrc = (wp_a[:, o, :] if o < KT // 2
                        else wp_b[:, o - KT // 2, :])
                nc.tensor.matmul(
                    ps, wsrc[:, m * 128:(m + 1) * 128],
                    o_lnT[:, o, s * 512:(s + 1) * 512],
                    start=(o == 0), stop=(o == KT - 1))
            if ms == 0:
                emit_subln_stats(oj7, sq7)
                emit_subln_rows()
            if pending_ep is not None:
                proj_epilogue(*pending_ep)
                if pending_ep[1] == 1:
                    emit_ln2_tile_stats(pending_ep[0], hold)
            pending_ep = (m, s, ps)
        proj_epilogue(*pending_ep)
        emit_ln2_tile_stats(DT - 1, hold)
        for s in range(2):
            nc.scalar.copy(out=sum2_row[:, s * 512:(s + 1) * 512],
                           in_=hold[0][s])
            nc.scalar.copy(out=sq2_row[:, s * 512:(s + 1) * 512],
                           in_=hold[1][s])
    p_xTr.release()
    p_wB.release()
    p_OlnT.release()
    p_wA.release()
    if "xnT" in dbg_d:
        nc.sync.dma_start(out=dbg_d["xnT"][:], in_=xnT[:])

    # =====================================================================
    # PH7: LN2 -> x2T (bf16)
    # =====================================================================
    p_x2T = tc.alloc_tile_pool(name="p_x2T", bufs=1)
    x2T = p_x2T.tile([128, DT, NTOK], BF16, name="x2T")
    m_r, s_r, negm_r = ln_rows(sum2_row, sq2_row, D)
    fm_ln_apply(xnT, x2T, DT, negm_r, s_r, "n2")
    if "x2T" in dbg_d:
        nc.sync.dma_start(out=dbg_d["x2T"][:], in_=x2T[:])

    # =====================================================================
    # PH8: fc1 + gelu -> hT (bf16), with fused ffn_ln stats
    # =====================================================================
    p_hT = tc.alloc_tile_pool(name="p_hT", bufs=1, side="right")
    hT = p_hT.tile([128, HT, NTOK], BF16, name="hT")
    hsum_row = rows.tile([1, NTOK], F32, tag="sum_row", name="hsum_row")
    hsq_row = rows.tile([1, NTOK], F32, tag="sq_row", name="hsq_row")
    skip_b1 = "b1" in triv
    # ffn_ln stats via held accumulating ones-matmuls on the PE (the old DVE
    # accumulation made Vector the fc1 bottleneck at 93% busy); squares on DVE
    # (bf16, cheap), cross-partition sums ride 4 held psum banks.
    with tc.tile_pool(name="p_wblk", bufs=3) as p_wblk, \
         tc.tile_pool(name="ps_ffn", bufs=1, space="PSUM") as ps_ffn:
        ps_sum = [ps_ffn.tile([1, 512], F32, tag=f"ffsum{s}", name=f"ffsum{s}")
                  for s in range(2)]
        ps_sq = [ps_ffn.tile([1, 512], F32, tag=f"ffsq{s}", name=f"ffsq{s}")
                 for s in range(2)]
        for hm in range(HT):
            w1blk = p_wblk.tile([128, KT, 128], BF16, tag="w1blk",
                                name="w1blk")
            nc.sync.dma_start(out=w1blk, in_=w1_d[hm])
            for s in range(2):
                ps = mm_psum()
                for o in range(KT):
                    nc.tensor.matmul(
                        ps, w1blk[:, o, :],
                        x2T[:, o, s * 512:(s + 1) * 512],
                        start=(o == 0), stop=(o == KT - 1))
                hslice = hT[:, hm, s * 512:(s + 1) * 512]
                if skip_b1:
                    nc.scalar.activation(out=hslice, in_=ps, func=AF.Gelu)
                else:
                    nc.scalar.activation(
                        out=hslice, in_=ps, func=AF.Gelu,
                        bias=st["b1"][:, hm:hm + 1], scale=1.0)
                sq_t = sqp.tile([128, NTOK], BF16, tag="sq_t",
                                name="sq_tf")[:, :512]
                nc.vector.tensor_tensor(sq_t, hslice, hslice, ALU.mult)
                nc.tensor.matmul(
                    ps_sum[s], ones_b, hslice,
                    start=(hm == 0), stop=(hm == HT - 1),
                    skip_group_check=True)
                nc.tensor.matmul(
                    ps_sq[s], ones_b, sq_t,
                    start=(hm == 0), stop=(hm == HT - 1),
                    skip_group_check=True)
        for s in range(2):
            nc.scalar.copy(out=hsum_row[:, s * 512:(s + 1) * 512],
                           in_=ps_sum[s])
            nc.scalar.copy(out=hsq_row[:, s * 512:(s + 1) * 512],
                           in_=ps_sq[s])
    p_x2T.release()
    if "hT" in dbg_d:
        nc.sync.dma_start(out=dbg_d["hT"][:], in_=hT[:])

    # =====================================================================
    # PH9: ffn_ln rows only (normalization deferred into fc2: an extra K=1
    # matmul row adds -mean*colsum(W2); psum scaled by rstd in the epilogue)
    # =====================================================================
    _, s_ffn, negm_ffn = ln_rows(hsum_row, hsq_row, HID)
    sffn_b = bcast_b(s_ffn)
    w2sum_st = consts.tile([128, DT], F32, name="w2sum_st")
    nc.sync.dma_start(out=w2sum_st,
                      in_=pvec["w2sum"].rearrange("(o p) -> p o", p=128))
    ns_ffn = rows.tile([1, NTOK], BF16, tag="nsrow", name="ns_ffn", bufs=1)
    nc.vector.tensor_tensor(ns_ffn, negm_ffn, s_ffn, ALU.mult)
    nffn_b = bcast_b(ns_ffn)

    # =====================================================================
    # PH10: fc2 (with deferred ffn_ln) + residual + transpose + store
    # =====================================================================
    skip_b2 = "b2" in triv
    with tc.tile_pool(name="p_w2blk", bufs=3) as p_w2blk, \
         tc.tile_pool(name="p_out", bufs=4) as p_out:
        for m in range(DT):
            w2blk = p_w2blk.tile([128, HT, 128], BF16, tag="w2blk",
                                 name="w2blk")
            nc.sync.dma_start(out=w2blk, in_=w2_d[m])
            for s in range(2):
                corr2 = tmps.tile([128, 512], BF16, tag="corr",
                                  name="corr2", bufs=2)
                nc.vector.tensor_scalar_mul(
                    out=corr2, in0=nffn_b[:, s * 512:(s + 1) * 512],
                    scalar1=w2sum_st[:, m:m + 1])
                outm = p_out.tile([128, 512], F32, tag="outm", name="outm")
                ps = mm_psum()
                for o in range(HT):
                    nc.tensor.matmul(
                        ps, w2blk[:, o, :],
                        hT[:, o, s * 512:(s + 1) * 512],
                        start=(o == 0), stop=(o == HT - 1))
                nc.vector.tensor_tensor(
                    ps, ps, sffn_b[:, s * 512:(s + 1) * 512], ALU.mult)
                nc.vector.tensor_tensor(ps, ps, corr2, ALU.add)
                if not skip_b2:
                    nc.vector.tensor_scalar(
                        out=ps, in0=ps, scalar1=st["b2"][:, m:m + 1],
                        scalar2=None, op0=ALU.add)
                nc.vector.tensor_tensor(
                    outm, ps, xnT[:, m, s * 512:(s + 1) * 512], ALU.add)
                nc.sync.dma_start(
                    out=y_d[:, m, s * 512:(s + 1) * 512], in_=outm)
    p_hT.release()
    p_xnT.release()
    ctx.close()


# --------------------------------------------------------------------------
# host glue
# --------------------------------------------------------------------------

_PROGRAM_CACHE = {}


def get_program(debug=(), triv=frozenset()):
    key = (tuple(sorted(debug)), tuple(sorted(triv)))
    if key not in _PROGRAM_CACHE:
        _PROGRAM_CACHE[key] = build_program(debug=key[0], triv=key[1])
    return _PROGRAM_CACHE[key]


def compute_triv(inputs):
    f32 = np.float32
    triv = set()
    for k in ["n1", "n3", "ln", "n2", "ffn"]:
        g = np.asarray(inputs[k + "_g"], f32)
        b = np.asarray(inputs[k + "_b"], f32)
        if np.all(g == 1.0) and np.all(b == 0.0):
            triv.add(k)
    for k in ["bq", "bv", "b1"]:
        if np.all(np.asarray(inputs[k], f32) == 0.0):
            triv.add(k)
    bp_eff = (np.asarray(inputs["bp"], f32)
              + np.asarray(inputs["ln_b"], f32) @ np.asarray(inputs["Wp"], f32))
    if np.all(bp_eff == 0.0):
        triv.add("bp")
    b2_eff = (np.asarray(inputs["b2"], f32)
              + np.asarray(inputs["ffn_b"], f32) @ np.asarray(inputs["W2"], f32))
    if np.all(b2_eff == 0.0):
        triv.add("b2")
    return frozenset(triv)


def make_in_maps(inputs):
    """Build the 8 per-core input maps from the full-problem input dict."""
    bf = ml_dtypes.bfloat16
    f32 = np.float32

    def host(name):
        return np.asarray(inputs[name], dtype=f32)

    # fold the subln (ln_g/ln_b) into Wp/bp and the ffn_ln (ffn_g/ffn_b)
    # into W2/b2 — the kernel defers those norms into the matmuls and only
    # applies (x-mean)*rstd
    wp_eff = host("ln_g")[:, None] * host("Wp")
    bp_eff = host("bp") + host("ln_b") @ host("Wp")
    w2_eff = host("ffn_g")[:, None] * host("W2")
    b2_eff = host("b2") + host("ffn_b") @ host("W2")
    wp_bf = wp_eff.astype(bf)
    w2_bf = w2_eff.astype(bf)
    shared = dict(
        wq=np.ascontiguousarray(
            host("Wq").reshape(KT, 128, D).astype(bf).transpose(1, 0, 2)),
        wk=np.ascontiguousarray(
            host("Wk").reshape(KT, 128, D).astype(bf).transpose(1, 0, 2)),
        wv=np.ascontiguousarray(
            host("Wv").reshape(KT, 128, D).astype(bf).transpose(1, 0, 2)),
        wp=np.ascontiguousarray(
            wp_bf.reshape(KT, 128, D).transpose(1, 0, 2)),
        wpsum=wp_bf.astype(np.float32).sum(0).astype(f32),
        w1=np.ascontiguousarray(
            host("W1").reshape(KT, 128, HT, 128).transpose(2, 1, 0, 3)
        ).astype(bf),
        w2=np.ascontiguousarray(
            w2_bf.reshape(HT, 128, DT, 128).transpose(2, 1, 0, 3)),
        w2sum=w2_bf.astype(np.float32).sum(0).astype(f32),
        n1_g=host("n1_g"), n1_b=host("n1_b"),
        n3_g=host("n3_g"), n3_b=host("n3_b"),
        bq=host("bq"), bv=host("bv"),
        ln_g=host("ln_g"), ln_b=host("ln_b"),
        bp=bp_eff.astype(f32),
        n2_g=host("n2_g"), n2_b=host("n2_b"),
        b1=host("b1"), ffn_g=host("ffn_g"), ffn_b=host("ffn_b"),
        b2=b2_eff.astype(f32),
        gate=host("gate").reshape(H),
    )
    x = host("x")
    xt = host("x_text")
    in_maps = []
    for b in range(B):
        m = dict(shared)
        # device consumes feature-major, partition-major inputs
        m["x"] = np.ascontiguousarray(
            x[b].T.reshape(DT, 128, N).transpose(1, 0, 2)).astype(bf)
        m["x_text"] = np.ascontiguousarray(
            xt[b].T.reshape(DT, 128, PT).transpose(1, 0, 2)).astype(bf)
        in_maps.append(m)
    return in_maps


def unpack_y(y):
    """Device output is feature-major [128, DT, NTOK]; back to [NTOK, D]."""
    y = np.asarray(y)
    return np.transpose(y, (2, 1, 0)).reshape(NTOK, D)


def kernel(**inputs) -> np.ndarray:
    from concourse.bass_utils import run_bass_kernel_spmd

    nc = get_program(triv=compute_triv(inputs))
    in_maps = make_in_maps(inputs)
    res = run_bass_kernel_spmd(nc, in_maps, list(range(B)))
    out = np.stack([unpack_y(res.results[b]["y"]) for b in range(B)], axis=0)
    return out.astype(np.float32)



# revision 50
# speedup vs baseline: 1.3081x; 1.0212x over previous
"""Trainium2 Bass kernel for nn_Block_45724221833665 (dense transformer block).

Strategy: pure data-parallel over batch — 8 batch elements, 8 NeuronCores, one
batch element per core, no collectives.  Inside a core everything is computed
"feature-major" (features on SBUF partitions, tokens on the free dim) so that:
  * all matmuls consume operands in their natural layout (weights as lhsT),
  * all per-feature biases/gains are per-partition scalars,
  * attention needs no transposes at all (scores are computed as S^T, attn@v
    takes exp(S^T) directly as the moving operand with V as the stationary
    operand, and the per-token softmax denominator comes from an extra all-ones
    column appended to V).
Inputs arrive host-pre-transposed (feature-major) and the output is returned
feature-major and transposed back on the host, so the device does no
transposes at all.

LayerNorms in feature-major form need cross-partition sums; those are done on
the tensor engine with an all-ones stationary vector over bf16 casts, and the
per-token mean/rstd rows are broadcast across partitions by DMA via a small
DRAM bounce.  Reciprocals run on ACT (the DVE iterative divide is ~8
cycles/element and our rows live on one partition); the LN rstd gets one
Newton step on DVE to recover full precision.

The gated dual softmax simplifies: softmax over a single key (column 0) is
identically 1, so the reference's w_prefix column is just tanh(gate) per head.
Attention output = softmax over the other 1087 keys @ v  +  tanh(gate)*v[key0]
(+ (1+tanh(gate))*bv once the v-bias is folded out of the softmax-weighted sum,
since the main softmax weights sum to 1).

Gains that are identically 1 and biases identically 0 (the common case for
this block) are detected on the host and their application elided; the cache
key of the built program includes those flags, so nontrivial parameters still
take the general path.
"""

import numpy as np
import ml_dtypes

import concourse.bass as bass
import concourse.mybir as mybir
import concourse.tile as tile
from concourse import bacc

B, N, PT, D, H, HD, HID = 8, 1024, 64, 1024, 16, 64, 4096
KT = D // 128          # 8 contraction tiles over D
HT = HID // 128        # 32 tiles over HID
DT = D // 128          # 8 output tiles over D
NTOK = N               # 1024 main tokens per core
NKEY = N + PT          # 1088 keys (main tokens + text prefix)
EPS = 1e-5

F32 = mybir.dt.float32
BF16 = mybir.dt.bfloat16
XDT = BF16  # wire dtype of the (host-pre-transposed) x / x_text inputs
AF = mybir.ActivationFunctionType
ALU = mybir.AluOpType


def build_program(debug=(), triv=frozenset()):
    """Build the single-core Bass program.
    debug: iterable of intermediate names to emit as DRAM outputs.
    triv: parameter groups whose gain/bias application can be skipped."""
    nc = bacc.Bacc("TRN2")
    dbg = set(debug)
    triv = set(triv)

    # ---- I/O ------------------------------------------------------------
    # x / x_text arrive HOST-PRE-TRANSPOSED (feature-major [D, tokens])
    x_d = nc.declare_dram_parameter("x", [128, DT, NTOK], XDT, isOutput=False)
    xt_d = nc.declare_dram_parameter("x_text", [128, DT, PT], XDT,
                                     isOutput=False)
    wq_d = nc.declare_dram_parameter("wq", [128, KT, D], BF16, isOutput=False)
    wk_d = nc.declare_dram_parameter("wk", [128, KT, D], BF16, isOutput=False)
    wv_d = nc.declare_dram_parameter("wv", [128, KT, D], BF16, isOutput=False)
    wp_d = nc.declare_dram_parameter("wp", [128, KT, D], BF16, isOutput=False)
    w1_d = nc.declare_dram_parameter("w1", [HT, 128, KT, 128], BF16,
                                 isOutput=False)
    w2_d = nc.declare_dram_parameter("w2", [DT, 128, HT, 128], BF16,
                                     isOutput=False)
    pvec = {}
    for nm, sz in [
        ("n1_g", D), ("n1_b", D), ("n3_g", D), ("n3_b", D),
        ("bq", D), ("bv", D), ("ln_g", D), ("ln_b", D), ("bp", D),
        ("n2_g", D), ("n2_b", D), ("b1", HID), ("ffn_g", HID), ("ffn_b", HID),
        ("b2", D),
    ]:
        pvec[nm] = nc.declare_dram_parameter(nm, [sz], F32, isOutput=False)
    pvec["wpsum"] = nc.declare_dram_parameter("wpsum", [D], F32,
                                              isOutput=False)
    pvec["w2sum"] = nc.declare_dram_parameter("w2sum", [D], F32,
                                              isOutput=False)
    gate_d = nc.declare_dram_parameter("gate", [H], F32, isOutput=False)
    # y is produced feature-major [128, DT, NTOK]; host transposes back
    y_d = nc.declare_dram_parameter("y", [128, DT, NTOK], F32, isOutput=True)

    dbg_d = {}

    def dbg_out(name, shape, dtype):
        if name in dbg:
            dbg_d[name] = nc.declare_dram_parameter(
                "dbg_" + name, list(shape), dtype, isOutput=True
            )

    dbg_out("xT", [128, DT, NTOK], F32)
    dbg_out("x1T", [128, DT, NTOK], BF16)
    dbg_out("xt1T", [128, DT, PT], BF16)
    dbg_out("qT", [128, DT, NTOK], BF16)
    dbg_out("kT", [128, DT, NKEY], BF16)
    dbg_out("v", [128, KT, H, HD + 1], BF16)
    dbg_out("v_text", [PT, H, HD + 1], BF16)
    dbg_out("c_vec", [D], F32)
    dbg_out("pp0", [128, 2, KT, NTOK], BF16)     # exp(S^T) for heads 0,1
    dbg_out("pp0_text", [PT, 2, NTOK], BF16)
    dbg_out("o_full", [128, DT, NTOK], F32)
    dbg_out("o_lnT", [128, DT, NTOK], BF16)
    dbg_out("xnT", [128, DT, NTOK], F32)
    dbg_out("x2T", [128, DT, NTOK], BF16)
    dbg_out("hT", [128, HT, NTOK], BF16)

    with tile.TileContext(nc) as tc:
        _build_phases(nc, tc, x_d, xt_d, wq_d, wk_d, wv_d, wp_d,
                      w1_d, w2_d, pvec, gate_d, y_d, dbg_d, triv)
    nc.compile()
    return nc


def _build_phases(nc, tc, x_d, xt_d, wq_d, wk_d, wv_d, wp_d, w1_d, w2_d,
                  pvec, gate_d, y_d, dbg_d, triv):
    import contextlib
    ctx = contextlib.ExitStack()
    # ---- whole-kernel pools ---------------------------------------------
    consts = ctx.enter_context(tc.tile_pool(name="consts", bufs=1))
    rows = ctx.enter_context(tc.tile_pool(name="rows", bufs=1))
    tmps = ctx.enter_context(tc.tile_pool(name="tmps", bufs=2))
    sqp = ctx.enter_context(tc.tile_pool(name="sqp", bufs=2))
    dram = ctx.enter_context(tc.tile_pool(name="dram", bufs=1, space="DRAM"))
    ps_mm = ctx.enter_context(tc.tile_pool(name="ps_mm", bufs=3, space="PSUM"))
    # p_qkv sits below p_x1T on the pool stack so x1T can be freed after the
    # projections while qT/kT/v live on through attention
    p_qkv = tc.alloc_tile_pool(name="p_qkv", bufs=1)
    p_x1T = tc.alloc_tile_pool(name="p_x1T", bufs=1)

    def mm_psum(pfree=512, parts=128):
        t = ps_mm.tile([128, 512], F32, tag="mm", name="mmps")
        return t[:parts, :pfree]

    # ---- constants ------------------------------------------------------
    ones_b = consts.tile([128, 1], BF16, name="ones_b")
    nc.vector.memset(ones_b, 1.0)
    ones_r = consts.tile([1, 128], BF16, name="ones_r")
    nc.vector.memset(ones_r, 1.0)
    eps_c = consts.tile([1, 1], F32, name="eps_c")
    nc.vector.memset(eps_c, EPS)

    st = {}  # striped parameter tiles [128, tiles]; triv params never read,
    # so skip their DMAs (each is 128 tiny descriptors clogging the queues)
    def group_used(nm):
        g = nm.split("_")[0] if "_" in nm else nm
        return g not in triv
    for nm in ["n1_g", "n1_b", "n3_g", "n3_b", "bq", "ln_g", "ln_b", "bp",
               "n2_g", "n2_b", "b2"]:
        if not group_used(nm):
            continue
        t = consts.tile([128, DT], F32, name="st_" + nm)
        nc.sync.dma_start(out=t, in_=pvec[nm].rearrange("(o p) -> p o", p=128))
        st[nm] = t
    for nm in ["b1", "ffn_g", "ffn_b"]:
        if not group_used(nm):
            continue
        t = consts.tile([128, HT], F32, name="st_" + nm)
        nc.sync.dma_start(out=t, in_=pvec[nm].rearrange("(o p) -> p o", p=128))
        st[nm] = t

    def bcast_b(row, ntok=NTOK):
        """Broadcast a [1, ntok] row to a [128, ntok] bf16 tile via a K=1
        PE outer product with an all-ones stationary (the PE is idle at the
        LN phase boundaries where these sit, and its latency is ~1us vs the
        multi-us dispatch stalls seen on GPSIMD/DMA-bounce paths)."""
        if row.dtype != BF16:
            br = rows.tile([1, NTOK], BF16, tag="brow", name="brow",
                           bufs=1)[:, :ntok]
            nc.vector.tensor_copy(out=br, in_=row)
            row = br
        out = tmps.tile([128, NTOK], BF16, tag="wrk", name="bb")[:, :ntok]
        nsl = max(1, ntok // 512)
        sl = ntok // nsl
        for s in range(nsl):
            ps = mm_psum(sl)
            nc.tensor.matmul(ps, ones_r, row[:, s * sl:(s + 1) * sl],
                             start=True, stop=True, skip_group_check=True)
            nc.vector.tensor_copy(out=out[:, s * sl:(s + 1) * sl], in_=ps)
        return out

    def ln_rows(sum_row, sq_row, n_elems):
        """Turn per-token sums into (mean, rstd, -mean[bf16]) rows.  rstd =
        ACT abs_reciprocal_sqrt(var+eps) refined by one rsqrt-Newton step on
        DVE (the banned-for-accuracy ACT seed is fine once refined)."""
        ntok = sum_row.shape[-1]
        t_row = rows.tile([1, NTOK], F32, tag="t_row", name="t_row")[:, :ntok]
        s_row = rows.tile([1, NTOK], F32, tag="s_row", name="s_row")[:, :ntok]
        u_row = rows.tile([1, NTOK], F32, tag="u_row", name="u_row",
                          bufs=1)[:, :ntok]
        negm = rows.tile([1, NTOK], BF16, tag="negm", name="negm")[:, :ntok]
        inv = 1.0 / float(n_elems)
        nc.vector.tensor_scalar_mul(out=sum_row, in0=sum_row, scalar1=inv)
        nc.vector.tensor_scalar_mul(out=sq_row, in0=sq_row, scalar1=inv)
        nc.vector.tensor_tensor(t_row, sum_row, sum_row, ALU.mult)  # mean^2
        nc.vector.tensor_tensor(sq_row, sq_row, t_row, ALU.subtract)  # var
        nc.scalar.activation(out=s_row, in_=sq_row,
                             func=AF.Abs_reciprocal_sqrt,
                             bias=eps_c, scale=1.0)                 # ~rstd
        # rsqrt Newton: s <- s*(1.5 - 0.5*(var+eps)*s^2)
        nc.vector.tensor_scalar_add(out=t_row, in0=sq_row, scalar1=EPS)
        nc.vector.tensor_tensor(u_row, s_row, s_row, ALU.mult)
        nc.vector.tensor_tensor(u_row, u_row, t_row, ALU.mult)
        nc.vector.tensor_scalar(out=u_row, in0=u_row, scalar1=-0.5,
                                scalar2=1.5, op0=ALU.mult, op1=ALU.add)
        nc.vector.tensor_tensor(s_row, s_row, u_row, ALU.mult)
        nc.vector.tensor_scalar_mul(out=negm, in0=sum_row, scalar1=-1.0)
        return sum_row, s_row, negm

    # =====================================================================
    # PH1: load x / x_text directly in feature-major layout (host-transposed)
    # =====================================================================
    x1T = p_x1T.tile([128, DT, NTOK], BF16, name="x1T")
    p_xtB = tc.alloc_tile_pool(name="p_xtB", bufs=1)
    xt1T = p_xtB.tile([128, DT, PT], BF16, name="xt1T")
    p_xtA = tc.alloc_tile_pool(name="p_xtA", bufs=1)
    xtT = p_xtA.tile([128, DT, PT], XDT, name="xtT")
    p_xT = tc.alloc_tile_pool(name="p_xT", bufs=1)
    xT = p_xT.tile([128, DT, NTOK], XDT, name="xT")

    nc.sync.dma_start(out=xtT, in_=xt_d[:, :, :])
    for hf in range(2):  # two half-chunks: big descriptors, early stats
        nc.sync.dma_start(
            out=xT[:, hf * 4:(hf + 1) * 4, :],
            in_=x_d[:, hf * 4:(hf + 1) * 4, :])
    if "xT" in dbg_d:
        nc.sync.dma_start(out=dbg_d["xT"][:], in_=xT[:])

    # ---- feature-major layernorm helpers --------------------------------
    def fm_ln_stats(src, ntiles, ntok, sl=512):
        """Cross-partition LN stats for src [128, ntiles, ntok] (f32 or bf16).
        Returns (sum_row, sq_row) [1, ntok] f32 rows (in `rows` pool)."""
        sum_row = rows.tile([1, NTOK], F32, tag="sum_row",
                            name="sum_row")[:, :ntok]
        sq_row = rows.tile([1, NTOK], F32, tag="sq_row",
                           name="sq_row")[:, :ntok]
        nsl = ntok // sl
        with tc.tile_pool(name="ps_stat", bufs=4, space="PSUM") as ps_stat:
            ps_a = [ps_stat.tile([1, sl], F32, tag="stat", name=f"psa{s}",
                                 bufs=2) for s in range(nsl)]
            ps_b = [ps_stat.tile([1, sl], F32, tag="stat2", name=f"psb{s}",
                                 bufs=2) for s in range(nsl)]
            for o in range(ntiles):
                if src.dtype == F32:
                    cst = sqp.tile([128, NTOK], BF16, tag="cst",
                                   name="cst")[:, :ntok]
                    nc.scalar.copy(out=cst, in_=src[:, o, :])
                else:
                    cst = src[:, o, :]
                sq_t = sqp.tile([128, NTOK], BF16, tag="sq_t",
                                name="sq_t")[:, :ntok]
                nc.vector.tensor_tensor(sq_t, cst, cst, ALU.mult)
                for s in range(nsl):
                    nc.tensor.matmul(
                        ps_a[s], ones_b, cst[:, s * sl:(s + 1) * sl],
                        start=(o == 0), stop=(o == ntiles - 1),
                        skip_group_check=True)
                    nc.tensor.matmul(
                        ps_b[s], ones_b, sq_t[:, s * sl:(s + 1) * sl],
                        start=(o == 0), stop=(o == ntiles - 1),
                        skip_group_check=True)
            for s in range(nsl):
                nc.scalar.copy(out=sum_row[:, s * sl:(s + 1) * sl], in_=ps_a[s])
                nc.scalar.copy(out=sq_row[:, s * sl:(s + 1) * sl], in_=ps_b[s])
        return sum_row, sq_row

    def fm_ln_apply(src, dst, ntiles, negm_row, s_row, gkey):
        """dst[:,o,:] = ((src - m)*rstd)[*g + b], slice-pipelined so
        consumers ordered slice-outer can start on slice 0 early."""
        ntok = src.shape[-1]
        g_st = st.get(gkey + "_g")
        b_st = st.get(gkey + "_b")
        skip_gb = gkey in triv
        m_b = bcast_b(negm_row, ntok)
        s_b = bcast_b(s_row, ntok)
        nsl = max(1, ntok // 512)
        sl = ntok // nsl
        for s in range(nsl):
            ss = slice(s * sl, (s + 1) * sl)
            for o in range(ntiles):
                tA = tmps.tile([128, 512], BF16, tag="tA",
                               name="tA")[:, :sl]
                nc.vector.tensor_tensor(tA, src[:, o, ss], m_b[:, ss],
                                        ALU.add)
                if skip_gb:
                    nc.vector.tensor_tensor(dst[:, o, ss], tA, s_b[:, ss],
                                            ALU.mult)
                else:
                    nc.vector.tensor_tensor(tA, tA, s_b[:, ss], ALU.mult)
                    nc.scalar.activation(
                        out=dst[:, o, ss], in_=tA, func=AF.Identity,
                        scale=g_st[:, o:o + 1], bias=b_st[:, o:o + 1])

    # =====================================================================
    # PH2: LN1(x) -> x1T (bf16), LN3(x_text) -> xt1T (bf16)
    # =====================================================================
    sum_r, sq_r = fm_ln_stats(xtT, DT, PT, sl=PT)
    m_r, s_r, negm_r = ln_rows(sum_r, sq_r, D)
    fm_ln_apply(xtT, xt1T, DT, negm_r, s_r, "n3")

    sum_r, sq_r = fm_ln_stats(xT, DT, NTOK)
    m_r, s_r, negm_r = ln_rows(sum_r, sq_r, D)
    fm_ln_apply(xT, x1T, DT, negm_r, s_r, "n1")
    p_xT.release()
    p_xtA.release()

    if "x1T" in dbg_d:
        nc.sync.dma_start(out=dbg_d["x1T"][:], in_=x1T[:])
    if "xt1T" in dbg_d:
        nc.sync.dma_start(out=dbg_d["xt1T"][:], in_=xt1T[:])

    # =====================================================================
    # PH3: projections  Q^T, K^T (feature-major), V (token-major, +ones col)
    # =====================================================================
    qT = p_qkv.tile([128, DT, NTOK], BF16, name="qT")
    kT = p_qkv.tile([128, DT, NKEY], BF16, name="kT")
    v_sb = p_qkv.tile([128, KT, H, HD + 1], BF16, name="v_sb")
    vt_sb = p_qkv.tile([PT, H, HD + 1], BF16, name="vt_sb")

    p_w = tc.alloc_tile_pool(name="p_w", bufs=2)
    wq_sb = p_w.tile([128, KT, D], BF16, tag="wfull", name="wq_sb")
    nc.sync.dma_start(out=wq_sb, in_=wq_d[:, :, :])
    wk_sb = p_w.tile([128, KT, D], BF16, tag="wfull", name="wk_sb")
    nc.sync.dma_start(out=wk_sb, in_=wk_d[:, :, :])
    skip_bq = "bq" in triv
    for s in range(2):
        for m in range(DT):
            ps = mm_psum()
            for o in range(KT):
                nc.tensor.matmul(
                    ps, wq_sb[:, o, m * 128:(m + 1) * 128],
                    x1T[:, o, s * 512:(s + 1) * 512],
                    start=(o == 0), stop=(o == KT - 1))
            if skip_bq:
                nc.scalar.copy(out=qT[:, m, s * 512:(s + 1) * 512], in_=ps)
            else:
                nc.scalar.activation(
                    out=qT[:, m, s * 512:(s + 1) * 512], in_=ps,
                    func=AF.Identity, bias=st["bq"][:, m:m + 1], scale=1.0)
        for m in range(DT):
            ps = mm_psum()
            for o in range(KT):
                nc.tensor.matmul(
                    ps, wk_sb[:, o, m * 128:(m + 1) * 128],
                    x1T[:, o, s * 512:(s + 1) * 512],
                    start=(o == 0), stop=(o == KT - 1))
            nc.scalar.copy(out=kT[:, m, s * 512:(s + 1) * 512], in_=ps)
    for m in range(DT):  # K^T text keys
        ps = mm_psum(PT)
        for o in range(KT):
            nc.tensor.matmul(
                ps, wk_sb[:, o, m * 128:(m + 1) * 128], xt1T[:, o, :],
                start=(o == 0), stop=(o == KT - 1))
        nc.scalar.copy(out=kT[:, m, N:N + PT], in_=ps)
    # V token-major, heads interleaved with the all-ones 65th column
    wv_sb = p_w.tile([128, KT, D], BF16, tag="wfull", name="wv_sb")
    nc.sync.dma_start(out=wv_sb, in_=wv_d[:, :, :])
    for t in range(8):
        for sn in range(2):
            ps = mm_psum()
            for o in range(KT):
                nc.tensor.matmul(
                    ps, x1T[:, o, t * 128:(t + 1) * 128],
                    wv_sb[:, o, sn * 512:(sn + 1) * 512],
                    start=(o == 0), stop=(o == KT - 1))
            nc.scalar.copy(
                out=v_sb[:, t, sn * 8:(sn + 1) * 8, 0:HD], in_=ps)
    for sn in range(2):
        ps = mm_psum(parts=PT)
        for o in range(KT):
            nc.tensor.matmul(
                ps, xt1T[:, o, :], wv_sb[:, o, sn * 512:(sn + 1) * 512],
                start=(o == 0), stop=(o == KT - 1))
        nc.scalar.copy(out=vt_sb[:, sn * 8:(sn + 1) * 8, 0:HD], in_=ps)
    p_w.release()
    p_xtB.release()
    p_x1T.release()

    # ---- c_vec: tanh(gate)*v0_raw + (1+tanh(gate))*bv  ------------------
    g_row = rows.tile([1, H], F32, tag="g_row", name="g_row")
    nc.sync.dma_start(out=g_row, in_=gate_d.rearrange("(a h) -> a h", a=1))
    th_row = rows.tile([1, H], F32, tag="th_row", name="th_row")
    nc.scalar.activation(out=th_row, in_=g_row, func=AF.Tanh)
    c_work = rows.tile([1, H, HD], F32, tag="t_row", name="c_work")
    nc.vector.tensor_copy(out=c_work, in_=vt_sb[0:1, :, 0:HD])
    nc.vector.tensor_tensor(
        c_work, c_work, th_row[:, :, None].to_broadcast((1, H, HD)), ALU.mult)
    if "bv" not in triv:
        th1_row = rows.tile([1, H], F32, tag="th1_row", name="th1_row")
        nc.scalar.activation(out=th1_row, in_=th_row, func=AF.Identity,
                             bias=1.0)
        bv_row = rows.tile([1, H, HD], F32, tag="s_row", name="bv_row")
        nc.sync.dma_start(
            out=bv_row, in_=pvec["bv"].rearrange("(a h d) -> a h d", a=1, h=H))
        nc.vector.tensor_tensor(
            bv_row, bv_row, th1_row[:, :, None].to_broadcast((1, H, HD)),
            ALU.mult)
        nc.vector.tensor_tensor(c_work, c_work, bv_row, ALU.add)
    c_dram = dram.tile([D], F32, name="c_dram")
    nc.sync.dma_start(
        out=c_dram.rearrange("(a h d) -> a h d", a=1, h=H), in_=c_work)
    c_st = consts.tile([128, DT], F32, name="c_st")
    nc.sync.dma_start(out=c_st, in_=c_dram.rearrange("(o p) -> p o", p=128))
    if "c_vec" in dbg_d:
        nc.sync.dma_start(out=dbg_d["c_vec"][:], in_=c_dram[:])

    # ones column + zero out reference-key-0 (first text token)
    nc.vector.memset(v_sb[:, :, :, HD:HD + 1], 1.0)
    nc.vector.memset(vt_sb[:, :, HD:HD + 1], 1.0)
    nc.vector.memset(vt_sb[0:1, :, :], 0.0)

    # prefetch the first half of the proj weight while attention runs
    p_wA = tc.alloc_tile_pool(name="p_wA", bufs=1, side="right")
    wp_a = p_wA.tile([128, KT // 2, D], BF16, name="wp_a")
    nc.sync.dma_start(out=wp_a, in_=wp_d[:, 0:KT // 2, :])

    # =====================================================================
    # PH4: attention — per (pair, kt): 4 score matmuls into one 4-bank
    # [128,2048] psum, ONE exp over all 2048 cols (amortizes the ~350-cycle
    # ACT ramp), attn@v of the previous pair interleaved into emission so the
    # PE fills the exp-wait bubbles; softmax reciprocal on DVE (no ACT table
    # switches); subln stats of the last pair deferred into the proj phase.
    # =====================================================================
    p_OlnT = tc.alloc_tile_pool(name="p_OlnT", bufs=1, side="right")
    o_lnT = p_OlnT.tile([128, DT, NTOK], BF16, name="o_lnT")
    sumO_row = rows.tile([1, NTOK], F32, tag="sum_row", name="sumO_row")
    sqO_row = rows.tile([1, NTOK], F32, tag="sq_row", name="sqO_row")
    nc.vector.memset(sumO_row, 0.0)
    nc.vector.memset(sqO_row, 0.0)

    def emit_subln_stats(oj, sq_t4):
        for s in range(2):
            ps_a = mm_psum()[:1, :]
            nc.tensor.matmul(
                ps_a, ones_b, oj[:, s * 512:(s + 1) * 512],
                start=True, stop=True, skip_group_check=True)
            nc.vector.tensor_tensor(
                sumO_row[:, s * 512:(s + 1) * 512],
                sumO_row[:, s * 512:(s + 1) * 512], ps_a, ALU.add)
            ps_b = mm_psum()[:1, :]
            nc.tensor.matmul(
                ps_b, ones_b, sq_t4[:, s * 512:(s + 1) * 512],
                start=True, stop=True, skip_group_check=True)
            nc.vector.tensor_tensor(
                sqO_row[:, s * 512:(s + 1) * 512],
                sqO_row[:, s * 512:(s + 1) * 512], ps_b, ALU.add)

    attn_ctx = contextlib.ExitStack()
    p_attn = attn_ctx.enter_context(tc.tile_pool(name="p_attn", bufs=2))
    ps_big = attn_ctx.enter_context(
        tc.tile_pool(name="ps_big", bufs=2, space="PSUM"))

    def emit_normalize(j, se_pr, with_stats=True):
        nc.vector.reciprocal_approx_fast(out=se_pr, in_=se_pr)
        se_b = rows.tile([1, 2, NTOK], BF16, tag="se_b", name="se_b",
                         bufs=1)
        nc.vector.tensor_copy(out=se_b, in_=se_pr)
        rb = tmps.tile([128, 2, NTOK], BF16, tag="rb", name="rb", bufs=1)
        nc.gpsimd.partition_broadcast(rb, se_b)
        oj = o_lnT[:, j, :]
        nc.vector.tensor_tensor(oj[0:64, :], oj[0:64, :], rb[0:64, 0, :],
                                ALU.mult)
        nc.vector.tensor_tensor(oj[64:128, :], oj[64:128, :],
                                rb[64:128, 1, :], ALU.mult)
        nc.vector.tensor_scalar_add(out=oj, in0=oj,
                                    scalar1=c_st[:, j:j + 1])
        sq_t4 = sqp.tile([128, NTOK], BF16, tag="sq_t", name="sq_t4")
        nc.vector.tensor_tensor(sq_t4, oj, oj, ALU.mult)
        if with_stats:
            emit_subln_stats(oj, sq_t4)
        return oj, sq_t4

    def attnv_chunks(jj, pp, ppt, se_pr):
        chunks = []
        for hh in range(2):
            for s in range(2):
                def ch(hh=hh, s=s):
                    h = 2 * jj + hh
                    base = hh * 64
                    ps = mm_psum()[:HD + 1, :]
                    for kt in range(KT):
                        nc.tensor.matmul(
                            ps, v_sb[:, kt, h, :],
                            pp[:, kt,
                               hh * 1024 + s * 512:hh * 1024 + (s + 1) * 512],
                            start=(kt == 0), stop=False,
                            skip_group_check=True)
                    nc.tensor.matmul(
                        ps, vt_sb[:, h, :],
                        ppt[:, hh, s * 512:(s + 1) * 512],
                        start=False, stop=True, skip_group_check=True)
                    nc.vector.tensor_copy(
                        out=o_lnT[base:base + 64, jj, s * 512:(s + 1) * 512],
                        in_=ps[0:HD, :])
                    nc.vector.tensor_copy(
                        out=se_pr[:, hh, s * 512:(s + 1) * 512],
                        in_=ps[HD:HD + 1, :])
                chunks.append(ch)
        return chunks

    prev = None
    for j in range(8):      # head pairs
        pp = p_attn.tile([128, KT, 2048], BF16, tag="pp", name="pp")
        ppt = p_attn.tile([PT, 2, NTOK], BF16, tag="ppt", name="ppt")
        pend = attnv_chunks(*prev) if prev is not None else []
        for kt in range(KT):
            for hh in range(2):
                base = hh * 64
                psb = ps_big.tile([128, 1024], F32, tag="sc", name="psb")
                for s in range(2):
                    nc.tensor.matmul(
                        psb[:, s * 512:(s + 1) * 512],
                        kT[base:base + 64, j, kt * 128:(kt + 1) * 128],
                        qT[base:base + 64, j, s * 512:(s + 1) * 512],
                        start=True, stop=True, tile_position=(base, 0),
                        skip_group_check=True)
                nc.scalar.activation(
                    out=pp[:, kt, hh * 1024:(hh + 1) * 1024], in_=psb,
                    func=AF.Exp, scale=0.125)
            if kt % 2 == 0 and pend:
                pend.pop(0)()
        for hh in range(2):
            base = hh * 64
            psb = ps_big.tile([128, 1024], F32, tag="sc", name="psb")
            for s in range(2):
                nc.tensor.matmul(
                    psb[:PT, s * 512:(s + 1) * 512],
                    kT[base:base + 64, j, N:N + PT],
                    qT[base:base + 64, j, s * 512:(s + 1) * 512],
                    start=True, stop=True, tile_position=(base, 0),
                    skip_group_check=True)
            nc.scalar.activation(
                out=ppt[:, hh, :], in_=psb[:PT, :], func=AF.Exp,
                scale=0.125)
        for ch in pend:
            ch()
        if prev is not None:
            emit_normalize(prev[0], prev[3])
        se_pr = p_attn.tile([1, 2, NTOK], F32, tag="se_pr",
                            name="se_pr", bufs=1)
        prev = (j, pp, ppt, se_pr)
    # tail: attn@v + normalize of pair 7; its subln stats ride after the
    # first proj chain so they do not block the proj matmuls in the PE FIFO
    for ch in attnv_chunks(*prev):
        ch()
    oj7, sq7 = emit_normalize(7, prev[3], with_stats=False)
    attn_ctx.close()

    p_qkv.release()
    if "o_lnT" in dbg_d:
        nc.sync.dma_start(out=dbg_d["o_lnT"][:], in_=o_lnT[:])

    # =====================================================================
    # PH5+6: proj (deferred subln) + residual -> xnT, software-pipelined:
    # chain(ms) ... epilogue(ms-1); subln rows computed after chain 0;
    # LN2 stats interleaved per output tile.
    # =====================================================================
    p_xnT = tc.alloc_tile_pool(name="p_xnT", bufs=1)
    xnT = p_xnT.tile([128, DT, NTOK], F32, name="xnT")
    p_wB = tc.alloc_tile_pool(name="p_wB", bufs=1, side="right")
    wp_b = p_wB.tile([128, KT - KT // 2, D], BF16, name="wp_b")
    nc.sync.dma_start(out=wp_b, in_=wp_d[:, KT // 2:, :])
    wpsum_st = consts.tile([128, DT], F32, name="wpsum_st")
    nc.sync.dma_start(out=wpsum_st,
                      in_=pvec["wpsum"].rearrange("(o p) -> p o", p=128))
    bc = {}

    # reload x^T for the residual (straight from the pre-transposed input)
    p_xTr = tc.alloc_tile_pool(name="p_xTr", bufs=1)
    xTr = p_xTr.tile([128, DT, NTOK], XDT, name="xTr")
    nc.sync.dma_start(out=xTr, in_=x_d[:, :, :])

    skip_bp = "bp" in triv
    sum2_row = rows.tile([1, NTOK], F32, tag="sum_row", name="sum2_row")
    sq2_row = rows.tile([1, NTOK], F32, tag="sq_row", name="sq2_row")

    def emit_subln_rows():
        _, s_sub, negm_sub = ln_rows(sumO_row, sqO_row, D)
        bc["ssub"] = bcast_b(s_sub)
        ns_sub = rows.tile([1, NTOK], BF16, tag="nsrow", name="ns_sub",
                           bufs=1)
        nc.vector.tensor_tensor(ns_sub, negm_sub, s_sub, ALU.mult)
        bc["nsub"] = bcast_b(ns_sub)

    def proj_epilogue(m, s, ps):
        corr_m = tmps.tile([128, 512], BF16, tag="corr", name="corr_m",
                           bufs=2)
        nc.vector.tensor_scalar_mul(
            out=corr_m, in0=bc["nsub"][:, s * 512:(s + 1) * 512],
            scalar1=wpsum_st[:, m:m + 1])
        nc.vector.tensor_tensor(
            ps, ps, bc["ssub"][:, s * 512:(s + 1) * 512], ALU.mult)
        nc.vector.tensor_tensor(ps, ps, corr_m, ALU.add)
        if not skip_bp:
            nc.vector.tensor_scalar(
                out=ps, in0=ps, scalar1=st["bp"][:, m:m + 1],
                scalar2=None, op0=ALU.add)
        nc.vector.tensor_tensor(
            xnT[:, m, s * 512:(s + 1) * 512], ps,
            xTr[:, m, s * 512:(s + 1) * 512], ALU.add)

    def emit_ln2_tile_stats(m, hold):
        cst = sqp.tile([128, NTOK], BF16, tag="cst", name="cst2")
        nc.scalar.copy(out=cst, in_=xnT[:, m, :])
        sq_t = sqp.tile([128, NTOK], BF16, tag="sq_t", name="sq_t2")
        nc.vector.tensor_tensor(sq_t, cst, cst, ALU.mult)
        for s in range(2):
            nc.tensor.matmul(
                hold[0][s], ones_b, cst[:, s * 512:(s + 1) * 512],
                start=(m == 0), stop=(m == DT - 1), skip_group_check=True)
            nc.tensor.matmul(
                hold[1][s], ones_b, sq_t[:, s * 512:(s + 1) * 512],
                start=(m == 0), stop=(m == DT - 1), skip_group_check=True)

    with tc.tile_pool(name="ps_ln2", bufs=1, space="PSUM") as ps_ln2:
        hold = [[ps_ln2.tile([1, 512], F32, tag=f"l2{a}{s}",
                             name=f"l2{a}{s}") for s in range(2)]
                for a in range(2)]
        pending_ep = None
        for ms in range(16):
            m, s = divmod(ms, 2)
            ps = mm_psum()
            for o in range(KT):
                wsrc = (wp_a[:, o, :] if o < KT // 2
                        else wp_b[:, o - KT // 2, :])
                nc.tensor.matmul(
                    ps, wsrc[:, m * 128:(m + 1) * 128],
                    o_lnT[:, o, s * 512:(s + 1) * 512],
                    start=(o == 0), stop=(o == KT - 1))
            if ms == 0:
                emit_subln_stats(oj7, sq7)
                emit_subln_rows()
            if pending_ep is not None:
                proj_epilogue(*pending_ep)
                if pending_ep[1] == 1:
                    emit_ln2_tile_stats(pending_ep[0], hold)
            pending_ep = (m, s, ps)
        proj_epilogue(*pending_ep)
        emit_ln2_tile_stats(DT - 1, hold)
        for s in range(2):
            nc.scalar.copy(out=sum2_row[:, s * 512:(s + 1) * 512],
                           in_=hold[0][s])
            nc.scalar.copy(out=sq2_row[:, s * 512:(s + 1) * 512],
                           in_=hold[1][s])
    p_xTr.release()
    p_wB.release()
    p_OlnT.release()
    p_wA.release()
    if "xnT" in dbg_d:
        nc.sync.dma_start(out=dbg_d["xnT"][:], in_=xnT[:])

    # =====================================================================
    # PH7: LN2 -> x2T (bf16)
    # =====================================================================
    p_x2T = tc.alloc_tile_pool(name="p_x2T", bufs=1)
    x2T = p_x2T.tile([128, DT, NTOK], BF16, name="x2T")
    m_r, s_r, negm_r = ln_rows(sum2_row, sq2_row, D)
    fm_ln_apply(xnT, x2T, DT, negm_r, s_r, "n2")
    if "x2T" in dbg_d:
        nc.sync.dma_start(out=dbg_d["x2T"][:], in_=x2T[:])

    # =====================================================================
    # PH8: fc1 + gelu -> hT (bf16), with fused ffn_ln stats
    # =====================================================================
    p_hT = tc.alloc_tile_pool(name="p_hT", bufs=1, side="right")
    hT = p_hT.tile([128, HT, NTOK], BF16, name="hT")
    hsum_row = rows.tile([1, NTOK], F32, tag="sum_row", name="hsum_row")
    hsq_row = rows.tile([1, NTOK], F32, tag="sq_row", name="hsq_row")
    skip_b1 = "b1" in triv
    # ffn_ln stats via held accumulating ones-matmuls on the PE (the old DVE
    # accumulation made Vector the fc1 bottleneck at 93% busy); squares on DVE
    # (bf16, cheap), cross-partition sums ride 4 held psum banks.
    with tc.tile_pool(name="p_wblk", bufs=3) as p_wblk, \
         tc.tile_pool(name="ps_ffn", bufs=1, space="PSUM") as ps_ffn:
        ps_sum = [ps_ffn.tile([1, 512], F32, tag=f"ffsum{s}", name=f"ffsum{s}")
                  for s in range(2)]
        ps_sq = [ps_ffn.tile([1, 512], F32, tag=f"ffsq{s}", name=f"ffsq{s}")
                 for s in range(2)]
        for hm in range(HT):
            w1blk = p_wblk.tile([128, KT, 128], BF16, tag="w1blk",
                                name="w1blk")
            nc.sync.dma_start(out=w1blk, in_=w1_d[hm])
            for s in range(2):
                ps = mm_psum()
                for o in range(KT):
                    nc.tensor.matmul(
                        ps, w1blk[:, o, :],
                        x2T[:, o, s * 512:(s + 1) * 512],
                        start=(o == 0), stop=(o == KT - 1))
                hslice = hT[:, hm, s * 512:(s + 1) * 512]
                if skip_b1:
                    nc.scalar.activation(out=hslice, in_=ps, func=AF.Gelu)
                else:
                    nc.scalar.activation(
                        out=hslice, in_=ps, func=AF.Gelu,
                        bias=st["b1"][:, hm:hm + 1], scale=1.0)
                sq_t = sqp.tile([128, NTOK], BF16, tag="sq_t",
                                name="sq_tf")[:, :512]
                nc.vector.tensor_tensor(sq_t, hslice, hslice, ALU.mult)
                nc.tensor.matmul(
                    ps_sum[s], ones_b, hslice,
                    start=(hm == 0), stop=(hm == HT - 1),
                    skip_group_check=True)
                nc.tensor.matmul(
                    ps_sq[s], ones_b, sq_t,
                    start=(hm == 0), stop=(hm == HT - 1),
                    skip_group_check=True)
        for s in range(2):
            nc.scalar.copy(out=hsum_row[:, s * 512:(s + 1) * 512],
                           in_=ps_sum[s])
            nc.scalar.copy(out=hsq_row[:, s * 512:(s + 1) * 512],
                           in_=ps_sq[s])
    p_x2T.release()
    if "hT" in dbg_d:
        nc.sync.dma_start(out=dbg_d["hT"][:], in_=hT[:])

    # =====================================================================
    # PH9: ffn_ln rows only (normalization deferred into fc2: an extra K=1
    # matmul row adds -mean*colsum(W2); psum scaled by rstd in the epilogue)
    # =====================================================================
    _, s_ffn, negm_ffn = ln_rows(hsum_row, hsq_row, HID)
    w2sum_st = consts.tile([128, DT], F32, name="w2sum_st")
    nc.sync.dma_start(out=w2sum_st,
                      in_=pvec["w2sum"].rearrange("(o p) -> p o", p=128))
    ns_ffn = rows.tile([1, NTOK], BF16, tag="nsrow", name="ns_ffn", bufs=1)
    nc.vector.tensor_tensor(ns_ffn, negm_ffn, s_ffn, ALU.mult)
    fbc = {}

    # =====================================================================
    # PH10: fc2 (with deferred ffn_ln) + residual + transpose + store
    # =====================================================================
    skip_b2 = "b2" in triv

    def fc2_epilogue(m, s, ps):
        corr2 = tmps.tile([128, 512], BF16, tag="corr",
                          name="corr2", bufs=2)
        nc.vector.tensor_scalar_mul(
            out=corr2, in0=fbc["nffn"][:, s * 512:(s + 1) * 512],
            scalar1=w2sum_st[:, m:m + 1])
        outm = p_out.tile([128, 512], F32, tag="outm", name="outm")
        nc.vector.tensor_tensor(
            ps, ps, fbc["sffn"][:, s * 512:(s + 1) * 512], ALU.mult)
        nc.vector.tensor_tensor(ps, ps, corr2, ALU.add)
        if not skip_b2:
            nc.vector.tensor_scalar(
                out=ps, in0=ps, scalar1=st["b2"][:, m:m + 1],
                scalar2=None, op0=ALU.add)
        nc.vector.tensor_tensor(
            outm, ps, xnT[:, m, s * 512:(s + 1) * 512], ALU.add)
        nc.sync.dma_start(
            out=y_d[:, m, s * 512:(s + 1) * 512], in_=outm)

    with tc.tile_pool(name="p_w2blk", bufs=3) as p_w2blk, \
         tc.tile_pool(name="p_out", bufs=4) as p_out:
        pending = None
        for ms in range(16):
            m, s = divmod(ms, 2)
            if s == 0:
                w2blk = p_w2blk.tile([128, HT, 128], BF16, tag="w2blk",
                                     name="w2blk")
                nc.sync.dma_start(out=w2blk, in_=w2_d[m])
            ps = mm_psum()
            for o in range(HT):
                nc.tensor.matmul(
                    ps, w2blk[:, o, :],
                    hT[:, o, s * 512:(s + 1) * 512],
                    start=(o == 0), stop=(o == HT - 1))
            if ms == 0:
                # emit the row broadcasts here: their PE matmuls resolve
                # under the first 32-MM chain instead of blocking it
                fbc["sffn"] = bcast_b(s_ffn)
                fbc["nffn"] = bcast_b(ns_ffn)
            if pending is not None:
                fc2_epilogue(*pending)
            pending = (m, s, ps)
        fc2_epilogue(*pending)
    p_hT.release()
    p_xnT.release()
    ctx.close()


# --------------------------------------------------------------------------
# host glue
# --------------------------------------------------------------------------

_PROGRAM_CACHE = {}


def get_program(debug=(), triv=frozenset()):
    key = (tuple(sorted(debug)), tuple(sorted(triv)))
    if key not in _PROGRAM_CACHE:
        _PROGRAM_CACHE[key] = build_program(debug=key[0], triv=key[1])
    return _PROGRAM_CACHE[key]


def compute_triv(inputs):
    f32 = np.float32
    triv = set()
    for k in ["n1", "n3", "ln", "n2", "ffn"]:
        g = np.asarray(inputs[k + "_g"], f32)
        b = np.asarray(inputs[k + "_b"], f32)
        if np.all(g == 1.0) and np.all(b == 0.0):
            triv.add(k)
    for k in ["bq", "bv", "b1"]:
        if np.all(np.asarray(inputs[k], f32) == 0.0):
            triv.add(k)
    bp_eff = (np.asarray(inputs["bp"], f32)
              + np.asarray(inputs["ln_b"], f32) @ np.asarray(inputs["Wp"], f32))
    if np.all(bp_eff == 0.0):
        triv.add("bp")
    b2_eff = (np.asarray(inputs["b2"], f32)
              + np.asarray(inputs["ffn_b"], f32) @ np.asarray(inputs["W2"], f32))
    if np.all(b2_eff == 0.0):
        triv.add("b2")
    return frozenset(triv)


def make_in_maps(inputs):
    """Build the 8 per-core input maps from the full-problem input dict."""
    bf = ml_dtypes.bfloat16
    f32 = np.float32

    def host(name):
        return np.asarray(inputs[name], dtype=f32)

    # fold the subln (ln_g/ln_b) into Wp/bp and the ffn_ln (ffn_g/ffn_b)
    # into W2/b2 — the kernel defers those norms into the matmuls and only
    # applies (x-mean)*rstd
    wp_eff = host("ln_g")[:, None] * host("Wp")
    bp_eff = host("bp") + host("ln_b") @ host("Wp")
    w2_eff = host("ffn_g")[:, None] * host("W2")
    b2_eff = host("b2") + host("ffn_b") @ host("W2")
    wp_bf = wp_eff.astype(bf)
    w2_bf = w2_eff.astype(bf)
    shared = dict(
        wq=np.ascontiguousarray(
            host("Wq").reshape(KT, 128, D).astype(bf).transpose(1, 0, 2)),
        wk=np.ascontiguousarray(
            host("Wk").reshape(KT, 128, D).astype(bf).transpose(1, 0, 2)),
        wv=np.ascontiguousarray(
            host("Wv").reshape(KT, 128, D).astype(bf).transpose(1, 0, 2)),
        wp=np.ascontiguousarray(
            wp_bf.reshape(KT, 128, D).transpose(1, 0, 2)),
        wpsum=wp_bf.astype(np.float32).sum(0).astype(f32),
        w1=np.ascontiguousarray(
            host("W1").reshape(KT, 128, HT, 128).transpose(2, 1, 0, 3)
        ).astype(bf),
        w2=np.ascontiguousarray(
            w2_bf.reshape(HT, 128, DT, 128).transpose(2, 1, 0, 3)),
        w2sum=w2_bf.astype(np.float32).sum(0).astype(f32),
        n1_g=host("n1_g"), n1_b=host("n1_b"),
        n3_g=host("n3_g"), n3_b=host("n3_b"),
        bq=host("bq"), bv=host("bv"),
        ln_g=host("ln_g"), ln_b=host("ln_b"),
        bp=bp_eff.astype(f32),
        n2_g=host("n2_g"), n2_b=host("n2_b"),
        b1=host("b1"), ffn_g=host("ffn_g"), ffn_b=host("ffn_b"),
        b2=b2_eff.astype(f32),
        gate=host("gate").reshape(H),
    )
    x = host("x")
    xt = host("x_text")
    in_maps = []
    for b in range(B):
        m = dict(shared)
        # device consumes feature-major, partition-major inputs
        m["x"] = np.ascontiguousarray(
            x[b].T.reshape(DT, 128, N).transpose(1, 0, 2)).astype(bf)
        m["x_text"] = np.ascontiguousarray(
            xt[b].T.reshape(DT, 128, PT).transpose(1, 0, 2)).astype(bf)
        in_maps.append(m)
    return in_maps


def unpack_y(y):
    """Device output is feature-major [128, DT, NTOK]; back to [NTOK, D]."""
    y = np.asarray(y)
    return np.transpose(y, (2, 1, 0)).reshape(NTOK, D)


def kernel(**inputs) -> np.ndarray:
    from concourse.bass_utils import run_bass_kernel_spmd

    nc = get_program(triv=compute_triv(inputs))
    in_maps = make_in_maps(inputs)
    res = run_bass_kernel_spmd(nc, in_maps, list(range(B)))
    out = np.stack([unpack_y(res.results[b]["y"]) for b in range(B)], axis=0)
    return out.astype(np.float32)



# revision 52
# speedup vs baseline: 1.3099x; 1.0014x over previous
"""Trainium2 Bass kernel for nn_Block_45724221833665 (dense transformer block).

Strategy: pure data-parallel over batch — 8 batch elements, 8 NeuronCores, one
batch element per core, no collectives.  Inside a core everything is computed
"feature-major" (features on SBUF partitions, tokens on the free dim) so that:
  * all matmuls consume operands in their natural layout (weights as lhsT),
  * all per-feature biases/gains are per-partition scalars,
  * attention needs no transposes at all (scores are computed as S^T, attn@v
    takes exp(S^T) directly as the moving operand with V as the stationary
    operand, and the per-token softmax denominator comes from an extra all-ones
    column appended to V).
Inputs arrive host-pre-transposed (feature-major) and the output is returned
feature-major and transposed back on the host, so the device does no
transposes at all.

LayerNorms in feature-major form need cross-partition sums; those are done on
the tensor engine with an all-ones stationary vector over bf16 casts, and the
per-token mean/rstd rows are broadcast across partitions by DMA via a small
DRAM bounce.  Reciprocals run on ACT (the DVE iterative divide is ~8
cycles/element and our rows live on one partition); the LN rstd gets one
Newton step on DVE to recover full precision.

The gated dual softmax simplifies: softmax over a single key (column 0) is
identically 1, so the reference's w_prefix column is just tanh(gate) per head.
Attention output = softmax over the other 1087 keys @ v  +  tanh(gate)*v[key0]
(+ (1+tanh(gate))*bv once the v-bias is folded out of the softmax-weighted sum,
since the main softmax weights sum to 1).

Gains that are identically 1 and biases identically 0 (the common case for
this block) are detected on the host and their application elided; the cache
key of the built program includes those flags, so nontrivial parameters still
take the general path.
"""

import numpy as np
import ml_dtypes

import concourse.bass as bass
import concourse.mybir as mybir
import concourse.tile as tile
from concourse import bacc

B, N, PT, D, H, HD, HID = 8, 1024, 64, 1024, 16, 64, 4096
KT = D // 128          # 8 contraction tiles over D
HT = HID // 128        # 32 tiles over HID
DT = D // 128          # 8 output tiles over D
NTOK = N               # 1024 main tokens per core
NKEY = N + PT          # 1088 keys (main tokens + text prefix)
EPS = 1e-5

F32 = mybir.dt.float32
BF16 = mybir.dt.bfloat16
XDT = BF16  # wire dtype of the (host-pre-transposed) x / x_text inputs
AF = mybir.ActivationFunctionType
ALU = mybir.AluOpType


def build_program(debug=(), triv=frozenset()):
    """Build the single-core Bass program.
    debug: iterable of intermediate names to emit as DRAM outputs.
    triv: parameter groups whose gain/bias application can be skipped."""
    nc = bacc.Bacc("TRN2")
    dbg = set(debug)
    triv = set(triv)

    # ---- I/O ------------------------------------------------------------
    # x / x_text arrive HOST-PRE-TRANSPOSED (feature-major [D, tokens])
    x_d = nc.declare_dram_parameter("x", [128, DT, NTOK], XDT, isOutput=False)
    xt_d = nc.declare_dram_parameter("x_text", [128, DT, PT], XDT,
                                     isOutput=False)
    wq_d = nc.declare_dram_parameter("wq", [128, KT, D], BF16, isOutput=False)
    wk_d = nc.declare_dram_parameter("wk", [128, KT, D], BF16, isOutput=False)
    wv_d = nc.declare_dram_parameter("wv", [128, KT, D], BF16, isOutput=False)
    wp_d = nc.declare_dram_parameter("wp", [128, KT, D], BF16, isOutput=False)
    w1_d = nc.declare_dram_parameter("w1", [HT, 128, KT, 128], BF16,
                                 isOutput=False)
    w2_d = nc.declare_dram_parameter("w2", [DT, 128, HT, 128], BF16,
                                     isOutput=False)
    pvec = {}
    for nm, sz in [
        ("n1_g", D), ("n1_b", D), ("n3_g", D), ("n3_b", D),
        ("bq", D), ("bv", D), ("ln_g", D), ("ln_b", D), ("bp", D),
        ("n2_g", D), ("n2_b", D), ("b1", HID), ("ffn_g", HID), ("ffn_b", HID),
        ("b2", D),
    ]:
        pvec[nm] = nc.declare_dram_parameter(nm, [sz], F32, isOutput=False)
    pvec["wpsum"] = nc.declare_dram_parameter("wpsum", [D], F32,
                                              isOutput=False)
    pvec["w2sum"] = nc.declare_dram_parameter("w2sum", [D], F32,
                                              isOutput=False)
    gate_d = nc.declare_dram_parameter("gate", [H], F32, isOutput=False)
    # y is produced feature-major [128, DT, NTOK]; host transposes back
    y_d = nc.declare_dram_parameter("y", [128, DT, NTOK], F32, isOutput=True)

    dbg_d = {}

    def dbg_out(name, shape, dtype):
        if name in dbg:
            dbg_d[name] = nc.declare_dram_parameter(
                "dbg_" + name, list(shape), dtype, isOutput=True
            )

    dbg_out("xT", [128, DT, NTOK], F32)
    dbg_out("x1T", [128, DT, NTOK], BF16)
    dbg_out("xt1T", [128, DT, PT], BF16)
    dbg_out("qT", [128, DT, NTOK], BF16)
    dbg_out("kT", [128, DT, NKEY], BF16)
    dbg_out("v", [128, KT, H, HD + 1], BF16)
    dbg_out("v_text", [PT, H, HD + 1], BF16)
    dbg_out("c_vec", [D], F32)
    dbg_out("pp0", [128, 2, KT, NTOK], BF16)     # exp(S^T) for heads 0,1
    dbg_out("pp0_text", [PT, 2, NTOK], BF16)
    dbg_out("o_full", [128, DT, NTOK], F32)
    dbg_out("o_lnT", [128, DT, NTOK], BF16)
    dbg_out("xnT", [128, DT, NTOK], F32)
    dbg_out("x2T", [128, DT, NTOK], BF16)
    dbg_out("hT", [128, HT, NTOK], BF16)

    with tile.TileContext(nc) as tc:
        _build_phases(nc, tc, x_d, xt_d, wq_d, wk_d, wv_d, wp_d,
                      w1_d, w2_d, pvec, gate_d, y_d, dbg_d, triv)
    nc.compile()
    return nc


def _build_phases(nc, tc, x_d, xt_d, wq_d, wk_d, wv_d, wp_d, w1_d, w2_d,
                  pvec, gate_d, y_d, dbg_d, triv):
    import contextlib
    ctx = contextlib.ExitStack()
    # ---- whole-kernel pools ---------------------------------------------
    consts = ctx.enter_context(tc.tile_pool(name="consts", bufs=1))
    rows = ctx.enter_context(tc.tile_pool(name="rows", bufs=1))
    tmps = ctx.enter_context(tc.tile_pool(name="tmps", bufs=2))
    sqp = ctx.enter_context(tc.tile_pool(name="sqp", bufs=2))
    dram = ctx.enter_context(tc.tile_pool(name="dram", bufs=1, space="DRAM"))
    ps_mm = ctx.enter_context(tc.tile_pool(name="ps_mm", bufs=3, space="PSUM"))
    # p_qkv sits below p_x1T on the pool stack so x1T can be freed after the
    # projections while qT/kT/v live on through attention
    p_qkv = tc.alloc_tile_pool(name="p_qkv", bufs=1)
    p_x1T = tc.alloc_tile_pool(name="p_x1T", bufs=1)

    def mm_psum(pfree=512, parts=128):
        t = ps_mm.tile([128, 512], F32, tag="mm", name="mmps")
        return t[:parts, :pfree]

    # ---- constants ------------------------------------------------------
    ones_b = consts.tile([128, 1], BF16, name="ones_b")
    nc.vector.memset(ones_b, 1.0)
    ones_r = consts.tile([1, 128], BF16, name="ones_r")
    nc.vector.memset(ones_r, 1.0)
    eps_c = consts.tile([1, 1], F32, name="eps_c")
    nc.vector.memset(eps_c, EPS)

    st = {}  # striped parameter tiles [128, tiles]; triv params never read,
    # so skip their DMAs (each is 128 tiny descriptors clogging the queues)
    def group_used(nm):
        g = nm.split("_")[0] if "_" in nm else nm
        return g not in triv
    for nm in ["n1_g", "n1_b", "n3_g", "n3_b", "bq", "ln_g", "ln_b", "bp",
               "n2_g", "n2_b", "b2"]:
        if not group_used(nm):
            continue
        t = consts.tile([128, DT], F32, name="st_" + nm)
        nc.sync.dma_start(out=t, in_=pvec[nm].rearrange("(o p) -> p o", p=128))
        st[nm] = t
    for nm in ["b1", "ffn_g", "ffn_b"]:
        if not group_used(nm):
            continue
        t = consts.tile([128, HT], F32, name="st_" + nm)
        nc.sync.dma_start(out=t, in_=pvec[nm].rearrange("(o p) -> p o", p=128))
        st[nm] = t

    def bcast_b(row, ntok=NTOK):
        """Broadcast a [1, ntok] row to a [128, ntok] bf16 tile via a K=1
        PE outer product with an all-ones stationary (the PE is idle at the
        LN phase boundaries where these sit, and its latency is ~1us vs the
        multi-us dispatch stalls seen on GPSIMD/DMA-bounce paths)."""
        if row.dtype != BF16:
            br = rows.tile([1, NTOK], BF16, tag="brow", name="brow",
                           bufs=1)[:, :ntok]
            nc.vector.tensor_copy(out=br, in_=row)
            row = br
        out = tmps.tile([128, NTOK], BF16, tag="wrk", name="bb")[:, :ntok]
        nsl = max(1, ntok // 512)
        sl = ntok // nsl
        for s in range(nsl):
            ps = mm_psum(sl)
            nc.tensor.matmul(ps, ones_r, row[:, s * sl:(s + 1) * sl],
                             start=True, stop=True, skip_group_check=True)
            nc.vector.tensor_copy(out=out[:, s * sl:(s + 1) * sl], in_=ps)
        return out

    def ln_rows(sum_row, sq_row, n_elems):
        """Turn per-token sums into (mean, rstd, -mean[bf16]) rows.  rstd =
        ACT abs_reciprocal_sqrt(var+eps) refined by one rsqrt-Newton step on
        DVE (the banned-for-accuracy ACT seed is fine once refined)."""
        ntok = sum_row.shape[-1]
        t_row = rows.tile([1, NTOK], F32, tag="t_row", name="t_row")[:, :ntok]
        s_row = rows.tile([1, NTOK], F32, tag="s_row", name="s_row")[:, :ntok]
        u_row = rows.tile([1, NTOK], F32, tag="u_row", name="u_row",
                          bufs=1)[:, :ntok]
        negm = rows.tile([1, NTOK], BF16, tag="negm", name="negm")[:, :ntok]
        inv = 1.0 / float(n_elems)
        nc.vector.tensor_scalar_mul(out=sum_row, in0=sum_row, scalar1=inv)
        nc.vector.tensor_scalar_mul(out=sq_row, in0=sq_row, scalar1=inv)
        nc.vector.tensor_tensor(t_row, sum_row, sum_row, ALU.mult)  # mean^2
        nc.vector.tensor_tensor(sq_row, sq_row, t_row, ALU.subtract)  # var
        nc.scalar.activation(out=s_row, in_=sq_row,
                             func=AF.Abs_reciprocal_sqrt,
                             bias=eps_c, scale=1.0)                 # ~rstd
        # rsqrt Newton: s <- s*(1.5 - 0.5*(var+eps)*s^2)
        nc.vector.tensor_scalar_add(out=t_row, in0=sq_row, scalar1=EPS)
        nc.vector.tensor_tensor(u_row, s_row, s_row, ALU.mult)
        nc.vector.tensor_tensor(u_row, u_row, t_row, ALU.mult)
        nc.vector.tensor_scalar(out=u_row, in0=u_row, scalar1=-0.5,
                                scalar2=1.5, op0=ALU.mult, op1=ALU.add)
        nc.vector.tensor_tensor(s_row, s_row, u_row, ALU.mult)
        nc.vector.tensor_scalar_mul(out=negm, in0=sum_row, scalar1=-1.0)
        return sum_row, s_row, negm

    # =====================================================================
    # PH1: load x / x_text directly in feature-major layout (host-transposed)
    # =====================================================================
    x1T = p_x1T.tile([128, DT, NTOK], BF16, name="x1T")
    p_xtB = tc.alloc_tile_pool(name="p_xtB", bufs=1)
    xt1T = p_xtB.tile([128, DT, PT], BF16, name="xt1T")
    p_xtA = tc.alloc_tile_pool(name="p_xtA", bufs=1)
    xtT = p_xtA.tile([128, DT, PT], XDT, name="xtT")
    p_xT = tc.alloc_tile_pool(name="p_xT", bufs=1)
    xT = p_xT.tile([128, DT, NTOK], XDT, name="xT")

    nc.sync.dma_start(out=xtT, in_=xt_d[:, :, :])
    for hf in range(2):  # two half-chunks: big descriptors, early stats
        nc.sync.dma_start(
            out=xT[:, hf * 4:(hf + 1) * 4, :],
            in_=x_d[:, hf * 4:(hf + 1) * 4, :])
    if "xT" in dbg_d:
        nc.sync.dma_start(out=dbg_d["xT"][:], in_=xT[:])

    # ---- feature-major layernorm helpers --------------------------------
    def fm_ln_stats(src, ntiles, ntok, sl=512):
        """Cross-partition LN stats for src [128, ntiles, ntok] (f32 or bf16).
        Returns (sum_row, sq_row) [1, ntok] f32 rows (in `rows` pool)."""
        sum_row = rows.tile([1, NTOK], F32, tag="sum_row",
                            name="sum_row")[:, :ntok]
        sq_row = rows.tile([1, NTOK], F32, tag="sq_row",
                           name="sq_row")[:, :ntok]
        nsl = ntok // sl
        with tc.tile_pool(name="ps_stat", bufs=4, space="PSUM") as ps_stat:
            ps_a = [ps_stat.tile([1, sl], F32, tag="stat", name=f"psa{s}",
                                 bufs=2) for s in range(nsl)]
            ps_b = [ps_stat.tile([1, sl], F32, tag="stat2", name=f"psb{s}",
                                 bufs=2) for s in range(nsl)]
            for o in range(ntiles):
                if src.dtype == F32:
                    cst = sqp.tile([128, NTOK], BF16, tag="cst",
                                   name="cst")[:, :ntok]
                    nc.scalar.copy(out=cst, in_=src[:, o, :])
                else:
                    cst = src[:, o, :]
                sq_t = sqp.tile([128, NTOK], BF16, tag="sq_t",
                                name="sq_t")[:, :ntok]
                nc.vector.tensor_tensor(sq_t, cst, cst, ALU.mult)
                for s in range(nsl):
                    nc.tensor.matmul(
                        ps_a[s], ones_b, cst[:, s * sl:(s + 1) * sl],
                        start=(o == 0), stop=(o == ntiles - 1),
                        skip_group_check=True)
                    nc.tensor.matmul(
                        ps_b[s], ones_b, sq_t[:, s * sl:(s + 1) * sl],
                        start=(o == 0), stop=(o == ntiles - 1),
                        skip_group_check=True)
            for s in range(nsl):
                nc.scalar.copy(out=sum_row[:, s * sl:(s + 1) * sl], in_=ps_a[s])
                nc.scalar.copy(out=sq_row[:, s * sl:(s + 1) * sl], in_=ps_b[s])
        return sum_row, sq_row

    def fm_ln_apply(src, dst, ntiles, negm_row, s_row, gkey):
        """dst[:,o,:] = ((src - m)*rstd)[*g + b], slice-pipelined so
        consumers ordered slice-outer can start on slice 0 early."""
        ntok = src.shape[-1]
        g_st = st.get(gkey + "_g")
        b_st = st.get(gkey + "_b")
        skip_gb = gkey in triv
        m_b = bcast_b(negm_row, ntok)
        s_b = bcast_b(s_row, ntok)
        nsl = max(1, ntok // 512)
        sl = ntok // nsl
        for s in range(nsl):
            ss = slice(s * sl, (s + 1) * sl)
            for o in range(ntiles):
                tA = tmps.tile([128, 512], BF16, tag="tA",
                               name="tA")[:, :sl]
                nc.vector.tensor_tensor(tA, src[:, o, ss], m_b[:, ss],
                                        ALU.add)
                if skip_gb:
                    nc.vector.tensor_tensor(dst[:, o, ss], tA, s_b[:, ss],
                                            ALU.mult)
                else:
                    nc.vector.tensor_tensor(tA, tA, s_b[:, ss], ALU.mult)
                    nc.scalar.activation(
                        out=dst[:, o, ss], in_=tA, func=AF.Identity,
                        scale=g_st[:, o:o + 1], bias=b_st[:, o:o + 1])

    # =====================================================================
    # PH2: LN1(x) -> x1T (bf16), LN3(x_text) -> xt1T (bf16)
    # =====================================================================
    sum_r, sq_r = fm_ln_stats(xtT, DT, PT, sl=PT)
    m_r, s_r, negm_r = ln_rows(sum_r, sq_r, D)
    fm_ln_apply(xtT, xt1T, DT, negm_r, s_r, "n3")

    sum_r, sq_r = fm_ln_stats(xT, DT, NTOK)
    m_r, s_r, negm_r = ln_rows(sum_r, sq_r, D)
    fm_ln_apply(xT, x1T, DT, negm_r, s_r, "n1")
    p_xT.release()
    p_xtA.release()

    if "x1T" in dbg_d:
        nc.sync.dma_start(out=dbg_d["x1T"][:], in_=x1T[:])
    if "xt1T" in dbg_d:
        nc.sync.dma_start(out=dbg_d["xt1T"][:], in_=xt1T[:])

    # =====================================================================
    # PH3: projections  Q^T, K^T (feature-major), V (token-major, +ones col)
    # =====================================================================
    qT = p_qkv.tile([128, DT, NTOK], BF16, name="qT")
    kT = p_qkv.tile([128, DT, NKEY], BF16, name="kT")
    v_sb = p_qkv.tile([128, KT, H, HD + 1], BF16, name="v_sb")
    vt_sb = p_qkv.tile([PT, H, HD + 1], BF16, name="vt_sb")

    p_w = tc.alloc_tile_pool(name="p_w", bufs=2)
    wq_sb = p_w.tile([128, KT, D], BF16, tag="wfull", name="wq_sb")
    nc.sync.dma_start(out=wq_sb, in_=wq_d[:, :, :])
    wk_sb = p_w.tile([128, KT, D], BF16, tag="wfull", name="wk_sb")
    nc.sync.dma_start(out=wk_sb, in_=wk_d[:, :, :])
    skip_bq = "bq" in triv
    for s in range(2):
        for m in range(DT):
            ps = mm_psum()
            for o in range(KT):
                nc.tensor.matmul(
                    ps, wq_sb[:, o, m * 128:(m + 1) * 128],
                    x1T[:, o, s * 512:(s + 1) * 512],
                    start=(o == 0), stop=(o == KT - 1))
            if skip_bq:
                nc.scalar.copy(out=qT[:, m, s * 512:(s + 1) * 512], in_=ps)
            else:
                nc.scalar.activation(
                    out=qT[:, m, s * 512:(s + 1) * 512], in_=ps,
                    func=AF.Identity, bias=st["bq"][:, m:m + 1], scale=1.0)
        for m in range(DT):
            ps = mm_psum()
            for o in range(KT):
                nc.tensor.matmul(
                    ps, wk_sb[:, o, m * 128:(m + 1) * 128],
                    x1T[:, o, s * 512:(s + 1) * 512],
                    start=(o == 0), stop=(o == KT - 1))
            nc.scalar.copy(out=kT[:, m, s * 512:(s + 1) * 512], in_=ps)
    for m in range(DT):  # K^T text keys
        ps = mm_psum(PT)
        for o in range(KT):
            nc.tensor.matmul(
                ps, wk_sb[:, o, m * 128:(m + 1) * 128], xt1T[:, o, :],
                start=(o == 0), stop=(o == KT - 1))
        nc.scalar.copy(out=kT[:, m, N:N + PT], in_=ps)
    # V token-major, heads interleaved with the all-ones 65th column
    wv_sb = p_w.tile([128, KT, D], BF16, tag="wfull", name="wv_sb")
    nc.sync.dma_start(out=wv_sb, in_=wv_d[:, :, :])
    for t in range(8):
        for sn in range(2):
            ps = mm_psum()
            for o in range(KT):
                nc.tensor.matmul(
                    ps, x1T[:, o, t * 128:(t + 1) * 128],
                    wv_sb[:, o, sn * 512:(sn + 1) * 512],
                    start=(o == 0), stop=(o == KT - 1))
            nc.scalar.copy(
                out=v_sb[:, t, sn * 8:(sn + 1) * 8, 0:HD], in_=ps)
    for sn in range(2):
        ps = mm_psum(parts=PT)
        for o in range(KT):
            nc.tensor.matmul(
                ps, xt1T[:, o, :], wv_sb[:, o, sn * 512:(sn + 1) * 512],
                start=(o == 0), stop=(o == KT - 1))
        nc.scalar.copy(out=vt_sb[:, sn * 8:(sn + 1) * 8, 0:HD], in_=ps)
    p_w.release()
    p_xtB.release()
    p_x1T.release()

    # ---- c_vec: tanh(gate)*v0_raw + (1+tanh(gate))*bv  ------------------
    g_row = rows.tile([1, H], F32, tag="g_row", name="g_row")
    nc.sync.dma_start(out=g_row, in_=gate_d.rearrange("(a h) -> a h", a=1))
    th_row = rows.tile([1, H], F32, tag="th_row", name="th_row")
    nc.scalar.activation(out=th_row, in_=g_row, func=AF.Tanh)
    c_work = rows.tile([1, H, HD], F32, tag="t_row", name="c_work")
    nc.vector.tensor_copy(out=c_work, in_=vt_sb[0:1, :, 0:HD])
    nc.vector.tensor_tensor(
        c_work, c_work, th_row[:, :, None].to_broadcast((1, H, HD)), ALU.mult)
    if "bv" not in triv:
        th1_row = rows.tile([1, H], F32, tag="th1_row", name="th1_row")
        nc.scalar.activation(out=th1_row, in_=th_row, func=AF.Identity,
                             bias=1.0)
        bv_row = rows.tile([1, H, HD], F32, tag="s_row", name="bv_row")
        nc.sync.dma_start(
            out=bv_row, in_=pvec["bv"].rearrange("(a h d) -> a h d", a=1, h=H))
        nc.vector.tensor_tensor(
            bv_row, bv_row, th1_row[:, :, None].to_broadcast((1, H, HD)),
            ALU.mult)
        nc.vector.tensor_tensor(c_work, c_work, bv_row, ALU.add)
    c_dram = dram.tile([D], F32, name="c_dram")
    nc.sync.dma_start(
        out=c_dram.rearrange("(a h d) -> a h d", a=1, h=H), in_=c_work)
    c_st = consts.tile([128, DT], F32, name="c_st")
    nc.sync.dma_start(out=c_st, in_=c_dram.rearrange("(o p) -> p o", p=128))
    if "c_vec" in dbg_d:
        nc.sync.dma_start(out=dbg_d["c_vec"][:], in_=c_dram[:])

    # ones column + zero out reference-key-0 (first text token)
    nc.vector.memset(v_sb[:, :, :, HD:HD + 1], 1.0)
    nc.vector.memset(vt_sb[:, :, HD:HD + 1], 1.0)
    nc.vector.memset(vt_sb[0:1, :, :], 0.0)

    # prefetch the first half of the proj weight while attention runs
    p_wA = tc.alloc_tile_pool(name="p_wA", bufs=1, side="right")
    wp_a = p_wA.tile([128, KT // 2, D], BF16, name="wp_a")
    nc.sync.dma_start(out=wp_a, in_=wp_d[:, 0:KT // 2, :])

    # =====================================================================
    # PH4: attention — per (pair, kt): 4 score matmuls into one 4-bank
    # [128,2048] psum, ONE exp over all 2048 cols (amortizes the ~350-cycle
    # ACT ramp), attn@v of the previous pair interleaved into emission so the
    # PE fills the exp-wait bubbles; softmax reciprocal on DVE (no ACT table
    # switches); subln stats of the last pair deferred into the proj phase.
    # =====================================================================
    p_OlnT = tc.alloc_tile_pool(name="p_OlnT", bufs=1, side="right")
    o_lnT = p_OlnT.tile([128, DT, NTOK], BF16, name="o_lnT")
    sumO_row = rows.tile([1, NTOK], F32, tag="sum_row", name="sumO_row")
    sqO_row = rows.tile([1, NTOK], F32, tag="sq_row", name="sqO_row")
    nc.vector.memset(sumO_row, 0.0)
    nc.vector.memset(sqO_row, 0.0)

    def emit_subln_stats(oj, sq_t4):
        for s in range(2):
            ps_a = mm_psum()[:1, :]
            nc.tensor.matmul(
                ps_a, ones_b, oj[:, s * 512:(s + 1) * 512],
                start=True, stop=True, skip_group_check=True)
            nc.vector.tensor_tensor(
                sumO_row[:, s * 512:(s + 1) * 512],
                sumO_row[:, s * 512:(s + 1) * 512], ps_a, ALU.add)
            ps_b = mm_psum()[:1, :]
            nc.tensor.matmul(
                ps_b, ones_b, sq_t4[:, s * 512:(s + 1) * 512],
                start=True, stop=True, skip_group_check=True)
            nc.vector.tensor_tensor(
                sqO_row[:, s * 512:(s + 1) * 512],
                sqO_row[:, s * 512:(s + 1) * 512], ps_b, ALU.add)

    attn_ctx = contextlib.ExitStack()
    p_attn = attn_ctx.enter_context(tc.tile_pool(name="p_attn", bufs=2))
    ps_big = attn_ctx.enter_context(
        tc.tile_pool(name="ps_big", bufs=2, space="PSUM"))

    def emit_normalize(j, se_pr, with_stats=True):
        nc.vector.reciprocal_approx_fast(out=se_pr, in_=se_pr)
        se_b = rows.tile([1, 2, NTOK], BF16, tag="se_b", name="se_b",
                         bufs=1)
        nc.vector.tensor_copy(out=se_b, in_=se_pr)
        rb = tmps.tile([128, 2, NTOK], BF16, tag="rb", name="rb", bufs=1)
        nc.gpsimd.partition_broadcast(rb, se_b)
        oj = o_lnT[:, j, :]
        nc.vector.tensor_tensor(oj[0:64, :], oj[0:64, :], rb[0:64, 0, :],
                                ALU.mult)
        nc.vector.tensor_tensor(oj[64:128, :], oj[64:128, :],
                                rb[64:128, 1, :], ALU.mult)
        nc.vector.tensor_scalar_add(out=oj, in0=oj,
                                    scalar1=c_st[:, j:j + 1])
        sq_t4 = sqp.tile([128, NTOK], BF16, tag="sq_t", name="sq_t4")
        nc.vector.tensor_tensor(sq_t4, oj, oj, ALU.mult)
        if with_stats:
            emit_subln_stats(oj, sq_t4)
        return oj, sq_t4

    def attnv_chunks(jj, pp, ppt, se_pr):
        chunks = []
        for hh in range(2):
            for s in range(2):
                def ch(hh=hh, s=s):
                    h = 2 * jj + hh
                    base = hh * 64
                    ps = mm_psum()[:HD + 1, :]
                    for kt in range(KT):
                        nc.tensor.matmul(
                            ps, v_sb[:, kt, h, :],
                            pp[:, kt,
                               hh * 1024 + s * 512:hh * 1024 + (s + 1) * 512],
                            start=(kt == 0), stop=False,
                            skip_group_check=True)
                    nc.tensor.matmul(
                        ps, vt_sb[:, h, :],
                        ppt[:, hh, s * 512:(s + 1) * 512],
                        start=False, stop=True, skip_group_check=True)
                    nc.vector.tensor_copy(
                        out=o_lnT[base:base + 64, jj, s * 512:(s + 1) * 512],
                        in_=ps[0:HD, :])
                    nc.vector.tensor_copy(
                        out=se_pr[:, hh, s * 512:(s + 1) * 512],
                        in_=ps[HD:HD + 1, :])
                chunks.append(ch)
        return chunks

    prev = None
    for j in range(8):      # head pairs
        pp = p_attn.tile([128, KT, 2048], BF16, tag="pp", name="pp")
        ppt = p_attn.tile([PT, 2, NTOK], BF16, tag="ppt", name="ppt")
        pend = attnv_chunks(*prev) if prev is not None else []
        for kt in range(KT):
            for hh in range(2):
                base = hh * 64
                psb = ps_big.tile([128, 1024], F32, tag="sc", name="psb")
                for s in range(2):
                    nc.tensor.matmul(
                        psb[:, s * 512:(s + 1) * 512],
                        kT[base:base + 64, j, kt * 128:(kt + 1) * 128],
                        qT[base:base + 64, j, s * 512:(s + 1) * 512],
                        start=True, stop=True, tile_position=(base, 0),
                        skip_group_check=True)
                nc.scalar.activation(
                    out=pp[:, kt, hh * 1024:(hh + 1) * 1024], in_=psb,
                    func=AF.Exp, scale=0.125)
            if kt % 2 == 0 and pend:
                pend.pop(0)()
        for hh in range(2):
            base = hh * 64
            psb = ps_big.tile([128, 1024], F32, tag="sc", name="psb")
            for s in range(2):
                nc.tensor.matmul(
                    psb[:PT, s * 512:(s + 1) * 512],
                    kT[base:base + 64, j, N:N + PT],
                    qT[base:base + 64, j, s * 512:(s + 1) * 512],
                    start=True, stop=True, tile_position=(base, 0),
                    skip_group_check=True)
            nc.scalar.activation(
                out=ppt[:, hh, :], in_=psb[:PT, :], func=AF.Exp,
                scale=0.125)
        for ch in pend:
            ch()
        if prev is not None:
            emit_normalize(prev[0], prev[3])
        se_pr = p_attn.tile([1, 2, NTOK], F32, tag="se_pr",
                            name="se_pr", bufs=1)
        prev = (j, pp, ppt, se_pr)
    # tail: attn@v + normalize of pair 7; its subln stats ride after the
    # first proj chain so they do not block the proj matmuls in the PE FIFO
    for ch in attnv_chunks(*prev):
        ch()
    oj7, sq7 = emit_normalize(7, prev[3], with_stats=False)
    attn_ctx.close()

    p_qkv.release()
    if "o_lnT" in dbg_d:
        nc.sync.dma_start(out=dbg_d["o_lnT"][:], in_=o_lnT[:])

    # =====================================================================
    # PH5+6: proj (deferred subln) + residual -> xnT, software-pipelined:
    # chain(ms) ... epilogue(ms-1); subln rows computed after chain 0;
    # LN2 stats interleaved per output tile.
    # =====================================================================
    p_xnT = tc.alloc_tile_pool(name="p_xnT", bufs=1)
    xnT = p_xnT.tile([128, DT, NTOK], F32, name="xnT")
    p_wB = tc.alloc_tile_pool(name="p_wB", bufs=1, side="right")
    wp_b = p_wB.tile([128, KT - KT // 2, D], BF16, name="wp_b")
    nc.sync.dma_start(out=wp_b, in_=wp_d[:, KT // 2:, :])
    wpsum_st = consts.tile([128, DT], F32, name="wpsum_st")
    nc.sync.dma_start(out=wpsum_st,
                      in_=pvec["wpsum"].rearrange("(o p) -> p o", p=128))
    bc = {}

    # reload x^T for the residual (straight from the pre-transposed input)
    p_xTr = tc.alloc_tile_pool(name="p_xTr", bufs=1)
    xTr = p_xTr.tile([128, DT, NTOK], XDT, name="xTr")
    nc.sync.dma_start(out=xTr, in_=x_d[:, :, :])

    skip_bp = "bp" in triv
    sum2_row = rows.tile([1, NTOK], F32, tag="sum_row", name="sum2_row")
    sq2_row = rows.tile([1, NTOK], F32, tag="sq_row", name="sq2_row")

    def emit_subln_rows():
        _, s_sub, negm_sub = ln_rows(sumO_row, sqO_row, D)
        bc["ssub"] = bcast_b(s_sub)
        ns_sub = rows.tile([1, NTOK], BF16, tag="nsrow", name="ns_sub",
                           bufs=1)
        nc.vector.tensor_tensor(ns_sub, negm_sub, s_sub, ALU.mult)
        bc["nsub"] = bcast_b(ns_sub)

    def proj_epilogue(m, s, ps):
        corr_m = tmps.tile([128, 512], BF16, tag="corr", name="corr_m",
                           bufs=2)
        nc.vector.tensor_scalar_mul(
            out=corr_m, in0=bc["nsub"][:, s * 512:(s + 1) * 512],
            scalar1=wpsum_st[:, m:m + 1])
        nc.vector.tensor_tensor(
            ps, ps, bc["ssub"][:, s * 512:(s + 1) * 512], ALU.mult)
        nc.vector.tensor_tensor(ps, ps, corr_m, ALU.add)
        if not skip_bp:
            nc.vector.tensor_scalar(
                out=ps, in0=ps, scalar1=st["bp"][:, m:m + 1],
                scalar2=None, op0=ALU.add)
        nc.vector.tensor_tensor(
            xnT[:, m, s * 512:(s + 1) * 512], ps,
            xTr[:, m, s * 512:(s + 1) * 512], ALU.add)

    def emit_ln2_tile_stats(m, hold):
        cst = sqp.tile([128, NTOK], BF16, tag="cst", name="cst2")
        nc.scalar.copy(out=cst, in_=xnT[:, m, :])
        sq_t = sqp.tile([128, NTOK], BF16, tag="sq_t", name="sq_t2")
        nc.vector.tensor_tensor(sq_t, cst, cst, ALU.mult)
        for s in range(2):
            nc.tensor.matmul(
                hold[0][s], ones_b, cst[:, s * 512:(s + 1) * 512],
                start=(m == 0), stop=(m == DT - 1), skip_group_check=True)
            nc.tensor.matmul(
                hold[1][s], ones_b, sq_t[:, s * 512:(s + 1) * 512],
                start=(m == 0), stop=(m == DT - 1), skip_group_check=True)

    with tc.tile_pool(name="ps_ln2", bufs=1, space="PSUM") as ps_ln2:
        hold = [[ps_ln2.tile([1, 512], F32, tag=f"l2{a}{s}",
                             name=f"l2{a}{s}") for s in range(2)]
                for a in range(2)]
        pend_ep = []
        for ms in range(16):
            m, s = divmod(ms, 2)
            ps = mm_psum()
            for o in range(KT):
                wsrc = (wp_a[:, o, :] if o < KT // 2
                        else wp_b[:, o - KT // 2, :])
                nc.tensor.matmul(
                    ps, wsrc[:, m * 128:(m + 1) * 128],
                    o_lnT[:, o, s * 512:(s + 1) * 512],
                    start=(o == 0), stop=(o == KT - 1))
            if ms == 0:
                emit_subln_stats(oj7, sq7)
            if ms == 1:
                emit_subln_rows()
            pend_ep.append((m, s, ps))
            if ms >= 2:
                ep = pend_ep.pop(0)
                proj_epilogue(*ep)
                if ep[1] == 1:
                    emit_ln2_tile_stats(ep[0], hold)
        for ep in pend_ep:
            proj_epilogue(*ep)
            if ep[1] == 1:
                emit_ln2_tile_stats(ep[0], hold)
        for s in range(2):
            nc.scalar.copy(out=sum2_row[:, s * 512:(s + 1) * 512],
                           in_=hold[0][s])
            nc.scalar.copy(out=sq2_row[:, s * 512:(s + 1) * 512],
                           in_=hold[1][s])
    p_xTr.release()
    p_wB.release()
    p_OlnT.release()
    p_wA.release()
    if "xnT" in dbg_d:
        nc.sync.dma_start(out=dbg_d["xnT"][:], in_=xnT[:])

    # =====================================================================
    # PH7: LN2 -> x2T (bf16)
    # =====================================================================
    p_x2T = tc.alloc_tile_pool(name="p_x2T", bufs=1)
    x2T = p_x2T.tile([128, DT, NTOK], BF16, name="x2T")
    m_r, s_r, negm_r = ln_rows(sum2_row, sq2_row, D)
    fm_ln_apply(xnT, x2T, DT, negm_r, s_r, "n2")
    if "x2T" in dbg_d:
        nc.sync.dma_start(out=dbg_d["x2T"][:], in_=x2T[:])

    # =====================================================================
    # PH8: fc1 + gelu -> hT (bf16), with fused ffn_ln stats
    # =====================================================================
    p_hT = tc.alloc_tile_pool(name="p_hT", bufs=1, side="right")
    hT = p_hT.tile([128, HT, NTOK], BF16, name="hT")
    hsum_row = rows.tile([1, NTOK], F32, tag="sum_row", name="hsum_row")
    hsq_row = rows.tile([1, NTOK], F32, tag="sq_row", name="hsq_row")
    skip_b1 = "b1" in triv
    # ffn_ln stats via held accumulating ones-matmuls on the PE (the old DVE
    # accumulation made Vector the fc1 bottleneck at 93% busy); squares on DVE
    # (bf16, cheap), cross-partition sums ride 4 held psum banks.
    with tc.tile_pool(name="p_wblk", bufs=3) as p_wblk, \
         tc.tile_pool(name="ps_ffn", bufs=1, space="PSUM") as ps_ffn:
        ps_sum = [ps_ffn.tile([1, 512], F32, tag=f"ffsum{s}", name=f"ffsum{s}")
                  for s in range(2)]
        ps_sq = [ps_ffn.tile([1, 512], F32, tag=f"ffsq{s}", name=f"ffsq{s}")
                 for s in range(2)]
        for hm in range(HT):
            w1blk = p_wblk.tile([128, KT, 128], BF16, tag="w1blk",
                                name="w1blk")
            nc.sync.dma_start(out=w1blk, in_=w1_d[hm])
            for s in range(2):
                ps = mm_psum()
                for o in range(KT):
                    nc.tensor.matmul(
                        ps, w1blk[:, o, :],
                        x2T[:, o, s * 512:(s + 1) * 512],
                        start=(o == 0), stop=(o == KT - 1))
                hslice = hT[:, hm, s * 512:(s + 1) * 512]
                if skip_b1:
                    nc.scalar.activation(out=hslice, in_=ps, func=AF.Gelu)
                else:
                    nc.scalar.activation(
                        out=hslice, in_=ps, func=AF.Gelu,
                        bias=st["b1"][:, hm:hm + 1], scale=1.0)
                sq_t = sqp.tile([128, NTOK], BF16, tag="sq_t",
                                name="sq_tf")[:, :512]
                nc.vector.tensor_tensor(sq_t, hslice, hslice, ALU.mult)
                nc.tensor.matmul(
                    ps_sum[s], ones_b, hslice,
                    start=(hm == 0), stop=(hm == HT - 1),
                    skip_group_check=True)
                nc.tensor.matmul(
                    ps_sq[s], ones_b, sq_t,
                    start=(hm == 0), stop=(hm == HT - 1),
                    skip_group_check=True)
        for s in range(2):
            nc.scalar.copy(out=hsum_row[:, s * 512:(s + 1) * 512],
                           in_=ps_sum[s])
            nc.scalar.copy(out=hsq_row[:, s * 512:(s + 1) * 512],
                           in_=ps_sq[s])
    p_x2T.release()
    if "hT" in dbg_d:
        nc.sync.dma_start(out=dbg_d["hT"][:], in_=hT[:])

    # =====================================================================
    # PH9: ffn_ln rows only (normalization deferred into fc2: an extra K=1
    # matmul row adds -mean*colsum(W2); psum scaled by rstd in the epilogue)
    # =====================================================================
    _, s_ffn, negm_ffn = ln_rows(hsum_row, hsq_row, HID)
    w2sum_st = consts.tile([128, DT], F32, name="w2sum_st")
    nc.sync.dma_start(out=w2sum_st,
                      in_=pvec["w2sum"].rearrange("(o p) -> p o", p=128))
    ns_ffn = rows.tile([1, NTOK], BF16, tag="nsrow", name="ns_ffn", bufs=1)
    nc.vector.tensor_tensor(ns_ffn, negm_ffn, s_ffn, ALU.mult)
    fbc = {}

    # =====================================================================
    # PH10: fc2 (with deferred ffn_ln) + residual + transpose + store
    # =====================================================================
    skip_b2 = "b2" in triv

    def fc2_epilogue(m, s, ps):
        corr2 = tmps.tile([128, 512], BF16, tag="corr",
                          name="corr2", bufs=2)
        nc.vector.tensor_scalar_mul(
            out=corr2, in0=fbc["nffn"][:, s * 512:(s + 1) * 512],
            scalar1=w2sum_st[:, m:m + 1])
        outm = p_out.tile([128, 512], F32, tag="outm", name="outm")
        nc.vector.tensor_tensor(
            ps, ps, fbc["sffn"][:, s * 512:(s + 1) * 512], ALU.mult)
        nc.vector.tensor_tensor(ps, ps, corr2, ALU.add)
        if not skip_b2:
            nc.vector.tensor_scalar(
                out=ps, in0=ps, scalar1=st["b2"][:, m:m + 1],
                scalar2=None, op0=ALU.add)
        nc.vector.tensor_tensor(
            outm, ps, xnT[:, m, s * 512:(s + 1) * 512], ALU.add)
        nc.sync.dma_start(
            out=y_d[:, m, s * 512:(s + 1) * 512], in_=outm)

    with tc.tile_pool(name="p_w2blk", bufs=3) as p_w2blk, \
         tc.tile_pool(name="p_out", bufs=4) as p_out:
        pending = None
        for ms in range(16):
            m, s = divmod(ms, 2)
            if s == 0:
                w2blk = p_w2blk.tile([128, HT, 128], BF16, tag="w2blk",
                                     name="w2blk")
                nc.sync.dma_start(out=w2blk, in_=w2_d[m])
            ps = mm_psum()
            for o in range(HT):
                nc.tensor.matmul(
                    ps, w2blk[:, o, :],
                    hT[:, o, s * 512:(s + 1) * 512],
                    start=(o == 0), stop=(o == HT - 1))
            if ms == 0:
                # emit the row broadcasts here: their PE matmuls resolve
                # under the first 32-MM chain instead of blocking it
                fbc["sffn"] = bcast_b(s_ffn)
                fbc["nffn"] = bcast_b(ns_ffn)
            if pending is not None:
                fc2_epilogue(*pending)
            pending = (m, s, ps)
        fc2_epilogue(*pending)
    p_hT.release()
    p_xnT.release()
    ctx.close()


# --------------------------------------------------------------------------
# host glue
# --------------------------------------------------------------------------

_PROGRAM_CACHE = {}


def get_program(debug=(), triv=frozenset()):
    key = (tuple(sorted(debug)), tuple(sorted(triv)))
    if key not in _PROGRAM_CACHE:
        _PROGRAM_CACHE[key] = build_program(debug=key[0], triv=key[1])
    return _PROGRAM_CACHE[key]


def compute_triv(inputs):
    f32 = np.float32
    triv = set()
    for k in ["n1", "n3", "ln", "n2", "ffn"]:
        g = np.asarray(inputs[k + "_g"], f32)
        b = np.asarray(inputs[k + "_b"], f32)
        if np.all(g == 1.0) and np.all(b == 0.0):
            triv.add(k)
    for k in ["bq", "bv", "b1"]:
        if np.all(np.asarray(inputs[k], f32) == 0.0):
            triv.add(k)
    bp_eff = (np.asarray(inputs["bp"], f32)
              + np.asarray(inputs["ln_b"], f32) @ np.asarray(inputs["Wp"], f32))
    if np.all(bp_eff == 0.0):
        triv.add("bp")
    b2_eff = (np.asarray(inputs["b2"], f32)
              + np.asarray(inputs["ffn_b"], f32) @ np.asarray(inputs["W2"], f32))
    if np.all(b2_eff == 0.0):
        triv.add("b2")
    return frozenset(triv)


def make_in_maps(inputs):
    """Build the 8 per-core input maps from the full-problem input dict."""
    bf = ml_dtypes.bfloat16
    f32 = np.float32

    def host(name):
        return np.asarray(inputs[name], dtype=f32)

    # fold the subln (ln_g/ln_b) into Wp/bp and the ffn_ln (ffn_g/ffn_b)
    # into W2/b2 — the kernel defers those norms into the matmuls and only
    # applies (x-mean)*rstd
    wp_eff = host("ln_g")[:, None] * host("Wp")
    bp_eff = host("bp") + host("ln_b") @ host("Wp")
    w2_eff = host("ffn_g")[:, None] * host("W2")
    b2_eff = host("b2") + host("ffn_b") @ host("W2")
    wp_bf = wp_eff.astype(bf)
    w2_bf = w2_eff.astype(bf)
    shared = dict(
        wq=np.ascontiguousarray(
            host("Wq").reshape(KT, 128, D).astype(bf).transpose(1, 0, 2)),
        wk=np.ascontiguousarray(
            host("Wk").reshape(KT, 128, D).astype(bf).transpose(1, 0, 2)),
        wv=np.ascontiguousarray(
            host("Wv").reshape(KT, 128, D).astype(bf).transpose(1, 0, 2)),
        wp=np.ascontiguousarray(
            wp_bf.reshape(KT, 128, D).transpose(1, 0, 2)),
        wpsum=wp_bf.astype(np.float32).sum(0).astype(f32),
        w1=np.ascontiguousarray(
            host("W1").reshape(KT, 128, HT, 128).transpose(2, 1, 0, 3)
        ).astype(bf),
        w2=np.ascontiguousarray(
            w2_bf.reshape(HT, 128, DT, 128).transpose(2, 1, 0, 3)),
        w2sum=w2_bf.astype(np.float32).sum(0).astype(f32),
        n1_g=host("n1_g"), n1_b=host("n1_b"),
        n3_g=host("n3_g"), n3_b=host("n3_b"),
        bq=host("bq"), bv=host("bv"),
        ln_g=host("ln_g"), ln_b=host("ln_b"),
        bp=bp_eff.astype(f32),
        n2_g=host("n2_g"), n2_b=host("n2_b"),
        b1=host("b1"), ffn_g=host("ffn_g"), ffn_b=host("ffn_b"),
        b2=b2_eff.astype(f32),
        gate=host("gate").reshape(H),
    )
    x = host("x")
    xt = host("x_text")
    in_maps = []
    for b in range(B):
        m = dict(shared)
        # device consumes feature-major, partition-major inputs
        m["x"] = np.ascontiguousarray(
            x[b].T.reshape(DT, 128, N).transpose(1, 0, 2)).astype(bf)
        m["x_text"] = np.ascontiguousarray(
            xt[b].T.reshape(DT, 128, PT).transpose(1, 0, 2)).astype(bf)
        in_maps.append(m)
    return in_maps


def unpack_y(y):
    """Device output is feature-major [128, DT, NTOK]; back to [NTOK, D]."""
    y = np.asarray(y)
    return np.transpose(y, (2, 1, 0)).reshape(NTOK, D)


def kernel(**inputs) -> np.ndarray:
    from concourse.bass_utils import run_bass_kernel_spmd

    nc = get_program(triv=compute_triv(inputs))
    in_maps = make_in_maps(inputs)
    res = run_bass_kernel_spmd(nc, in_maps, list(range(B)))
    out = np.stack([unpack_y(res.results[b]["y"]) for b in range(B)], axis=0)
    return out.astype(np.float32)

